# revision 21
# baseline (speedup 1.0000x reference)
"""Trainium2 Bass kernel for a transformer decoder layer (self-attn + cross-attn + FFN).

Sharding: 8-way tensor parallel over heads for both attentions (2 heads/core);
token-sharded for wo projections, layernorms and FFN with each core owning 256
tokens of EACH batch. Head<->token redistribution uses six half-size (per-batch)
AllToAll collectives. The two batches are independent through the whole layer,
so the schedule is batch-pipelined: every collective flies while the other
batch computes (attn1(b1) covers a2a1(b0); wo1/q2(b1) and wo2+FFN-w1(b0) run as
PE filler work inside the ACT-bound cross-attention phases; etc.).

All matmuls run in bf16 with fp32 PSUM accumulation. Attention keeps the
[feature, token] (transposed) layout throughout: scoresT uses kT-chunk
stationary x qT moving, probs come out as PT[ki, qi] which feeds attnV
directly with V-natural (+ones column) stationary, producing attn^T and the
softmax denominator in one accumulation group. Normalization happens via a
reciprocal row broadcast with a rank-1 fp32r matmul, off the critical path.
The cross-attention padding mask is folded into V by zeroing masked key rows
(incl. the ones column), which removes them from output and denominator.

LayerNorm rstd is computed as exp(scale*bits(var+eps)+bias) (the exponent-bits
log approximation folded into ACT's free affine) polished by two Newton
iterations on DVE — Exp is the only ACT table function in the whole kernel,
avoiding the ~1.3us-per-switch activation-table ping-pong between Exp and Ln.
"""

import sys

TRN_REPO = "/opt/trn_rl_repo"
if TRN_REPO not in sys.path:
    sys.path.insert(0, TRN_REPO)

import numpy as np
import ml_dtypes

D_MODEL = 1024
N_HEADS = 16
DFF = 4096
B, S = 2, 2048
EPS = 1e-6
DEPTH = D_MODEL // N_HEADS  # 64

NCORES = 8
HPC = N_HEADS // NCORES     # heads per core = 2
TOK = B * S                 # 4096 flattened tokens
SB = S                      # tokens per batch = 2048
TB = SB // NCORES           # tokens per core per batch = 256
KD = D_MODEL // 128         # 8 contraction chunks over d_model
FC = DFF // 128             # 32 chunks over dff
NBT = SB // 512             # 4 q-tiles per batch
NBC = SB // 128             # 16 ki-chunks per batch

BF = ml_dtypes.bfloat16

# rsqrt-via-exp-bits constants: rsqrt(v) ~= exp(RS_SCALE*float(bits(v)) + RS_BIAS)
_LN2 = float(np.log(2.0))
RS_SCALE = -0.5 * _LN2 / (1 << 23)
RS_BIAS = 0.5 * _LN2 * (127 + 0.0430357)

_PROG_CACHE = {}


def _build_program(self_blocks, n_ctiles):
    """Emit the SPMD Bass program (same program on all 8 cores)."""
    import concourse.bacc as bacc
    import concourse.mybir as mybir
    from concourse import tile

    F32 = mybir.dt.float32
    F32R = mybir.dt.float32r
    I32 = mybir.dt.int32
    BF16 = mybir.dt.bfloat16
    EXP = mybir.ActivationFunctionType.Exp
    ADD = mybir.AluOpType.add
    MULT = mybir.AluOpType.mult
    SUB = mybir.AluOpType.subtract
    MAX = mybir.AluOpType.max

    nc = bacc.Bacc("TRN2", target_bir_lowering=False, debug=False,
                   num_devices=NCORES)

    def din(name, shape, dt=BF16):
        return nc.dram_tensor(name, shape, dt, kind="ExternalInput")

    xT_d = din("xT", [D_MODEL, TOK])
    encT_d = din("encT", [D_MODEL, TOK])
    xown_d = din("x_own", [2 * TB, D_MODEL], F32)   # bo1 pre-folded on host
    wq1_d = din("wq1", [128, KD * 128])
    wk1_d = din("wk1", [128, KD * 128])
    wv1_d = din("wv1", [128, KD * 128])
    bq1_d = din("bq1", [128, 1], F32)
    bk1_d = din("bk1", [128, 1], F32)
    bv1_d = din("bv1", [128, 1], F32)
    wo1_d = din("wo1", [128, KD * 1024])
    wq2_d = din("wq2", [128, KD * KD * 128])
    bq2_d = din("bq2", [128, KD], F32)
    wk2_d = din("wk2", [128, KD * 128])
    wv2_d = din("wv2", [128, KD * 128])
    bk2_d = din("bk2", [128, 1], F32)
    bv2_d = din("bv2", [128, 1], F32)
    wo2_d = din("wo2", [128, KD * 1024])
    bo2_d = din("bo2", [128, 1024], F32)   # pre-broadcast
    w1_d = din("w1", [128, FC * KD * 128])
    b1f_d = din("b1", [128, FC], F32)      # per-partition per-chunk
    w2_d = din("w2", [128, FC * 1024])
    b2_d = din("b2", [128, 1024], F32)     # pre-broadcast
    ident_d = din("ident", [128, 128])
    cm_d = din("cmask", [128, max(n_ctiles, 1) * 512])
    mb_d = din("mbias", [128, B * NBC], F32)
    out_d = nc.dram_tensor("out", [2 * TB, D_MODEL], F32, kind="ExternalOutput")

    CROSS_BLOCKS = {(t, c): 'full' for t in range(NBT) for c in range(NBC)}
    GROUPS = [list(range(NCORES))]
    GW = 2  # ki-chunks per merged exp group

    with tile.TileContext(nc) as tc:
      with tc.tile_pool(name="const", bufs=1) as constp, \
           tc.tile_pool(name="fbuf", bufs=1) as fbuf, \
           tc.tile_pool(name="lns", bufs=2) as lns, \
           tc.tile_pool(name="dram", bufs=1, space="DRAM") as dram, \
           tc.tile_pool(name="ps_aux", bufs=2, space="PSUM") as ps_aux:

        # ---- constants ----
        ones65 = constp.tile([1, 65], F32)
        nc.vector.memset(ones65[:], 1.0)
        rsb = constp.tile([128, 1], F32)
        nc.vector.memset(rsb[:], RS_BIAS)
        ident = constp.tile([128, 128], BF16)
        nc.sync.dma_start(out=ident[:], in_=ident_d[:])
        cm = constp.tile([128, max(n_ctiles, 1) * 512], BF16)
        nc.sync.dma_start(out=cm[:], in_=cm_d[:])
        mb = constp.tile([128, B * NBC], F32)
        nc.sync.dma_start(out=mb[:], in_=mb_d[:])

        # ---- persistent activations ----
        out1 = fbuf.tile([128, 4 * 1024], F32, tag="out1")
        out2 = fbuf.tile([128, 4 * 1024], F32, tag="out2")

        # ---- a2a dram buffers (per batch) ----
        bar_in = dram.tile([NCORES, 16], BF16)
        bar_out = dram.tile([NCORES, 16], BF16)
        a11i = [dram.tile([NCORES * 128, TB], BF16, name=f"a11i{b}")
                for b in range(B)]
        a11o = [dram.tile([NCORES * 128, TB], BF16, name=f"a11o{b}")
                for b in range(B)]
        a1qi = [dram.tile([NCORES * 128, TB], BF16, name=f"a1qi{b}")
                for b in range(B)]
        a1qo = [dram.tile([NCORES * 128, TB], BF16, name=f"a1qo{b}")
                for b in range(B)]
        a12i = [dram.tile([NCORES * 128, TB], BF16, name=f"a12i{b}")
                for b in range(B)]
        a12o = [dram.tile([NCORES * 128, TB], BF16, name=f"a12o{b}")
                for b in range(B)]

        # startup barrier: absorb cross-core launch skew here (overlapped
        # with the initial input DMAs) instead of inside the first real a2a
        nc.sync.dma_start(out=bar_in[:], in_=ident[0:NCORES, 0:16])
        nc.gpsimd.collective_compute(
            "AllToAll", mybir.AluOpType.bypass, replica_groups=GROUPS,
            ins=[bar_in.opt()], outs=[bar_out.opt()])

        def a2a(in_t, out_t):
            nc.gpsimd.collective_compute(
                "AllToAll", mybir.AluOpType.bypass, replica_groups=GROUPS,
                ins=[in_t.opt()], outs=[out_t.opt()])

        # ---------------- shared helpers ----------------
        def proj_tile(dst, w_sb, bias, src_sb, jj):
            # dst[:, 512*jj:...] = (w_chunk^T @ srcT)[dcol, tok] + bias.
            # src_sb is one batch half [128, KD*SB]; jj in 0..3.
            ps = ps_aux.tile([128, 512], F32, tag="psaux", name="psp")
            for k in range(KD):
                nc.tensor.matmul(
                    ps[:],
                    lhsT=w_sb[:, 128 * k:128 * (k + 1)],
                    rhs=src_sb[:, k * SB + 512 * jj:k * SB + 512 * jj + 512],
                    start=(k == 0), stop=(k == KD - 1))
            nc.vector.tensor_scalar_add(dst[:, 512 * jj:512 * (jj + 1)],
                                        ps[:], bias[:])

        def vaug_ones(vaug_sb, key_mask_col=None):
            # write only the 65th (ones/denominator) column of each group:
            # the 0:64 columns are fully overwritten by the chunk transposes
            v = vaug_sb.rearrange("p (h c d) -> p h c d", h=HPC, c=NBC)
            if key_mask_col is None:
                nc.vector.memset(v[:, :, :, 64:65], 1.0)
            else:
                for h in range(HPC):
                    nc.vector.tensor_copy(
                        v[:, h, :, 64],
                        mb[:, key_mask_col:key_mask_col + NBC])

        def vaug_chunk_tr(vT_sb, vaug_sb, c, key_mask_col=None):
            # PE-transpose V chunk c ([128 (h,d), 128 tok] -> [128 tok,
            # (h,d)]) and scatter into vaug's per-head 65-column groups;
            # key_mask zeroes dropped keys (per-partition scalar, fused
            # into the scatter copy).
            ptr = ps_aux.tile([128, 128], BF16, tag="psaux", name="ptr")
            nc.tensor.transpose(ptr[:], vT_sb[:, 128 * c:128 * (c + 1)],
                                ident[:])
            dst = vaug_sb.rearrange("p (h c d) -> p h c d", h=HPC, c=NBC)
            src = ptr.rearrange("p (h d) -> p h d", h=HPC)
            if key_mask_col is None:
                nc.vector.tensor_copy(dst[:, :, c, 0:64], src)
            else:
                nc.vector.tensor_scalar_mul(
                    dst[:, :, c, 0:64], src,
                    mb[:, key_mask_col + c:key_mask_col + c + 1])

        def vaug_slice(vaug_sb, h, c):
            base = 65 * (NBC * h + c)
            return vaug_sb[:, base:base + 65]

        def rsqrt_of(veps):
            # [128, 1] f32 -> rstd = (veps)^-0.5 via exp-bits seed (~1.5%
            # err) + one fused Newton step (3 DVE ops, ~3e-3 max err)
            bfi = lns.tile([128, 1], F32, tag="bfi")
            nc.vector.tensor_copy(bfi[:], veps.bitcast(I32))
            y = lns.tile([128, 1], F32, tag="rsy")
            nc.scalar.activation(y[:], bfi[:], EXP, scale=RS_SCALE,
                                 bias=rsb[:])
            t2 = lns.tile([128, 1], F32, tag="rst")
            nc.vector.scalar_tensor_tensor(t2[:], veps, y[:], y[:],
                                           op0=MULT, op1=MULT)
            c2 = lns.tile([128, 1], F32, tag="rsc")
            nc.vector.tensor_scalar(c2[:], t2[:], -0.5, 1.5,
                                    op0=MULT, op1=ADD)
            yn = lns.tile([128, 1], F32, tag="rsn")
            nc.vector.tensor_scalar(yn[:], c2[:], y[:], 0.0,
                                    op0=MULT, op1=ADD)
            return yn

        def ln_inplace(pre, dst):
            bnst = lns.tile([128, 12], F32, tag="bnst")
            nc.vector.bn_stats(bnst[:, 0:6], pre[:, 0:512])
            nc.vector.bn_stats(bnst[:, 6:12], pre[:, 512:1024])
            stats = lns.tile([128, 2], F32, tag="stats")
            nc.vector.bn_aggr(stats[:], bnst[:])
            veps = lns.tile([128, 1], F32, tag="veps")
            nc.vector.tensor_scalar_add(veps[:], stats[:, 1:2], EPS)
            rstd = rsqrt_of(veps[:])
            nc.vector.tensor_scalar(dst[:], pre[:], stats[:, 0:1], rstd[:],
                                    op0=SUB, op1=MULT)

        def attention(pools, QT_sb, KT_sb, vaug_sb, stage_sb, blocks,
                      fillers=None, filler_delay=0):
            # Software-pipelined attention over ONE batch. Per work unit
            # (t, chunk-group): scores for GW ki-chunks of both heads land
            # in per-head psums (h0 on PE rows 0-63, h1 on rows 64-127 —
            # row tiling), one Exp per head covers the group. The attnV
            # matmuls of the PREVIOUS unit are emitted after this unit's
            # scores so the PE queue never stalls on the exp; softmax
            # divisions are delayed one more unit.
            ps_s, ps_o, ptp, smalls = pools
            units = []
            for t in range(NBT):
                clist = [c for c in range(NBC) if (t, c) in blocks]
                groups = [clist[i:i + GW] for i in range(0, len(clist), GW)]
                for gi, grp in enumerate(groups):
                    units.append((t, grp, gi == 0, gi == len(groups) - 1))

            po = {}          # live accumulation psums, per head
            pending = None   # (unit, pt4 dict)
            div_q = []       # (t, po) awaiting division emission

            def emit_attnv(unit, pt4):
                t, grp, isfirst, islast = unit
                if isfirst:
                    for h in range(HPC):
                        po[h] = ps_o.tile([65, 512], F32, tag=f"po{h}",
                                          name=f"po{h}")
                for ci, c in enumerate(grp):
                    kind = blocks[(t, c)]
                    for h in range(HPC):
                        rhs = pt4[:, 512 * (GW * h + ci):
                                  512 * (GW * h + ci) + 512]
                        if kind != 'full':
                            idx = kind[1]
                            nc.vector.tensor_tensor(
                                rhs, rhs, cm[:, 512 * idx:512 * (idx + 1)],
                                op=MULT)
                        nc.tensor.matmul(
                            po[h][:], lhsT=vaug_slice(vaug_sb, h, c),
                            rhs=rhs, start=(isfirst and ci == 0),
                            stop=(islast and ci == len(grp) - 1))
                if islast:
                    div_q.append((t, dict(po)))

            def emit_division():
                t, po_t = div_q.pop(0)
                for h in range(HPC):
                    recip = smalls.tile([1, 512], F32R, tag="recip")
                    with nc.allow_low_precision(reason="softmax recip row"):
                        nc.vector.reciprocal(recip[:], po_t[h][64:65, :])
                    pb = ps_aux.tile([65, 512], F32, tag="psaux", name="pb")
                    nc.tensor.matmul(pb[:], lhsT=ones65[:].bitcast(F32R),
                                     rhs=recip[:], start=True, stop=True)
                    pbsb = smalls.tile([64, 512], F32, tag="pbsb")
                    nc.vector.tensor_copy(pbsb[:], pb[0:64, :])
                    nc.vector.tensor_tensor(
                        stage_sb[0:64,
                                 h * SB + 512 * t:h * SB + 512 * t + 512],
                        po_t[h][0:64, :], pbsb[:], op=MULT)

            fillers = list(fillers) if fillers else []
            unit_no = [0]

            def emit_warmer():
                # Keep PE array activity high: attention's half-array matmuls
                # don't generate enough PE activity for the HAM to ever
                # unthrottle the clock. Emit either a unit of real full-array
                # work (pipelined filler) or a dummy matmul.
                if fillers and unit_no[0] >= filler_delay:
                    fillers.pop(0)()
                    return
                pd = ps_aux.tile([128, 512], F32, tag="psaux", name="pd")
                nc.tensor.matmul(pd[:], lhsT=KT_sb[:, 0:128],
                                 rhs=KT_sb[:, 1024:1536],
                                 start=True, stop=True)

            for unit in units:
                t, grp, isfirst, islast = unit
                # both heads' score chunks share one 4-bank psum so a
                # single ACTIVATE covers the whole unit's exp
                ps4 = ps_s.tile([128, HPC * GW * 512], F32,
                                tag="ps4", name="ps4")
                for ci, c in enumerate(grp):
                    for h in range(HPC):
                        nc.tensor.matmul(
                            ps4[:, 512 * (GW * h + ci):
                                512 * (GW * h + ci) + 512],
                            lhsT=KT_sb[64 * h:64 * (h + 1),
                                       128 * c:128 * c + 128],
                            rhs=QT_sb[64 * h:64 * (h + 1),
                                      512 * t:512 * t + 512],
                            start=True, stop=True)
                pt4 = ptp.tile([128, HPC * GW * 512], BF16,
                               tag="pt4", name="pt4")
                nc.scalar.activation(pt4[:], ps4[:], EXP, scale=0.125)
                emit_warmer()
                if pending is not None:
                    emit_attnv(*pending)
                    if div_q and len(div_q) > 1:
                        emit_division()
                emit_warmer()
                pending = (unit, pt4)
                unit_no[0] += 1
            emit_attnv(*pending)
            while div_q:
                emit_division()
            for f in fillers:
                f()

        def stage_to_a2a(stage_sb, a2a_in_t):
            for h in range(HPC):
                o = a2a_in_t.rearrange("(j r) s -> r j s", r=128)
                nc.sync.dma_start(
                    out=o[64 * h:64 * (h + 1)],
                    in_=stage_sb.rearrange("r (h j s) -> r h j s",
                                           h=HPC, j=NCORES)[:, h])

        def wo_ln_closures(pool, pget, at_sb, wo_sb, resid_of, outm_of,
                           outT_sb, post=None):
            # Returns filler closures computing, for m in {0,1}:
            # outm_of(m) = LN(resid_of(m) + at^T @ wo), plus the bf16
            # PE-transpose of the LN output into outT_sb, then post().
            # Order: all 4 matmul closures first, then the two LN chains
            # (so their serial DVE/ACT latency overlaps PE work), then the
            # PE transposes.
            closures = []
            pre_box = {}
            obf_box = {}

            def mk_mm(m, eh):
                def f():
                    if pget is not None:
                        pget()
                    if m not in pre_box:
                        pre_box[m] = pool.tile([128, 1024], F32, tag="pre",
                                               name=f"pre{m}")
                    pre = pre_box[m]
                    pw = ps_aux.tile([128, 512], F32, tag="psaux", name="pw")
                    for dc in range(KD):
                        nc.tensor.matmul(
                            pw[:],
                            lhsT=at_sb[:, TB * dc + 128 * m:
                                       TB * dc + 128 * m + 128],
                            rhs=wo_sb[:, 1024 * dc + 512 * eh:
                                      1024 * dc + 512 * eh + 512],
                            start=(dc == 0), stop=(dc == KD - 1))
                    nc.vector.tensor_tensor(
                        pre[:, 512 * eh:512 * (eh + 1)], pw[:],
                        resid_of(m)[:, 512 * eh:512 * (eh + 1)], op=ADD)
                return f

            def mk_ln(m):
                def f():
                    pre = pre_box.pop(m)
                    outm = outm_of(m)
                    ln_inplace(pre, outm)
                    obf = pool.tile([128, 1024], BF16, tag="obf",
                                    name=f"obf{m}")
                    nc.vector.tensor_copy(obf[:], outm)
                    obf_box[m] = obf
                return f

            def mk_tr(m):
                def f():
                    obf = obf_box.pop(m)
                    for j in range(KD):
                        ptr = ps_aux.tile([128, 128], BF16, tag="psaux",
                                          name="ptr2")
                        nc.tensor.transpose(ptr[:],
                                            obf[:, 128 * j:128 * (j + 1)],
                                            ident[:])
                        nc.vector.tensor_copy(
                            outT_sb[:, TB * j + 128 * m:
                                    TB * j + 128 * m + 128], ptr[:])
                    if post is not None and m == 1:
                        post()
                return f

            for m in range(2):
                closures.append(mk_mm(m, 0))
                closures.append(mk_mm(m, 1))
            for m in range(2):
                closures.append(mk_ln(m))
            for m in range(2):
                closures.append(mk_tr(m))
            return closures

        def q2_closures(pool, wq2_sb, bq2_sb, outT_sb, qt2_box):
            closures = []

            def mk(j):
                def f():
                    if "t" not in qt2_box:
                        qt2_box["t"] = pool.tile([128, KD * TB], BF16,
                                                 tag="qt2", name="qt2")
                    qt2 = qt2_box["t"]
                    pq = ps_aux.tile([128, TB], F32, tag="psaux", name="pq")
                    for k in range(KD):
                        nc.tensor.matmul(
                            pq[:],
                            lhsT=wq2_sb[:, 1024 * j + 128 * k:
                                        1024 * j + 128 * k + 128],
                            rhs=outT_sb[:, TB * k:TB * (k + 1)],
                            start=(k == 0), stop=(k == KD - 1))
                    nc.vector.tensor_scalar_add(
                        qt2[:, TB * j:TB * (j + 1)], pq[:],
                        bq2_sb[:, j:j + 1])
                return f

            for j in range(KD):
                closures.append(mk(j))
            return closures

        def ffn_w1_closures(pool, w1s_pool, b1f_sb, outT_sb, hT_box, tag):
            closures = []

            def mk(fc):
                def f():
                    if "t" not in hT_box:
                        hT_box["t"] = pool.tile([128, FC * TB], BF16,
                                                tag=tag, name=tag)
                    hT = hT_box["t"]
                    w1t = w1s_pool.tile([128, KD * 128], BF16, tag="w1s")
                    nc.sync.dma_start(out=w1t[:],
                                      in_=w1_d[:, 1024 * fc:1024 * (fc + 1)])
                    ph = ps_aux.tile([128, TB], F32, tag="psaux", name="ph")
                    for k in range(KD):
                        nc.tensor.matmul(
                            ph[:],
                            lhsT=w1t[:, 128 * k:128 * (k + 1)],
                            rhs=outT_sb[:, TB * k:TB * (k + 1)],
                            start=(k == 0), stop=(k == KD - 1))
                    nc.vector.tensor_scalar(hT[:, TB * fc:TB * (fc + 1)],
                                            ph[:], b1f_sb[:, fc:fc + 1], 0.0,
                                            op0=ADD, op1=MAX)
                return f

            for fc in range(FC):
                closures.append(mk(fc))
            return closures

        out1v = out1.rearrange("p (m e) -> p m e", m=4)
        out2v = out2.rearrange("p (m e) -> p m e", m=4)

        # =====================================================================
        # p3k: cross-attn K/V/Q tensors that survive into attn2 phases
        with tc.tile_pool(name="p3k", bufs=1) as p3k:
            KT2 = [p3k.tile([128, SB], BF16, tag=f"KT2{b}", name=f"KT2{b}")
                   for b in range(B)]
            vaug2 = [p3k.tile([128, HPC * NBC * 65], BF16, tag=f"vaug2{b}",
                              name=f"vaug2{b}") for b in range(B)]
            QT2 = [p3k.tile([128, SB], BF16, tag=f"QT2{b}", name=f"QT2{b}")
                   for b in range(B)]

            # ============ phases 1-2: QKV1 + self attention ==================
            with tc.tile_pool(name="pA", bufs=1) as pA, \
                 tc.tile_pool(name="p12s", bufs=2) as p12s:
                wq1 = pA.tile([128, KD * 128], BF16, tag="wq1")
                wk1 = pA.tile([128, KD * 128], BF16, tag="wk1")
                wv1 = pA.tile([128, KD * 128], BF16, tag="wv1")
                nc.sync.dma_start(out=wq1[:], in_=wq1_d[:])
                nc.sync.dma_start(out=wk1[:], in_=wk1_d[:])
                nc.sync.dma_start(out=wv1[:], in_=wv1_d[:])
                bq1 = pA.tile([128, 1], F32, tag="bq1")
                bk1 = pA.tile([128, 1], F32, tag="bk1")
                bv1 = pA.tile([128, 1], F32, tag="bv1")
                nc.sync.dma_start(out=bq1[:], in_=bq1_d[:])
                nc.sync.dma_start(out=bk1[:], in_=bk1_d[:])
                nc.sync.dma_start(out=bv1[:], in_=bv1_d[:])
                wk2 = pA.tile([128, KD * 128], BF16, tag="wk2")
                wv2 = pA.tile([128, KD * 128], BF16, tag="wv2")
                nc.sync.dma_start(out=wk2[:], in_=wk2_d[:])
                nc.sync.dma_start(out=wv2[:], in_=wv2_d[:])
                bk2 = pA.tile([128, 1], F32, tag="bk2")
                bv2 = pA.tile([128, 1], F32, tag="bv2")
                nc.sync.dma_start(out=bk2[:], in_=bk2_d[:])
                nc.sync.dma_start(out=bv2[:], in_=bv2_d[:])

                xTd = xT_d.rearrange("(k p) t -> p k t", p=128)
                encTd = encT_d.rearrange("(k p) t -> p k t", p=128)

                def load_half(tl, src_view, b):
                    # chunked per-512-token-tile DMA of one batch half
                    v = tl.rearrange("p (k t) -> p k t", k=KD)
                    for jj in range(4):
                        nc.sync.dma_start(
                            out=v[:, :, 512 * jj:512 * (jj + 1)],
                            in_=src_view[:, :, SB * b + 512 * jj:
                                         SB * b + 512 * jj + 512])

                # xT halves share one 32KB slot; the b1 half's DMA waits
                # for the b0 projections to finish reading the slot.
                xTh0 = pA.tile([128, KD * SB], BF16, tag="xTh", name="xTh0")
                load_half(xTh0, xTd, 0)
                encTh0 = pA.tile([128, KD * SB], BF16, tag="encTh",
                                 name="encTh0")
                load_half(encTh0, encTd, 0)

                QT = [pA.tile([128, SB], BF16, tag=f"QT{b}", name=f"QT{b}")
                      for b in range(B)]
                KT = [pA.tile([128, SB], BF16, tag=f"KT{b}", name=f"KT{b}")
                      for b in range(B)]
                vT1 = pA.tile([128, SB], BF16, tag="vTs", name="vT1")
                vaug1 = [pA.tile([128, HPC * NBC * 65], BF16,
                                 tag=f"vaug1{b}", name=f"vaug1{b}")
                         for b in range(B)]
                stage1 = [pA.tile([64, HPC * SB], BF16, tag=f"stage1{b}",
                                  name=f"stage1{b}") for b in range(B)]


                # QKV1(b0) directly
                for jj in range(4):
                    proj_tile(vT1, wv1, bv1, xTh0, jj)
                nc.vector.memset(vaug1[0][:], 1.0)
                for c in range(NBC):
                    vaug_chunk_tr(vT1, vaug1[0], c)
                for jj in range(4):
                    proj_tile(QT[0], wq1, bq1, xTh0, jj)
                for jj in range(4):
                    proj_tile(KT[0], wk1, bk1, xTh0, jj)

                # xT b1 half: slot reuse waits for the QKV1(b0) reads, DMA
                # flies while attn1(b0) computes
                xTh1 = pA.tile([128, KD * SB], BF16, tag="xTh", name="xTh1")
                load_half(xTh1, xTd, 1)

                # attn1(b0) with QKV1(b1) as fillers
                f_a1b0 = []
                for jj in range(4):
                    f_a1b0.append(lambda jj=jj: proj_tile(vT1, wv1, bv1,
                                                          xTh1, jj))
                f_a1b0.append(lambda: nc.vector.memset(vaug1[1][:], 1.0))
                for c0 in range(0, NBC, 4):
                    def fv(c0=c0):
                        for c in range(c0, c0 + 4):
                            vaug_chunk_tr(vT1, vaug1[1], c)
                    f_a1b0.append(fv)
                for jj in range(4):
                    f_a1b0.append(lambda jj=jj: proj_tile(QT[1], wq1, bq1,
                                                          xTh1, jj))
                for jj in range(4):
                    f_a1b0.append(lambda jj=jj: proj_tile(KT[1], wk1, bk1,
                                                          xTh1, jj))

                with tc.tile_pool(name="ps_s1", bufs=1, space="PSUM") as ps_s, \
                     tc.tile_pool(name="ps_o1", bufs=1, space="PSUM") as ps_o, \
                     tc.tile_pool(name="pt1", bufs=2) as ptp:
                    attention((ps_s, ps_o, ptp, p12s),
                              QT[0], KT[0], vaug1[0], stage1[0], self_blocks,
                              fillers=f_a1b0, filler_delay=3)
                stage_to_a2a(stage1[0], a11i[0])
                a2a(a11i[0], a11o[0])

                # vT2 shares vT1's slot: vT1 dies once vaug1[1] is built
                # (a filler of attn1(b0), emitted above)
                vT2 = pA.tile([128, SB], BF16, tag="vTs", name="vT2")

                # attn1(b1) with cross K/V prep as fillers; the encT b1
                # half-load is itself a filler (its slot-reuse waits for
                # all b0 readers, which precede it in the list)
                ench = {0: encTh0}

                def load_ench1():
                    ench[1] = pA.tile([128, KD * SB], BF16, tag="encTh",
                                      name="encTh1")
                    load_half(ench[1], encTd, 1)

                f_a1b1 = []
                for b in range(B):
                    if b == 1:
                        f_a1b1.append(load_ench1)
                    for jj in range(4):
                        f_a1b1.append(lambda b=b, jj=jj: proj_tile(
                            vT2, wv2, bv2, ench[b], jj))
                    f_a1b1.append(lambda b=b: nc.vector.memset(
                        vaug2[b][:], 1.0))
                    for c0 in range(0, NBC, 4):
                        def fv2(b=b, c0=c0):
                            for c in range(c0, c0 + 4):
                                vaug_chunk_tr(vT2, vaug2[b], c,
                                              key_mask_col=NBC * b)
                        f_a1b1.append(fv2)
                    for jj in range(4):
                        f_a1b1.append(lambda b=b, jj=jj: proj_tile(
                            KT2[b], wk2, bk2, ench[b], jj))

                with tc.tile_pool(name="ps_s2", bufs=1, space="PSUM") as ps_s, \
                     tc.tile_pool(name="ps_o2", bufs=1, space="PSUM") as ps_o, \
                     tc.tile_pool(name="pt2", bufs=2) as ptp:
                    attention((ps_s, ps_o, ptp, p12s),
                              QT[1], KT[1], vaug1[1], stage1[1], self_blocks,
                              fillers=f_a1b1, filler_delay=0)
                stage_to_a2a(stage1[1], a11i[1])
                # NOTE: a2a1(b1)'s trigger is deferred until after a2aq(b0)
                # so attn2(b0) can start as early as possible; nothing needs
                # a2a1(b1) before the W1(b1) fillers inside attn2(b0).
            # pA closed

            # ============ phase 3: wo1+LN1+q2 (b0 direct, b1 as fillers) ====
            with tc.tile_pool(name="pW1", bufs=1) as pW1, \
                 tc.tile_pool(name="pW1s", bufs=2) as pW1s:
                wo1 = pW1.tile([128, KD * 1024], BF16, tag="wo1")
                nc.sync.dma_start(out=wo1[:], in_=wo1_d[:])
                wq2 = pW1.tile([128, KD * KD * 128], BF16, tag="wq2")
                nc.sync.dma_start(out=wq2[:], in_=wq2_d[:])
                bq2 = pW1.tile([128, KD], F32, tag="bq2")
                nc.sync.dma_start(out=bq2[:], in_=bq2_d[:])
                bo2 = pW1.tile([128, 1024], F32, tag="bo2")
                nc.sync.dma_start(out=bo2[:], in_=bo2_d[:])
                outT1 = [pW1.tile([128, KD * TB], BF16, tag=f"outT1{b}",
                                  name=f"outT1{b}") for b in range(B)]
                xow = pW1.tile([128, 4 * 1024], F32, tag="xow")
                nc.sync.dma_start(
                    out=xow.rearrange("p (m e) -> p m e", m=4),
                    in_=xown_d.rearrange("(m p) e -> p m e", p=128))
                xowv = xow.rearrange("p (m e) -> p m e", m=4)
                at1 = [pW1.tile([128, KD * TB], BF16, tag=f"at1{b}",
                                name=f"at1{b}") for b in range(B)]
                at1_loaded = [False, False]

                def load_at1(b):
                    if not at1_loaded[b]:
                        at1_loaded[b] = True
                        nc.sync.dma_start(
                            out=at1[b].rearrange("p (dc s) -> p dc s", dc=KD),
                            in_=a11o[b].rearrange("(dc p) s -> p dc s",
                                                  p=128))

                qt2_box = [{}, {}]

                def post_bias1(b):
                    # fold bo2 into out1 (after outT1 transpose, before wo2)
                    for m in range(2):
                        mm = 2 * b + m
                        nc.vector.tensor_tensor(
                            out1v[:, mm], out1v[:, mm], bo2[:], op=ADD)

                def finish_q2(b):
                    qt2 = qt2_box[b]["t"]
                    nc.sync.dma_start(
                        out=a1qi[b].rearrange("(j p) s -> p j s", p=128),
                        in_=qt2.rearrange("p (j s) -> p j s", j=KD))
                    a2a(a1qi[b], a1qo[b])
                    nc.sync.dma_start(
                        out=QT2[b].rearrange("p (i s) -> p i s", i=NCORES),
                        in_=a1qo[b].rearrange("(i p) s -> p i s", p=128))

                # b0 direct
                load_at1(0)
                w1cl = wo_ln_closures(
                    pW1s, None, at1[0], wo1,
                    resid_of=lambda m: xowv[:, m],
                    outm_of=lambda m: out1v[:, m],
                    outT_sb=outT1[0], post=lambda: post_bias1(0))
                for f in w1cl:
                    f()
                for f in q2_closures(pW1, wq2, bq2, outT1[0], qt2_box[0]):
                    f()
                finish_q2(0)
                # trigger a2a1(b1) only now: the cc stream is serial, and
                # a2aq(b0) gates attn2(b0) while a2a1(b1) is only needed by
                # the W1(b1) fillers ~10 units into attn2(b0)
                a2a(a11i[1], a11o[1])

                # attn2(b0) with W1(b1) + q2(b1) as fillers
                f_a2b0 = wo_ln_closures(
                    pW1s, lambda: load_at1(1), at1[1], wo1,
                    resid_of=lambda m: xowv[:, 2 + m],
                    outm_of=lambda m: out1v[:, 2 + m],
                    outT_sb=outT1[1], post=lambda: post_bias1(1))
                f_a2b0 += q2_closures(pW1, wq2, bq2, outT1[1], qt2_box[1])
                f_a2b0.append(lambda: finish_q2(1))

                with tc.tile_pool(name="pX", bufs=1) as pX, \
                     tc.tile_pool(name="ps_s3", bufs=1, space="PSUM") as ps_s, \
                     tc.tile_pool(name="ps_o3", bufs=1, space="PSUM") as ps_o, \
                     tc.tile_pool(name="pt3", bufs=2) as ptp:
                    stage2_b0 = pX.tile([64, HPC * SB], BF16, tag="stage2")
                    attention((ps_s, ps_o, ptp, pW1s),
                              QT2[0], KT2[0], vaug2[0], stage2_b0,
                              CROSS_BLOCKS, fillers=f_a2b0, filler_delay=10)
                    stage_to_a2a(stage2_b0, a12i[0])
                a2a(a12i[0], a12o[0])
            # pW1 closed

            # ============ phase 4: attn2(b1) + wo2/FFN-w1(b0) fillers =======
            with tc.tile_pool(name="p78", bufs=1) as p78, \
                 tc.tile_pool(name="p78s", bufs=2) as p78s, \
                 tc.tile_pool(name="w1str", bufs=3) as w1s_pool, \
                 tc.tile_pool(name="w2str", bufs=3) as w2s_pool:
                wo2 = p78.tile([128, KD * 1024], BF16, tag="wo2")
                nc.sync.dma_start(out=wo2[:], in_=wo2_d[:])
                b1f = p78.tile([128, FC], F32, tag="b1f")
                nc.sync.dma_start(out=b1f[:], in_=b1f_d[:])
                b2 = p78.tile([128, 1024], F32, tag="b2")
                nc.sync.dma_start(out=b2[:], in_=b2_d[:])

                at2 = [p78.tile([128, KD * TB], BF16, tag=f"at2{b}",
                                name=f"at2{b}") for b in range(B)]
                at2_loaded = [False, False]

                def load_at2(b):
                    if not at2_loaded[b]:
                        at2_loaded[b] = True
                        nc.sync.dma_start(
                            out=at2[b].rearrange("p (dc s) -> p dc s", dc=KD),
                            in_=a12o[b].rearrange("(dc p) s -> p dc s",
                                                  p=128))

                outT2 = [p78.tile([128, KD * TB], BF16, tag=f"outT2{b}",
                                  name=f"outT2{b}") for b in range(B)]
                hT_box = [{}, {}]

                def post_bias2(b):
                    # fold b2 into out2 (after outT2 transpose, before FFN w2)
                    for m in range(2):
                        mm = 2 * b + m
                        nc.vector.tensor_tensor(
                            out2v[:, mm], out2v[:, mm], b2[:], op=ADD)

                f_a2b1 = wo_ln_closures(
                    p78s, lambda: load_at2(0), at2[0], wo2,
                    resid_of=lambda m: out1v[:, m],
                    outm_of=lambda m: out2v[:, m],
                    outT_sb=outT2[0], post=lambda: post_bias2(0))
                f_a2b1 += ffn_w1_closures(p78, w1s_pool, b1f, outT2[0],
                                          hT_box[0], tag="hT")

                with tc.tile_pool(name="pX2", bufs=1) as pX2, \
                     tc.tile_pool(name="ps_s4", bufs=1, space="PSUM") as ps_s, \
                     tc.tile_pool(name="ps_o4", bufs=1, space="PSUM") as ps_o, \
                     tc.tile_pool(name="pt4", bufs=2) as ptp:
                    stage2_b1 = pX2.tile([64, HPC * SB], BF16, tag="stage2b")
                    attention((ps_s, ps_o, ptp, p78s),
                              QT2[1], KT2[1], vaug2[1], stage2_b1,
                              CROSS_BLOCKS, fillers=f_a2b1, filler_delay=8)
                    stage_to_a2a(stage2_b1, a12i[1])
                a2a(a12i[1], a12o[1])

                # ============ phase 5: FFN-w2+LN3(b0); then all of b1 =======
                def ffn_w2_ln3(b):
                    hT = hT_box[b]["t"]
                    with tc.tile_pool(name=f"ps_f{b}", bufs=1,
                                      space="PSUM") as ps_f:
                        py = {(m, eh): ps_f.tile([128, 512], F32,
                                                 tag=f"py{m}{eh}",
                                                 name=f"py{m}{eh}")
                              for m in range(2) for eh in range(2)}
                        for fc in range(FC):
                            w2t = w2s_pool.tile([128, 1024], BF16, tag="w2s")
                            nc.sync.dma_start(
                                out=w2t[:],
                                in_=w2_d[:, 1024 * fc:1024 * (fc + 1)])
                            for m in range(2):
                                for eh in range(2):
                                    nc.tensor.matmul(
                                        py[(m, eh)][:],
                                        lhsT=hT[:, TB * fc + 128 * m:
                                                TB * fc + 128 * m + 128],
                                        rhs=w2t[:, 512 * eh:512 * (eh + 1)],
                                        start=(fc == 0), stop=(fc == FC - 1))
                        pres = {}
                        for m in range(2):
                            mm = 2 * b + m
                            pre = p78s.tile([128, 1024], F32, tag="pref",
                                            name=f"pref{m}")
                            for eh in range(2):
                                nc.vector.tensor_tensor(
                                    pre[:, 512 * eh:512 * (eh + 1)],
                                    py[(m, eh)][:],
                                    out2v[:, mm, 512 * eh:512 * (eh + 1)],
                                    op=ADD)
                            pres[m] = pre
                        for m in range(2):
                            outf = p78s.tile([128, 1024], F32, tag="outf",
                                             name=f"outf{m}")
                            ln_inplace(pres[m], outf)
                            nc.sync.dma_start(
                                out=out_d[256 * b + 128 * m:
                                          256 * b + 128 * m + 128, :],
                                in_=outf[:])

                ffn_w2_ln3(0)

                # b1 drain: wo2(b1) + LN2(b1) + FFN(b1)
                w2cl = wo_ln_closures(
                    p78s, lambda: load_at2(1), at2[1], wo2,
                    resid_of=lambda m: out1v[:, 2 + m],
                    outm_of=lambda m: out2v[:, 2 + m],
                    outT_sb=outT2[1], post=lambda: post_bias2(1))
                for f in w2cl:
                    f()
                for f in ffn_w1_closures(p78, w1s_pool, b1f, outT2[1],
                                         hT_box[1], tag="hT"):
                    f()
                ffn_w2_ln3(1)

    nc.compile()
    return nc


def _to_bf(a):
    return np.ascontiguousarray(np.asarray(a, np.float32).astype(BF))


def _rechunk_k(w):
    """[K*128, M] -> [128, K*M] with col k*M + m = w[k*128 + p, m]."""
    K = w.shape[0] // 128
    M = w.shape[1]
    return np.ascontiguousarray(
        w.reshape(K, 128, M).transpose(1, 0, 2).reshape(128, K * M))


def _analyze_self_mask(mask):
    """mask [S, S] (1 = disallowed), orientation [q, k].

    Returns blocks dict (t, c) -> 'full' | ('tile', idx), list of unique
    multiplicative tiles [128, 512] (bf16), for a block grid over one batch.
    Blocks where everything is disallowed are omitted.
    """
    add = np.float32(-1e9) * np.asarray(mask, np.float32)
    mult = np.exp(add.T)  # [k, q] multiplicative
    blocks = {}
    tiles = []
    tile_ids = {}
    for t in range(NBT):
        for c in range(NBC):
            sub = mult[128 * c:128 * (c + 1), 512 * t:512 * (t + 1)]
            if not sub.any():
                continue
            if (sub == 1.0).all():
                blocks[(t, c)] = 'full'
                continue
            key = sub.tobytes()
            if key not in tile_ids:
                tile_ids[key] = len(tiles)
                tiles.append(sub.astype(BF))
            blocks[(t, c)] = ('tile', tile_ids[key])
    return blocks, tiles


def kernel(**inputs):
    from concourse.bass_utils import run_bass_kernel_spmd

    x = np.asarray(inputs["x"], np.float32)
    enc = np.asarray(inputs["enc_output"], np.float32)
    lam = np.asarray(inputs["look_ahead_mask"], np.float32)[0, 0]
    pad = np.asarray(inputs["padding_mask"], np.float32)  # [B,1,1,S]

    self_blocks, ctiles = _analyze_self_mask(lam)
    n_ctiles = len(ctiles)
    key = (tuple(sorted(self_blocks.items())), n_ctiles)
    if key not in _PROG_CACHE:
        _PROG_CACHE[key] = _build_program(self_blocks, n_ctiles)
    nc = _PROG_CACHE[key]

    # ---- shared (core-independent) host prep ----
    xf = x.reshape(TOK, D_MODEL)             # flattened batch-major tokens
    encf = enc.reshape(TOK, D_MODEL)
    xT = _to_bf(xf.T)                        # [1024, 4096]
    encT = _to_bf(encf.T)
    if n_ctiles:
        cmask = np.concatenate(ctiles, axis=1)
    else:
        cmask = np.zeros((128, 512), BF)
    cmask = np.ascontiguousarray(cmask)
    # cross-attn key-keep mask per enc token: [128, B*16], col b*16+c
    mb = np.exp(np.float32(-1e9) * pad[:, 0, 0, :]).reshape(B, NBC, 128)
    mb = np.ascontiguousarray(mb.transpose(2, 0, 1).reshape(128, B * NBC)
                              ).astype(np.float32)

    w1f = np.asarray(inputs["ffn_w1"], np.float32)
    # w1 stationary layout: [128, fc*1024 + k*128 + m] = w1[k*128+p, fc*128+m]
    w1r = w1f.reshape(KD, 128, FC, 128).transpose(1, 2, 0, 3)
    w1r = _to_bf(w1r.reshape(128, FC * KD * 128))
    w2r = _to_bf(_rechunk_k(np.asarray(inputs["ffn_w2"], np.float32)))
    # b1 per-partition per-chunk [128, FC]; b2/bo2 pre-broadcast [128, 1024]
    b1 = np.ascontiguousarray(
        np.asarray(inputs["ffn_b1"], np.float32).reshape(FC, 128).T)
    b2 = np.ascontiguousarray(np.broadcast_to(
        np.asarray(inputs["ffn_b2"], np.float32)[None, :], (128, 1024)))

    wo1r = _to_bf(_rechunk_k(np.asarray(inputs["mha1_wo"], np.float32)))
    wo2r = _to_bf(_rechunk_k(np.asarray(inputs["mha2_wo"], np.float32)))
    bo1 = np.asarray(inputs["mha1_bo"], np.float32)
    bo2 = np.ascontiguousarray(np.broadcast_to(
        np.asarray(inputs["mha2_bo"], np.float32)[None, :], (128, 1024)))
    ident = np.eye(128, dtype=np.float32).astype(BF)

    wq2_full = np.asarray(inputs["mha2_wq"], np.float32)
    # wq2 stationary layout: [128, j*1024 + k*128 + m] = wq2[k*128+p, j*128+m]
    wq2r = wq2_full.reshape(KD, 128, KD, 128).transpose(1, 2, 0, 3)
    wq2r = _to_bf(wq2r.reshape(128, KD * KD * 128))
    bq2 = np.asarray(inputs["mha2_bq"], np.float32).reshape(KD, 128)
    bq2 = np.ascontiguousarray(bq2.T).astype(np.float32)  # [128, KD]

    in_maps = []
    for j in range(NCORES):
        hs = slice(128 * j, 128 * (j + 1))       # this core's 2 heads' cols
        xo = np.concatenate([xf[TB * j:TB * (j + 1)],
                             xf[S + TB * j:S + TB * (j + 1)]], axis=0)
        xo = np.ascontiguousarray(xo + bo1[None, :])
        m = {
            "xT": xT, "encT": encT, "x_own": xo,
            "cmask": cmask, "mbias": mb,
            "w1": w1r, "b1": b1, "w2": w2r, "b2": b2,
            "wo1": wo1r, "wo2": wo2r, "bo2": bo2,
            "wq2": wq2r, "bq2": bq2, "ident": ident,
        }
        for pre, name in (("wq1", "mha1_wq"), ("wk1", "mha1_wk"),
                          ("wv1", "mha1_wv"), ("wk2", "mha2_wk"),
                          ("wv2", "mha2_wv")):
            w = np.asarray(inputs[name], np.float32)[:, hs]
            m[pre] = _to_bf(_rechunk_k(w))
        for pre, name in (("bq1", "mha1_bq"), ("bk1", "mha1_bk"),
                          ("bv1", "mha1_bv"), ("bk2", "mha2_bk"),
                          ("bv2", "mha2_bv")):
            bvec = np.asarray(inputs[name], np.float32)[hs]
            m[pre] = np.ascontiguousarray(bvec[:, None])
        in_maps.append(m)

    res = run_bass_kernel_spmd(nc, in_maps, list(range(NCORES)))
    out = np.empty((TOK, D_MODEL), np.float32)
    for j in range(NCORES):
        r = res.results[j]["out"]
        out[TB * j:TB * (j + 1)] = r[0:TB]
        out[S + TB * j:S + TB * (j + 1)] = r[TB:2 * TB]
    return out.reshape(B, S, D_MODEL)


# revision 24
# speedup vs baseline: 1.0594x; 1.0594x over previous
"""Trainium2 Bass kernel for a transformer decoder layer (self-attn + cross-attn + FFN).

Sharding: 8-way tensor parallel over heads for both attentions (2 heads/core);
token-sharded for wo projections, layernorms and FFN with each core owning 256
tokens of EACH batch. Head<->token redistribution uses six half-size (per-batch)
AllToAll collectives. The two batches are independent through the whole layer,
so the schedule is batch-pipelined: every collective flies while the other
batch computes (attn1(b1) covers a2a1(b0); wo1/q2(b1) and wo2+FFN-w1(b0) run as
PE filler work inside the ACT-bound cross-attention phases; etc.).

All matmuls run in bf16 with fp32 PSUM accumulation. Attention keeps the
[feature, token] (transposed) layout throughout: scoresT uses kT-chunk
stationary x qT moving, probs come out as PT[ki, qi] which feeds attnV
directly with V-natural (+ones column) stationary, producing attn^T and the
softmax denominator in one accumulation group. Normalization happens via a
reciprocal row broadcast with a rank-1 fp32r matmul, off the critical path.
The cross-attention padding mask is folded into V by zeroing masked key rows
(incl. the ones column), which removes them from output and denominator.

LayerNorm rstd is computed as exp(scale*bits(var+eps)+bias) (the exponent-bits
log approximation folded into ACT's free affine) polished by two Newton
iterations on DVE — Exp is the only ACT table function in the whole kernel,
avoiding the ~1.3us-per-switch activation-table ping-pong between Exp and Ln.
"""

import sys

TRN_REPO = "/opt/trn_rl_repo"
if TRN_REPO not in sys.path:
    sys.path.insert(0, TRN_REPO)

import numpy as np
import ml_dtypes

D_MODEL = 1024
N_HEADS = 16
DFF = 4096
B, S = 2, 2048
EPS = 1e-6
DEPTH = D_MODEL // N_HEADS  # 64

NCORES = 8
HPC = N_HEADS // NCORES     # heads per core = 2
TOK = B * S                 # 4096 flattened tokens
SB = S                      # tokens per batch = 2048
TB = SB // NCORES           # tokens per core per batch = 256
KD = D_MODEL // 128         # 8 contraction chunks over d_model
FC = DFF // 128             # 32 chunks over dff
NBT = SB // 512             # 4 q-tiles per batch
NBC = SB // 128             # 16 ki-chunks per batch

BF = ml_dtypes.bfloat16

# rsqrt-via-exp-bits constants: rsqrt(v) ~= exp(RS_SCALE*float(bits(v)) + RS_BIAS)
_LN2 = float(np.log(2.0))
RS_SCALE = -0.5 * _LN2 / (1 << 23)
RS_BIAS = 0.5 * _LN2 * (127 + 0.0430357)

_PROG_CACHE = {}


def _build_program(self_blocks, n_ctiles):
    """Emit the SPMD Bass program (same program on all 8 cores)."""
    import concourse.bacc as bacc
    import concourse.mybir as mybir
    from concourse import tile

    F32 = mybir.dt.float32
    F32R = mybir.dt.float32r
    I32 = mybir.dt.int32
    BF16 = mybir.dt.bfloat16
    EXP = mybir.ActivationFunctionType.Exp
    ADD = mybir.AluOpType.add
    MULT = mybir.AluOpType.mult
    SUB = mybir.AluOpType.subtract
    MAX = mybir.AluOpType.max

    nc = bacc.Bacc("TRN2", target_bir_lowering=False, debug=False,
                   num_devices=NCORES)

    def din(name, shape, dt=BF16):
        return nc.dram_tensor(name, shape, dt, kind="ExternalInput")

    xT_d = din("xT", [D_MODEL, TOK])
    encT_d = din("encT", [D_MODEL, TOK])
    xown_d = din("x_own", [2 * TB, D_MODEL], F32)   # bo1 pre-folded on host
    wq1_d = din("wq1", [128, KD * 128])
    wk1_d = din("wk1", [128, KD * 128])
    wv1_d = din("wv1", [128, KD * 128])
    bq1_d = din("bq1", [128, 1], F32)
    bk1_d = din("bk1", [128, 1], F32)
    bv1_d = din("bv1", [128, 1], F32)
    wo1_d = din("wo1", [128, KD * 1024])
    wq2_d = din("wq2", [128, KD * KD * 128])
    bq2_d = din("bq2", [128, KD], F32)
    wk2_d = din("wk2", [128, KD * 128])
    wv2_d = din("wv2", [128, KD * 128])
    bk2_d = din("bk2", [128, 1], F32)
    bv2_d = din("bv2", [128, 1], F32)
    wo2_d = din("wo2", [128, KD * 1024])
    bo2_d = din("bo2", [128, 1024], F32)   # pre-broadcast
    w1_d = din("w1", [128, FC * KD * 128])
    b1f_d = din("b1", [128, FC], F32)      # per-partition per-chunk
    w2_d = din("w2", [128, FC * 1024])
    b2_d = din("b2", [128, 1024], F32)     # pre-broadcast
    ident_d = din("ident", [128, 128])
    cm_d = din("cmask", [128, max(n_ctiles, 1) * 512])
    mb_d = din("mbias", [128, B * NBC], F32)
    out_d = nc.dram_tensor("out", [2 * TB, D_MODEL], F32, kind="ExternalOutput")

    CROSS_BLOCKS = {(t, c): 'full' for t in range(NBT) for c in range(NBC)}
    GROUPS = [list(range(NCORES))]
    GW = 2  # ki-chunks per merged exp group

    with tile.TileContext(nc) as tc:
      with tc.tile_pool(name="const", bufs=1) as constp, \
           tc.tile_pool(name="fbuf", bufs=1) as fbuf, \
           tc.tile_pool(name="lns", bufs=2) as lns, \
           tc.tile_pool(name="dram", bufs=1, space="DRAM") as dram, \
           tc.tile_pool(name="ps_aux", bufs=2, space="PSUM") as ps_aux:

        # ---- constants ----
        ones65 = constp.tile([1, 65], F32)
        nc.vector.memset(ones65[:], 1.0)
        rsb = constp.tile([128, 1], F32)
        nc.vector.memset(rsb[:], RS_BIAS)
        ident = constp.tile([128, 128], BF16)
        nc.sync.dma_start(out=ident[:], in_=ident_d[:])
        cm = constp.tile([128, max(n_ctiles, 1) * 512], BF16)
        nc.sync.dma_start(out=cm[:], in_=cm_d[:])
        mb = constp.tile([128, B * NBC], F32)
        nc.sync.dma_start(out=mb[:], in_=mb_d[:])

        # ---- persistent activations ----
        out1 = fbuf.tile([128, 4 * 1024], F32, tag="out1")
        out2 = fbuf.tile([128, 4 * 1024], F32, tag="out2")

        # ---- a2a dram buffers (per batch) ----
        bar_in = dram.tile([NCORES, 16], BF16)
        bar_out = dram.tile([NCORES, 16], BF16)
        a11i = [dram.tile([NCORES * 128, TB], BF16, name=f"a11i{b}")
                for b in range(B)]
        a11o = [dram.tile([NCORES * 128, TB], BF16, name=f"a11o{b}")
                for b in range(B)]
        a1qi = [dram.tile([NCORES * 128, TB], BF16, name=f"a1qi{b}")
                for b in range(B)]
        a1qo = [dram.tile([NCORES * 128, TB], BF16, name=f"a1qo{b}")
                for b in range(B)]
        a12i = [dram.tile([NCORES * 128, TB], BF16, name=f"a12i{b}")
                for b in range(B)]
        a12o = [dram.tile([NCORES * 128, TB], BF16, name=f"a12o{b}")
                for b in range(B)]

        # startup barrier: absorb cross-core launch skew here (overlapped
        # with the initial input DMAs) instead of inside the first real a2a
        nc.sync.dma_start(out=bar_in[:], in_=ident[0:NCORES, 0:16])
        nc.gpsimd.collective_compute(
            "AllToAll", mybir.AluOpType.bypass, replica_groups=GROUPS,
            ins=[bar_in.opt()], outs=[bar_out.opt()])

        def a2a(in_t, out_t):
            nc.gpsimd.collective_compute(
                "AllToAll", mybir.AluOpType.bypass, replica_groups=GROUPS,
                ins=[in_t.opt()], outs=[out_t.opt()])

        # ---------------- shared helpers ----------------
        def proj_tile(dst, w_sb, bias, src_sb, jj):
            # dst[:, 512*jj:...] = (w_chunk^T @ srcT)[dcol, tok] + bias.
            # src_sb is one batch half [128, KD*SB]; jj in 0..3.
            ps = ps_aux.tile([128, 512], F32, tag="psaux", name="psp")
            for k in range(KD):
                nc.tensor.matmul(
                    ps[:],
                    lhsT=w_sb[:, 128 * k:128 * (k + 1)],
                    rhs=src_sb[:, k * SB + 512 * jj:k * SB + 512 * jj + 512],
                    start=(k == 0), stop=(k == KD - 1))
            nc.vector.tensor_scalar_add(dst[:, 512 * jj:512 * (jj + 1)],
                                        ps[:], bias[:])

        def vaug_ones(vaug_sb, key_mask_col=None):
            # write only the 65th (ones/denominator) column of each group:
            # the 0:64 columns are fully overwritten by the chunk transposes
            v = vaug_sb.rearrange("p (h c d) -> p h c d", h=HPC, c=NBC)
            if key_mask_col is None:
                nc.vector.memset(v[:, :, :, 64:65], 1.0)
            else:
                for h in range(HPC):
                    nc.vector.tensor_copy(
                        v[:, h, :, 64],
                        mb[:, key_mask_col:key_mask_col + NBC])

        def vaug_chunk_tr(vT_sb, vaug_sb, c, key_mask_col=None):
            # PE-transpose V chunk c ([128 (h,d), 128 tok] -> [128 tok,
            # (h,d)]) and scatter into vaug's per-head 65-column groups;
            # key_mask zeroes dropped keys (per-partition scalar, fused
            # into the scatter copy).
            ptr = ps_aux.tile([128, 128], BF16, tag="psaux", name="ptr")
            nc.tensor.transpose(ptr[:], vT_sb[:, 128 * c:128 * (c + 1)],
                                ident[:])
            dst = vaug_sb.rearrange("p (h c d) -> p h c d", h=HPC, c=NBC)
            src = ptr.rearrange("p (h d) -> p h d", h=HPC)
            if key_mask_col is None:
                nc.vector.tensor_copy(dst[:, :, c, 0:64], src)
            else:
                nc.vector.tensor_scalar_mul(
                    dst[:, :, c, 0:64], src,
                    mb[:, key_mask_col + c:key_mask_col + c + 1])

        def vaug_slice(vaug_sb, h, c):
            base = 65 * (NBC * h + c)
            return vaug_sb[:, base:base + 65]

        def rsqrt_of(veps):
            # [128, 1] f32 -> rstd = (veps)^-0.5 via exp-bits seed (~1.5%
            # err) + one fused Newton step (3 DVE ops, ~3e-3 max err)
            bfi = lns.tile([128, 1], F32, tag="bfi")
            nc.vector.tensor_copy(bfi[:], veps.bitcast(I32))
            y = lns.tile([128, 1], F32, tag="rsy")
            nc.scalar.activation(y[:], bfi[:], EXP, scale=RS_SCALE,
                                 bias=rsb[:])
            t2 = lns.tile([128, 1], F32, tag="rst")
            nc.vector.scalar_tensor_tensor(t2[:], veps, y[:], y[:],
                                           op0=MULT, op1=MULT)
            c2 = lns.tile([128, 1], F32, tag="rsc")
            nc.vector.tensor_scalar(c2[:], t2[:], -0.5, 1.5,
                                    op0=MULT, op1=ADD)
            yn = lns.tile([128, 1], F32, tag="rsn")
            nc.vector.tensor_scalar(yn[:], c2[:], y[:], 0.0,
                                    op0=MULT, op1=ADD)
            return yn

        def ln_inplace(pre, dst):
            bnst = lns.tile([128, 12], F32, tag="bnst")
            nc.vector.bn_stats(bnst[:, 0:6], pre[:, 0:512])
            nc.vector.bn_stats(bnst[:, 6:12], pre[:, 512:1024])
            stats = lns.tile([128, 2], F32, tag="stats")
            nc.vector.bn_aggr(stats[:], bnst[:])
            veps = lns.tile([128, 1], F32, tag="veps")
            nc.vector.tensor_scalar_add(veps[:], stats[:, 1:2], EPS)
            rstd = rsqrt_of(veps[:])
            nc.vector.tensor_scalar(dst[:], pre[:], stats[:, 0:1], rstd[:],
                                    op0=SUB, op1=MULT)

        def attention(pools, QT_sb, KT_sb, vaug_sb, stage_sb, blocks,
                      fillers=None, filler_delay=0):
            # Software-pipelined attention over ONE batch. Per work unit
            # (t, chunk-group): scores for GW ki-chunks of both heads land
            # in per-head psums (h0 on PE rows 0-63, h1 on rows 64-127 —
            # row tiling), one Exp per head covers the group. The attnV
            # matmuls of the PREVIOUS unit are emitted after this unit's
            # scores so the PE queue never stalls on the exp; softmax
            # divisions are delayed one more unit.
            ps_s, ps_o, ptp, smalls = pools
            units = []
            for t in range(NBT):
                clist = [c for c in range(NBC) if (t, c) in blocks]
                groups = [clist[i:i + GW] for i in range(0, len(clist), GW)]
                for gi, grp in enumerate(groups):
                    units.append((t, grp, gi == 0, gi == len(groups) - 1))

            po = {}          # live accumulation psums, per head
            pending = None   # (unit, pt4 dict)
            div_q = []       # (t, po) awaiting division emission

            def emit_attnv(unit, pt4):
                t, grp, isfirst, islast = unit
                if isfirst:
                    for h in range(HPC):
                        po[h] = ps_o.tile([65, 512], F32, tag=f"po{h}",
                                          name=f"po{h}")
                for ci, c in enumerate(grp):
                    kind = blocks[(t, c)]
                    for h in range(HPC):
                        rhs = pt4[h][:, 512 * ci:512 * (ci + 1)]
                        if kind != 'full':
                            idx = kind[1]
                            nc.vector.tensor_tensor(
                                rhs, rhs, cm[:, 512 * idx:512 * (idx + 1)],
                                op=MULT)
                        nc.tensor.matmul(
                            po[h][:], lhsT=vaug_slice(vaug_sb, h, c),
                            rhs=rhs, start=(isfirst and ci == 0),
                            stop=(islast and ci == len(grp) - 1))
                if islast:
                    div_q.append((t, dict(po)))

            def emit_division():
                t, po_t = div_q.pop(0)
                for h in range(HPC):
                    osb = smalls.tile([65, 512], F32, tag="osb")
                    nc.vector.tensor_copy(osb[:], po_t[h][:])
                    recip = smalls.tile([1, 512], F32R, tag="recip")
                    with nc.allow_low_precision(reason="softmax recip row"):
                        nc.vector.reciprocal(recip[:], osb[64:65, :])
                    pb = ps_aux.tile([65, 512], F32, tag="psaux", name="pb")
                    nc.tensor.matmul(pb[:], lhsT=ones65[:].bitcast(F32R),
                                     rhs=recip[:], start=True, stop=True)
                    pbsb = smalls.tile([64, 512], F32, tag="pbsb")
                    nc.vector.tensor_copy(pbsb[:], pb[0:64, :])
                    nc.vector.tensor_tensor(
                        stage_sb[0:64,
                                 h * SB + 512 * t:h * SB + 512 * t + 512],
                        osb[0:64, :], pbsb[:], op=MULT)

            fillers = list(fillers) if fillers else []
            unit_no = [0]

            def emit_warmer():
                # Keep PE array activity high: attention's half-array matmuls
                # don't generate enough PE activity for the HAM to ever
                # unthrottle the clock. Emit either a unit of real full-array
                # work (pipelined filler) or a dummy matmul.
                if fillers and unit_no[0] >= filler_delay:
                    fillers.pop(0)()
                    return
                pd = ps_aux.tile([128, 512], F32, tag="psaux", name="pd")
                nc.tensor.matmul(pd[:], lhsT=KT_sb[:, 0:128],
                                 rhs=KT_sb[:, 1024:1536],
                                 start=True, stop=True)

            for unit in units:
                t, grp, isfirst, islast = unit
                ps4 = {h: ps_s.tile([128, GW * 512], F32,
                                    tag=f"ps4h{h}", name=f"ps4h{h}")
                       for h in range(HPC)}
                for ci, c in enumerate(grp):
                    for h in range(HPC):
                        nc.tensor.matmul(
                            ps4[h][:, 512 * ci:512 * (ci + 1)],
                            lhsT=KT_sb[64 * h:64 * (h + 1),
                                       128 * c:128 * c + 128],
                            rhs=QT_sb[64 * h:64 * (h + 1),
                                      512 * t:512 * t + 512],
                            start=True, stop=True)
                pt4 = {}
                for h in range(HPC):
                    pt4[h] = ptp.tile([128, GW * 512], BF16,
                                      tag=f"pt4h{h}", name=f"pt4h{h}")
                    nc.scalar.activation(pt4[h][:], ps4[h][:], EXP,
                                         scale=0.125)
                emit_warmer()
                if pending is not None:
                    emit_attnv(*pending)
                    if div_q and len(div_q) > 1:
                        emit_division()
                emit_warmer()
                pending = (unit, pt4)
                unit_no[0] += 1
            emit_attnv(*pending)
            while div_q:
                emit_division()
            for f in fillers:
                f()

        def stage_to_a2a(stage_sb, a2a_in_t):
            for h in range(HPC):
                o = a2a_in_t.rearrange("(j r) s -> r j s", r=128)
                nc.sync.dma_start(
                    out=o[64 * h:64 * (h + 1)],
                    in_=stage_sb.rearrange("r (h j s) -> r h j s",
                                           h=HPC, j=NCORES)[:, h])

        def wo_ln_closures(pool, pget, at_sb, wo_sb, resid_of, outm_of,
                           outT_sb, post=None):
            # Returns filler closures computing, for m in {0,1}:
            # outm_of(m) = LN(resid_of(m) + at^T @ wo), plus the bf16
            # PE-transpose of the LN output into outT_sb, then post().
            # Order: all 4 matmul closures first, then the two LN chains
            # (so their serial DVE/ACT latency overlaps PE work), then the
            # PE transposes.
            closures = []
            pre_box = {}
            obf_box = {}

            def mk_mm(m, eh):
                def f():
                    if pget is not None:
                        pget()
                    if m not in pre_box:
                        pre_box[m] = pool.tile([128, 1024], F32, tag="pre",
                                               name=f"pre{m}")
                    pre = pre_box[m]
                    pw = ps_aux.tile([128, 512], F32, tag="psaux", name="pw")
                    for dc in range(KD):
                        nc.tensor.matmul(
                            pw[:],
                            lhsT=at_sb[:, TB * dc + 128 * m:
                                       TB * dc + 128 * m + 128],
                            rhs=wo_sb[:, 1024 * dc + 512 * eh:
                                      1024 * dc + 512 * eh + 512],
                            start=(dc == 0), stop=(dc == KD - 1))
                    nc.vector.tensor_tensor(
                        pre[:, 512 * eh:512 * (eh + 1)], pw[:],
                        resid_of(m)[:, 512 * eh:512 * (eh + 1)], op=ADD)
                return f

            def mk_ln(m):
                def f():
                    pre = pre_box.pop(m)
                    outm = outm_of(m)
                    ln_inplace(pre, outm)
                    obf = pool.tile([128, 1024], BF16, tag="obf",
                                    name=f"obf{m}")
                    nc.vector.tensor_copy(obf[:], outm)
                    obf_box[m] = obf
                return f

            def mk_tr(m):
                def f():
                    obf = obf_box.pop(m)
                    for j in range(KD):
                        ptr = ps_aux.tile([128, 128], BF16, tag="psaux",
                                          name="ptr2")
                        nc.tensor.transpose(ptr[:],
                                            obf[:, 128 * j:128 * (j + 1)],
                                            ident[:])
                        nc.vector.tensor_copy(
                            outT_sb[:, TB * j + 128 * m:
                                    TB * j + 128 * m + 128], ptr[:])
                    if post is not None and m == 1:
                        post()
                return f

            for m in range(2):
                closures.append(mk_mm(m, 0))
                closures.append(mk_mm(m, 1))
            for m in range(2):
                closures.append(mk_ln(m))
            for m in range(2):
                closures.append(mk_tr(m))
            return closures

        def q2_closures(pool, wq2_sb, bq2_sb, outT_sb, qt2_box):
            closures = []

            def mk(j):
                def f():
                    if "t" not in qt2_box:
                        qt2_box["t"] = pool.tile([128, KD * TB], BF16,
                                                 tag="qt2", name="qt2")
                    qt2 = qt2_box["t"]
                    pq = ps_aux.tile([128, TB], F32, tag="psaux", name="pq")
                    for k in range(KD):
                        nc.tensor.matmul(
                            pq[:],
                            lhsT=wq2_sb[:, 1024 * j + 128 * k:
                                        1024 * j + 128 * k + 128],
                            rhs=outT_sb[:, TB * k:TB * (k + 1)],
                            start=(k == 0), stop=(k == KD - 1))
                    nc.vector.tensor_scalar_add(
                        qt2[:, TB * j:TB * (j + 1)], pq[:],
                        bq2_sb[:, j:j + 1])
                return f

            for j in range(KD):
                closures.append(mk(j))
            return closures

        def ffn_w1_closures(pool, w1s_pool, b1f_sb, outT_sb, hT_box, tag):
            closures = []

            def mk(fc):
                def f():
                    if "t" not in hT_box:
                        hT_box["t"] = pool.tile([128, FC * TB], BF16,
                                                tag=tag, name=tag)
                    hT = hT_box["t"]
                    w1t = w1s_pool.tile([128, KD * 128], BF16, tag="w1s")
                    nc.sync.dma_start(out=w1t[:],
                                      in_=w1_d[:, 1024 * fc:1024 * (fc + 1)])
                    ph = ps_aux.tile([128, TB], F32, tag="psaux", name="ph")
                    for k in range(KD):
                        nc.tensor.matmul(
                            ph[:],
                            lhsT=w1t[:, 128 * k:128 * (k + 1)],
                            rhs=outT_sb[:, TB * k:TB * (k + 1)],
                            start=(k == 0), stop=(k == KD - 1))
                    nc.vector.tensor_scalar(hT[:, TB * fc:TB * (fc + 1)],
                                            ph[:], b1f_sb[:, fc:fc + 1], 0.0,
                                            op0=ADD, op1=MAX)
                return f

            for fc in range(FC):
                closures.append(mk(fc))
            return closures

        out1v = out1.rearrange("p (m e) -> p m e", m=4)
        out2v = out2.rearrange("p (m e) -> p m e", m=4)

        # =====================================================================
        # p3k: cross-attn K/V/Q tensors that survive into attn2 phases
        with tc.tile_pool(name="p3k", bufs=1) as p3k:
            KT2 = [p3k.tile([128, SB], BF16, tag=f"KT2{b}", name=f"KT2{b}")
                   for b in range(B)]
            vaug2 = [p3k.tile([128, HPC * NBC * 65], BF16, tag=f"vaug2{b}",
                              name=f"vaug2{b}") for b in range(B)]
            QT2 = [p3k.tile([128, SB], BF16, tag=f"QT2{b}", name=f"QT2{b}")
                   for b in range(B)]

            # ============ phases 1-2: QKV1 + self attention ==================
            with tc.tile_pool(name="pA", bufs=1) as pA, \
                 tc.tile_pool(name="p12s", bufs=2) as p12s:
                wq1 = pA.tile([128, KD * 128], BF16, tag="wq1")
                wk1 = pA.tile([128, KD * 128], BF16, tag="wk1")
                wv1 = pA.tile([128, KD * 128], BF16, tag="wv1")
                nc.sync.dma_start(out=wq1[:], in_=wq1_d[:])
                nc.sync.dma_start(out=wk1[:], in_=wk1_d[:])
                nc.sync.dma_start(out=wv1[:], in_=wv1_d[:])
                bq1 = pA.tile([128, 1], F32, tag="bq1")
                bk1 = pA.tile([128, 1], F32, tag="bk1")
                bv1 = pA.tile([128, 1], F32, tag="bv1")
                nc.sync.dma_start(out=bq1[:], in_=bq1_d[:])
                nc.sync.dma_start(out=bk1[:], in_=bk1_d[:])
                nc.sync.dma_start(out=bv1[:], in_=bv1_d[:])
                wk2 = pA.tile([128, KD * 128], BF16, tag="wk2")
                wv2 = pA.tile([128, KD * 128], BF16, tag="wv2")
                nc.sync.dma_start(out=wk2[:], in_=wk2_d[:])
                nc.sync.dma_start(out=wv2[:], in_=wv2_d[:])
                bk2 = pA.tile([128, 1], F32, tag="bk2")
                bv2 = pA.tile([128, 1], F32, tag="bv2")
                nc.sync.dma_start(out=bk2[:], in_=bk2_d[:])
                nc.sync.dma_start(out=bv2[:], in_=bv2_d[:])

                xTd = xT_d.rearrange("(k p) t -> p k t", p=128)
                encTd = encT_d.rearrange("(k p) t -> p k t", p=128)

                def load_half(tl, src_view, b):
                    # chunked per-512-token-tile DMA of one batch half
                    v = tl.rearrange("p (k t) -> p k t", k=KD)
                    for jj in range(4):
                        nc.sync.dma_start(
                            out=v[:, :, 512 * jj:512 * (jj + 1)],
                            in_=src_view[:, :, SB * b + 512 * jj:
                                         SB * b + 512 * jj + 512])

                # xT halves share one 32KB slot; the b1 half's DMA waits
                # for the b0 projections to finish reading the slot.
                xTh0 = pA.tile([128, KD * SB], BF16, tag="xTh", name="xTh0")
                load_half(xTh0, xTd, 0)
                encTh0 = pA.tile([128, KD * SB], BF16, tag="encTh",
                                 name="encTh0")
                load_half(encTh0, encTd, 0)

                QT = [pA.tile([128, SB], BF16, tag=f"QT{b}", name=f"QT{b}")
                      for b in range(B)]
                KT = [pA.tile([128, SB], BF16, tag=f"KT{b}", name=f"KT{b}")
                      for b in range(B)]
                vT1 = pA.tile([128, SB], BF16, tag="vTs", name="vT1")
                vaug1 = [pA.tile([128, HPC * NBC * 65], BF16,
                                 tag=f"vaug1{b}", name=f"vaug1{b}")
                         for b in range(B)]
                stage1 = [pA.tile([64, HPC * SB], BF16, tag=f"stage1{b}",
                                  name=f"stage1{b}") for b in range(B)]


                # QKV1(b0) directly
                for jj in range(4):
                    proj_tile(vT1, wv1, bv1, xTh0, jj)
                nc.vector.memset(vaug1[0][:], 1.0)
                for c in range(NBC):
                    vaug_chunk_tr(vT1, vaug1[0], c)
                for jj in range(4):
                    proj_tile(QT[0], wq1, bq1, xTh0, jj)
                for jj in range(4):
                    proj_tile(KT[0], wk1, bk1, xTh0, jj)

                # xT b1 half: slot reuse waits for the QKV1(b0) reads, DMA
                # flies while attn1(b0) computes
                xTh1 = pA.tile([128, KD * SB], BF16, tag="xTh", name="xTh1")
                load_half(xTh1, xTd, 1)

                # attn1(b0) with QKV1(b1) as fillers
                f_a1b0 = []
                for jj in range(4):
                    f_a1b0.append(lambda jj=jj: proj_tile(vT1, wv1, bv1,
                                                          xTh1, jj))
                f_a1b0.append(lambda: nc.vector.memset(vaug1[1][:], 1.0))
                for c0 in range(0, NBC, 4):
                    def fv(c0=c0):
                        for c in range(c0, c0 + 4):
                            vaug_chunk_tr(vT1, vaug1[1], c)
                    f_a1b0.append(fv)
                for jj in range(4):
                    f_a1b0.append(lambda jj=jj: proj_tile(QT[1], wq1, bq1,
                                                          xTh1, jj))
                for jj in range(4):
                    f_a1b0.append(lambda jj=jj: proj_tile(KT[1], wk1, bk1,
                                                          xTh1, jj))

                with tc.tile_pool(name="ps_s1", bufs=1, space="PSUM") as ps_s, \
                     tc.tile_pool(name="ps_o1", bufs=1, space="PSUM") as ps_o, \
                     tc.tile_pool(name="pt1", bufs=2) as ptp:
                    attention((ps_s, ps_o, ptp, p12s),
                              QT[0], KT[0], vaug1[0], stage1[0], self_blocks,
                              fillers=f_a1b0, filler_delay=3)
                stage_to_a2a(stage1[0], a11i[0])
                a2a(a11i[0], a11o[0])

                # vT2 shares vT1's slot: vT1 dies once vaug1[1] is built
                # (a filler of attn1(b0), emitted above)
                vT2 = pA.tile([128, SB], BF16, tag="vTs", name="vT2")

                # attn1(b1) with cross K/V prep as fillers; the encT b1
                # half-load is itself a filler (its slot-reuse waits for
                # all b0 readers, which precede it in the list)
                ench = {0: encTh0}

                def load_ench1():
                    ench[1] = pA.tile([128, KD * SB], BF16, tag="encTh",
                                      name="encTh1")
                    load_half(ench[1], encTd, 1)

                f_a1b1 = []
                for b in range(B):
                    if b == 1:
                        f_a1b1.append(load_ench1)
                    for jj in range(4):
                        f_a1b1.append(lambda b=b, jj=jj: proj_tile(
                            vT2, wv2, bv2, ench[b], jj))
                    f_a1b1.append(lambda b=b: nc.vector.memset(
                        vaug2[b][:], 1.0))
                    for c0 in range(0, NBC, 4):
                        def fv2(b=b, c0=c0):
                            for c in range(c0, c0 + 4):
                                vaug_chunk_tr(vT2, vaug2[b], c,
                                              key_mask_col=NBC * b)
                        f_a1b1.append(fv2)
                    for jj in range(4):
                        f_a1b1.append(lambda b=b, jj=jj: proj_tile(
                            KT2[b], wk2, bk2, ench[b], jj))

                with tc.tile_pool(name="ps_s2", bufs=1, space="PSUM") as ps_s, \
                     tc.tile_pool(name="ps_o2", bufs=1, space="PSUM") as ps_o, \
                     tc.tile_pool(name="pt2", bufs=2) as ptp:
                    attention((ps_s, ps_o, ptp, p12s),
                              QT[1], KT[1], vaug1[1], stage1[1], self_blocks,
                              fillers=f_a1b1, filler_delay=0)
                stage_to_a2a(stage1[1], a11i[1])
                # NOTE: a2a1(b1)'s trigger is deferred until after a2aq(b0)
                # so attn2(b0) can start as early as possible; nothing needs
                # a2a1(b1) before the W1(b1) fillers inside attn2(b0).
            # pA closed

            # ============ phase 3: wo1+LN1+q2 (b0 direct, b1 as fillers) ====
            with tc.tile_pool(name="pW1", bufs=1) as pW1, \
                 tc.tile_pool(name="pW1s", bufs=2) as pW1s:
                wo1 = pW1.tile([128, KD * 1024], BF16, tag="wo1")
                nc.sync.dma_start(out=wo1[:], in_=wo1_d[:])
                wq2 = pW1.tile([128, KD * KD * 128], BF16, tag="wq2")
                nc.sync.dma_start(out=wq2[:], in_=wq2_d[:])
                bq2 = pW1.tile([128, KD], F32, tag="bq2")
                nc.sync.dma_start(out=bq2[:], in_=bq2_d[:])
                bo2 = pW1.tile([128, 1024], F32, tag="bo2")
                nc.sync.dma_start(out=bo2[:], in_=bo2_d[:])
                outT1 = [pW1.tile([128, KD * TB], BF16, tag=f"outT1{b}",
                                  name=f"outT1{b}") for b in range(B)]
                xow = pW1.tile([128, 4 * 1024], F32, tag="xow")
                nc.sync.dma_start(
                    out=xow.rearrange("p (m e) -> p m e", m=4),
                    in_=xown_d.rearrange("(m p) e -> p m e", p=128))
                xowv = xow.rearrange("p (m e) -> p m e", m=4)
                at1 = [pW1.tile([128, KD * TB], BF16, tag=f"at1{b}",
                                name=f"at1{b}") for b in range(B)]
                at1_loaded = [False, False]

                def load_at1(b):
                    if not at1_loaded[b]:
                        at1_loaded[b] = True
                        nc.sync.dma_start(
                            out=at1[b].rearrange("p (dc s) -> p dc s", dc=KD),
                            in_=a11o[b].rearrange("(dc p) s -> p dc s",
                                                  p=128))

                qt2_box = [{}, {}]

                def post_bias1(b):
                    # fold bo2 into out1 (after outT1 transpose, before wo2)
                    for m in range(2):
                        mm = 2 * b + m
                        nc.vector.tensor_tensor(
                            out1v[:, mm], out1v[:, mm], bo2[:], op=ADD)

                def finish_q2(b):
                    qt2 = qt2_box[b]["t"]
                    nc.sync.dma_start(
                        out=a1qi[b].rearrange("(j p) s -> p j s", p=128),
                        in_=qt2.rearrange("p (j s) -> p j s", j=KD))
                    a2a(a1qi[b], a1qo[b])
                    nc.sync.dma_start(
                        out=QT2[b].rearrange("p (i s) -> p i s", i=NCORES),
                        in_=a1qo[b].rearrange("(i p) s -> p i s", p=128))

                # b0 direct
                load_at1(0)
                w1cl = wo_ln_closures(
                    pW1s, None, at1[0], wo1,
                    resid_of=lambda m: xowv[:, m],
                    outm_of=lambda m: out1v[:, m],
                    outT_sb=outT1[0], post=lambda: post_bias1(0))
                for f in w1cl:
                    f()
                for f in q2_closures(pW1, wq2, bq2, outT1[0], qt2_box[0]):
                    f()
                finish_q2(0)
                # trigger a2a1(b1) only now: the cc stream is serial, and
                # a2aq(b0) gates attn2(b0) while a2a1(b1) is only needed by
                # the W1(b1) fillers ~10 units into attn2(b0)
                a2a(a11i[1], a11o[1])

                # attn2(b0) with W1(b1) + q2(b1) as fillers
                f_a2b0 = wo_ln_closures(
                    pW1s, lambda: load_at1(1), at1[1], wo1,
                    resid_of=lambda m: xowv[:, 2 + m],
                    outm_of=lambda m: out1v[:, 2 + m],
                    outT_sb=outT1[1], post=lambda: post_bias1(1))
                f_a2b0 += q2_closures(pW1, wq2, bq2, outT1[1], qt2_box[1])
                f_a2b0.append(lambda: finish_q2(1))

                with tc.tile_pool(name="pX", bufs=1) as pX, \
                     tc.tile_pool(name="ps_s3", bufs=1, space="PSUM") as ps_s, \
                     tc.tile_pool(name="ps_o3", bufs=1, space="PSUM") as ps_o, \
                     tc.tile_pool(name="pt3", bufs=2) as ptp:
                    stage2_b0 = pX.tile([64, HPC * SB], BF16, tag="stage2")
                    attention((ps_s, ps_o, ptp, pW1s),
                              QT2[0], KT2[0], vaug2[0], stage2_b0,
                              CROSS_BLOCKS, fillers=f_a2b0, filler_delay=10)
                    stage_to_a2a(stage2_b0, a12i[0])
                a2a(a12i[0], a12o[0])
            # pW1 closed

            # ============ phase 4: attn2(b1) + wo2/FFN-w1(b0) fillers =======
            with tc.tile_pool(name="p78", bufs=1) as p78, \
                 tc.tile_pool(name="p78s", bufs=2) as p78s, \
                 tc.tile_pool(name="w1str", bufs=3) as w1s_pool, \
                 tc.tile_pool(name="w2str", bufs=3) as w2s_pool:
                wo2 = p78.tile([128, KD * 1024], BF16, tag="wo2")
                nc.sync.dma_start(out=wo2[:], in_=wo2_d[:])
                b1f = p78.tile([128, FC], F32, tag="b1f")
                nc.sync.dma_start(out=b1f[:], in_=b1f_d[:])
                b2 = p78.tile([128, 1024], F32, tag="b2")
                nc.sync.dma_start(out=b2[:], in_=b2_d[:])

                at2 = [p78.tile([128, KD * TB], BF16, tag=f"at2{b}",
                                name=f"at2{b}") for b in range(B)]
                at2_loaded = [False, False]

                def load_at2(b):
                    if not at2_loaded[b]:
                        at2_loaded[b] = True
                        nc.sync.dma_start(
                            out=at2[b].rearrange("p (dc s) -> p dc s", dc=KD),
                            in_=a12o[b].rearrange("(dc p) s -> p dc s",
                                                  p=128))

                outT2 = [p78.tile([128, KD * TB], BF16, tag=f"outT2{b}",
                                  name=f"outT2{b}") for b in range(B)]
                hT_box = [{}, {}]

                def post_bias2(b):
                    # fold b2 into out2 (after outT2 transpose, before FFN w2)
                    for m in range(2):
                        mm = 2 * b + m
                        nc.vector.tensor_tensor(
                            out2v[:, mm], out2v[:, mm], b2[:], op=ADD)

                f_a2b1 = wo_ln_closures(
                    p78s, lambda: load_at2(0), at2[0], wo2,
                    resid_of=lambda m: out1v[:, m],
                    outm_of=lambda m: out2v[:, m],
                    outT_sb=outT2[0], post=lambda: post_bias2(0))
                f_a2b1 += ffn_w1_closures(p78, w1s_pool, b1f, outT2[0],
                                          hT_box[0], tag="hT")

                with tc.tile_pool(name="pX2", bufs=1) as pX2, \
                     tc.tile_pool(name="ps_s4", bufs=1, space="PSUM") as ps_s, \
                     tc.tile_pool(name="ps_o4", bufs=1, space="PSUM") as ps_o, \
                     tc.tile_pool(name="pt4", bufs=2) as ptp:
                    stage2_b1 = pX2.tile([64, HPC * SB], BF16, tag="stage2b")
                    attention((ps_s, ps_o, ptp, p78s),
                              QT2[1], KT2[1], vaug2[1], stage2_b1,
                              CROSS_BLOCKS, fillers=f_a2b1, filler_delay=8)
                    stage_to_a2a(stage2_b1, a12i[1])
                a2a(a12i[1], a12o[1])

                # ============ phase 5: FFN-w2+LN3(b0); then all of b1 =======
                def ffn_w2_ln3(b):
                    hT = hT_box[b]["t"]
                    with tc.tile_pool(name=f"ps_f{b}", bufs=1,
                                      space="PSUM") as ps_f:
                        py = {(m, eh): ps_f.tile([128, 512], F32,
                                                 tag=f"py{m}{eh}",
                                                 name=f"py{m}{eh}")
                              for m in range(2) for eh in range(2)}
                        for fc in range(FC):
                            w2t = w2s_pool.tile([128, 1024], BF16, tag="w2s")
                            nc.sync.dma_start(
                                out=w2t[:],
                                in_=w2_d[:, 1024 * fc:1024 * (fc + 1)])
                            for m in range(2):
                                for eh in range(2):
                                    nc.tensor.matmul(
                                        py[(m, eh)][:],
                                        lhsT=hT[:, TB * fc + 128 * m:
                                                TB * fc + 128 * m + 128],
                                        rhs=w2t[:, 512 * eh:512 * (eh + 1)],
                                        start=(fc == 0), stop=(fc == FC - 1))
                        pres = {}
                        for m in range(2):
                            mm = 2 * b + m
                            pre = p78s.tile([128, 1024], F32, tag="pref",
                                            name=f"pref{m}")
                            for eh in range(2):
                                nc.vector.tensor_tensor(
                                    pre[:, 512 * eh:512 * (eh + 1)],
                                    py[(m, eh)][:],
                                    out2v[:, mm, 512 * eh:512 * (eh + 1)],
                                    op=ADD)
                            pres[m] = pre
                        for m in range(2):
                            outf = p78s.tile([128, 1024], F32, tag="outf",
                                             name=f"outf{m}")
                            ln_inplace(pres[m], outf)
                            nc.sync.dma_start(
                                out=out_d[256 * b + 128 * m:
                                          256 * b + 128 * m + 128, :],
                                in_=outf[:])

                ffn_w2_ln3(0)

                # b1 drain: wo2(b1) + LN2(b1) + FFN(b1)
                w2cl = wo_ln_closures(
                    p78s, lambda: load_at2(1), at2[1], wo2,
                    resid_of=lambda m: out1v[:, 2 + m],
                    outm_of=lambda m: out2v[:, 2 + m],
                    outT_sb=outT2[1], post=lambda: post_bias2(1))
                for f in w2cl:
                    f()
                for f in ffn_w1_closures(p78, w1s_pool, b1f, outT2[1],
                                         hT_box[1], tag="hT"):
                    f()
                ffn_w2_ln3(1)

    nc.compile()
    return nc


def _to_bf(a):
    return np.ascontiguousarray(np.asarray(a, np.float32).astype(BF))


def _rechunk_k(w):
    """[K*128, M] -> [128, K*M] with col k*M + m = w[k*128 + p, m]."""
    K = w.shape[0] // 128
    M = w.shape[1]
    return np.ascontiguousarray(
        w.reshape(K, 128, M).transpose(1, 0, 2).reshape(128, K * M))


def _analyze_self_mask(mask):
    """mask [S, S] (1 = disallowed), orientation [q, k].

    Returns blocks dict (t, c) -> 'full' | ('tile', idx), list of unique
    multiplicative tiles [128, 512] (bf16), for a block grid over one batch.
    Blocks where everything is disallowed are omitted.
    """
    add = np.float32(-1e9) * np.asarray(mask, np.float32)
    mult = np.exp(add.T)  # [k, q] multiplicative
    blocks = {}
    tiles = []
    tile_ids = {}
    for t in range(NBT):
        for c in range(NBC):
            sub = mult[128 * c:128 * (c + 1), 512 * t:512 * (t + 1)]
            if not sub.any():
                continue
            if (sub == 1.0).all():
                blocks[(t, c)] = 'full'
                continue
            key = sub.tobytes()
            if key not in tile_ids:
                tile_ids[key] = len(tiles)
                tiles.append(sub.astype(BF))
            blocks[(t, c)] = ('tile', tile_ids[key])
    return blocks, tiles


def kernel(**inputs):
    from concourse.bass_utils import run_bass_kernel_spmd

    x = np.asarray(inputs["x"], np.float32)
    enc = np.asarray(inputs["enc_output"], np.float32)
    lam = np.asarray(inputs["look_ahead_mask"], np.float32)[0, 0]
    pad = np.asarray(inputs["padding_mask"], np.float32)  # [B,1,1,S]

    self_blocks, ctiles = _analyze_self_mask(lam)
    n_ctiles = len(ctiles)
    key = (tuple(sorted(self_blocks.items())), n_ctiles)
    if key not in _PROG_CACHE:
        _PROG_CACHE[key] = _build_program(self_blocks, n_ctiles)
    nc = _PROG_CACHE[key]

    # ---- shared (core-independent) host prep ----
    xf = x.reshape(TOK, D_MODEL)             # flattened batch-major tokens
    encf = enc.reshape(TOK, D_MODEL)
    xT = _to_bf(xf.T)                        # [1024, 4096]
    encT = _to_bf(encf.T)
    if n_ctiles:
        cmask = np.concatenate(ctiles, axis=1)
    else:
        cmask = np.zeros((128, 512), BF)
    cmask = np.ascontiguousarray(cmask)
    # cross-attn key-keep mask per enc token: [128, B*16], col b*16+c
    mb = np.exp(np.float32(-1e9) * pad[:, 0, 0, :]).reshape(B, NBC, 128)
    mb = np.ascontiguousarray(mb.transpose(2, 0, 1).reshape(128, B * NBC)
                              ).astype(np.float32)

    w1f = np.asarray(inputs["ffn_w1"], np.float32)
    # w1 stationary layout: [128, fc*1024 + k*128 + m] = w1[k*128+p, fc*128+m]
    w1r = w1f.reshape(KD, 128, FC, 128).transpose(1, 2, 0, 3)
    w1r = _to_bf(w1r.reshape(128, FC * KD * 128))
    w2r = _to_bf(_rechunk_k(np.asarray(inputs["ffn_w2"], np.float32)))
    # b1 per-partition per-chunk [128, FC]; b2/bo2 pre-broadcast [128, 1024]
    b1 = np.ascontiguousarray(
        np.asarray(inputs["ffn_b1"], np.float32).reshape(FC, 128).T)
    b2 = np.ascontiguousarray(np.broadcast_to(
        np.asarray(inputs["ffn_b2"], np.float32)[None, :], (128, 1024)))

    wo1r = _to_bf(_rechunk_k(np.asarray(inputs["mha1_wo"], np.float32)))
    wo2r = _to_bf(_rechunk_k(np.asarray(inputs["mha2_wo"], np.float32)))
    bo1 = np.asarray(inputs["mha1_bo"], np.float32)
    bo2 = np.ascontiguousarray(np.broadcast_to(
        np.asarray(inputs["mha2_bo"], np.float32)[None, :], (128, 1024)))
    ident = np.eye(128, dtype=np.float32).astype(BF)

    wq2_full = np.asarray(inputs["mha2_wq"], np.float32)
    # wq2 stationary layout: [128, j*1024 + k*128 + m] = wq2[k*128+p, j*128+m]
    wq2r = wq2_full.reshape(KD, 128, KD, 128).transpose(1, 2, 0, 3)
    wq2r = _to_bf(wq2r.reshape(128, KD * KD * 128))
    bq2 = np.asarray(inputs["mha2_bq"], np.float32).reshape(KD, 128)
    bq2 = np.ascontiguousarray(bq2.T).astype(np.float32)  # [128, KD]

    in_maps = []
    for j in range(NCORES):
        hs = slice(128 * j, 128 * (j + 1))       # this core's 2 heads' cols
        xo = np.concatenate([xf[TB * j:TB * (j + 1)],
                             xf[S + TB * j:S + TB * (j + 1)]], axis=0)
        xo = np.ascontiguousarray(xo + bo1[None, :])
        m = {
            "xT": xT, "encT": encT, "x_own": xo,
            "cmask": cmask, "mbias": mb,
            "w1": w1r, "b1": b1, "w2": w2r, "b2": b2,
            "wo1": wo1r, "wo2": wo2r, "bo2": bo2,
            "wq2": wq2r, "bq2": bq2, "ident": ident,
        }
        for pre, name in (("wq1", "mha1_wq"), ("wk1", "mha1_wk"),
                          ("wv1", "mha1_wv"), ("wk2", "mha2_wk"),
                          ("wv2", "mha2_wv")):
            w = np.asarray(inputs[name], np.float32)[:, hs]
            m[pre] = _to_bf(_rechunk_k(w))
        for pre, name in (("bq1", "mha1_bq"), ("bk1", "mha1_bk"),
                          ("bv1", "mha1_bv"), ("bk2", "mha2_bk"),
                          ("bv2", "mha2_bv")):
            bvec = np.asarray(inputs[name], np.float32)[hs]
            m[pre] = np.ascontiguousarray(bvec[:, None])
        in_maps.append(m)

    res = run_bass_kernel_spmd(nc, in_maps, list(range(NCORES)))
    out = np.empty((TOK, D_MODEL), np.float32)
    for j in range(NCORES):
        r = res.results[j]["out"]
        out[TB * j:TB * (j + 1)] = r[0:TB]
        out[S + TB * j:S + TB * (j + 1)] = r[TB:2 * TB]
    return out.reshape(B, S, D_MODEL)


# revision 28
# speedup vs baseline: 1.0638x; 1.0042x over previous
"""Trainium2 Bass kernel for a transformer decoder layer (self-attn + cross-attn + FFN).

Sharding: 8-way tensor parallel over heads for both attentions (2 heads/core);
token-sharded for wo projections, layernorms and FFN with each core owning 256
tokens of EACH batch. Head<->token redistribution uses six half-size (per-batch)
AllToAll collectives. The two batches are independent through the whole layer,
so the schedule is batch-pipelined: every collective flies while the other
batch computes (attn1(b1) covers a2a1(b0); wo1/q2(b1) and wo2+FFN-w1(b0) run as
PE filler work inside the ACT-bound cross-attention phases; etc.).

All matmuls run in bf16 with fp32 PSUM accumulation. Attention keeps the
[feature, token] (transposed) layout throughout: scoresT uses kT-chunk
stationary x qT moving, probs come out as PT[ki, qi] which feeds attnV
directly with V-natural (+ones column) stationary, producing attn^T and the
softmax denominator in one accumulation group. Normalization happens via a
reciprocal row broadcast with a rank-1 fp32r matmul, off the critical path.
The cross-attention padding mask is folded into V by zeroing masked key rows
(incl. the ones column), which removes them from output and denominator.

LayerNorm rstd is computed as exp(scale*bits(var+eps)+bias) (the exponent-bits
log approximation folded into ACT's free affine) polished by two Newton
iterations on DVE — Exp is the only ACT table function in the whole kernel,
avoiding the ~1.3us-per-switch activation-table ping-pong between Exp and Ln.
"""

import sys

TRN_REPO = "/opt/trn_rl_repo"
if TRN_REPO not in sys.path:
    sys.path.insert(0, TRN_REPO)

import numpy as np
import ml_dtypes

D_MODEL = 1024
N_HEADS = 16
DFF = 4096
B, S = 2, 2048
EPS = 1e-6
DEPTH = D_MODEL // N_HEADS  # 64

NCORES = 8
HPC = N_HEADS // NCORES     # heads per core = 2
TOK = B * S                 # 4096 flattened tokens
SB = S                      # tokens per batch = 2048
TB = SB // NCORES           # tokens per core per batch = 256
KD = D_MODEL // 128         # 8 contraction chunks over d_model
FC = DFF // 128             # 32 chunks over dff
NBT = SB // 512             # 4 q-tiles per batch
NBC = SB // 128             # 16 ki-chunks per batch

BF = ml_dtypes.bfloat16

# rsqrt-via-exp-bits constants: rsqrt(v) ~= exp(RS_SCALE*float(bits(v)) + RS_BIAS)
_LN2 = float(np.log(2.0))
RS_SCALE = -0.5 * _LN2 / (1 << 23)
RS_BIAS = 0.5 * _LN2 * (127 + 0.0430357)

_PROG_CACHE = {}


def _build_program(self_blocks, n_ctiles):
    """Emit the SPMD Bass program (same program on all 8 cores)."""
    import concourse.bacc as bacc
    import concourse.mybir as mybir
    from concourse import tile

    F32 = mybir.dt.float32
    F32R = mybir.dt.float32r
    I32 = mybir.dt.int32
    BF16 = mybir.dt.bfloat16
    EXP = mybir.ActivationFunctionType.Exp
    ADD = mybir.AluOpType.add
    MULT = mybir.AluOpType.mult
    SUB = mybir.AluOpType.subtract
    MAX = mybir.AluOpType.max

    nc = bacc.Bacc("TRN2", target_bir_lowering=False, debug=False,
                   num_devices=NCORES)

    def din(name, shape, dt=BF16):
        return nc.dram_tensor(name, shape, dt, kind="ExternalInput")

    xT_d = din("xT", [D_MODEL, TOK])
    encT_d = din("encT", [D_MODEL, TOK])
    xown_d = din("x_own", [2 * TB, D_MODEL], F32)   # bo1 pre-folded on host
    wq1_d = din("wq1", [128, KD * 128])
    wk1_d = din("wk1", [128, KD * 128])
    wv1_d = din("wv1", [128, KD * 128])
    bq1_d = din("bq1", [128, 1], F32)
    bk1_d = din("bk1", [128, 1], F32)
    bv1_d = din("bv1", [128, 1], F32)
    wo1_d = din("wo1", [128, KD * 1024])
    wq2_d = din("wq2", [128, KD * KD * 128])
    bq2_d = din("bq2", [128, KD], F32)
    wk2_d = din("wk2", [128, KD * 128])
    wv2_d = din("wv2", [128, KD * 128])
    bk2_d = din("bk2", [128, 1], F32)
    bv2_d = din("bv2", [128, 1], F32)
    wo2_d = din("wo2", [128, KD * 1024])
    bo2_d = din("bo2", [128, 1024], F32)   # pre-broadcast
    w1_d = din("w1", [128, FC * KD * 128])
    b1f_d = din("b1", [128, FC], F32)      # per-partition per-chunk
    w2_d = din("w2", [128, FC * 1024])
    b2_d = din("b2", [128, 1024], F32)     # pre-broadcast
    ident_d = din("ident", [128, 128])
    cm_d = din("cmask", [128, max(n_ctiles, 1) * 512])
    mb_d = din("mbias", [128, B * NBC], F32)
    out_d = nc.dram_tensor("out", [2 * TB, D_MODEL], F32, kind="ExternalOutput")

    CROSS_BLOCKS = {(t, c): 'full' for t in range(NBT) for c in range(NBC)}
    GROUPS = [list(range(NCORES))]
    GW = 2  # ki-chunks per merged exp group

    with tile.TileContext(nc) as tc:
      with tc.tile_pool(name="const", bufs=1) as constp, \
           tc.tile_pool(name="fbuf", bufs=1) as fbuf, \
           tc.tile_pool(name="lns", bufs=2) as lns, \
           tc.tile_pool(name="dram", bufs=1, space="DRAM") as dram, \
           tc.tile_pool(name="ps_aux", bufs=2, space="PSUM") as ps_aux:

        # ---- constants ----
        ones65 = constp.tile([1, 65], F32)
        nc.vector.memset(ones65[:], 1.0)
        rsb = constp.tile([128, 1], F32)
        nc.vector.memset(rsb[:], RS_BIAS)
        ident = constp.tile([128, 128], BF16)
        nc.sync.dma_start(out=ident[:], in_=ident_d[:])
        cm = constp.tile([128, max(n_ctiles, 1) * 512], BF16)
        nc.sync.dma_start(out=cm[:], in_=cm_d[:])
        mb = constp.tile([128, B * NBC], F32)
        nc.sync.dma_start(out=mb[:], in_=mb_d[:])

        # ---- persistent activations ----
        out1 = fbuf.tile([128, 4 * 1024], F32, tag="out1")
        out2 = fbuf.tile([128, 4 * 1024], F32, tag="out2")

        # ---- a2a dram buffers (per batch) ----
        bar_in = dram.tile([NCORES, 16], BF16)
        bar_out = dram.tile([NCORES, 16], BF16)
        a11i = [dram.tile([NCORES * 128, TB], BF16, name=f"a11i{b}")
                for b in range(B)]
        a11o = [dram.tile([NCORES * 128, TB], BF16, name=f"a11o{b}")
                for b in range(B)]
        a1qi = [dram.tile([NCORES * 128, TB], BF16, name=f"a1qi{b}")
                for b in range(B)]
        a1qo = [dram.tile([NCORES * 128, TB], BF16, name=f"a1qo{b}")
                for b in range(B)]
        a12i = [dram.tile([NCORES * 128, TB], BF16, name=f"a12i{b}")
                for b in range(B)]
        a12o = [dram.tile([NCORES * 128, TB], BF16, name=f"a12o{b}")
                for b in range(B)]

        # startup barrier: absorb cross-core launch skew here (overlapped
        # with the initial input DMAs) instead of inside the first real a2a
        nc.sync.dma_start(out=bar_in[:], in_=ident[0:NCORES, 0:16])
        nc.gpsimd.collective_compute(
            "AllToAll", mybir.AluOpType.bypass, replica_groups=GROUPS,
            ins=[bar_in.opt()], outs=[bar_out.opt()])

        def a2a(in_t, out_t):
            nc.gpsimd.collective_compute(
                "AllToAll", mybir.AluOpType.bypass, replica_groups=GROUPS,
                ins=[in_t.opt()], outs=[out_t.opt()])

        # ---------------- shared helpers ----------------
        def proj_tile(dst, w_sb, bias, src_sb, jj):
            # dst[:, 512*jj:...] = (w_chunk^T @ srcT)[dcol, tok] + bias.
            # src_sb is one batch half [128, KD*SB]; jj in 0..3.
            ps = ps_aux.tile([128, 512], F32, tag="psaux", name="psp")
            for k in range(KD):
                nc.tensor.matmul(
                    ps[:],
                    lhsT=w_sb[:, 128 * k:128 * (k + 1)],
                    rhs=src_sb[:, k * SB + 512 * jj:k * SB + 512 * jj + 512],
                    start=(k == 0), stop=(k == KD - 1))
            nc.vector.tensor_scalar_add(dst[:, 512 * jj:512 * (jj + 1)],
                                        ps[:], bias[:])

        def vaug_ones(vaug_sb, key_mask_col=None):
            # write only the 65th (ones/denominator) column of each group:
            # the 0:64 columns are fully overwritten by the chunk transposes
            v = vaug_sb.rearrange("p (h c d) -> p h c d", h=HPC, c=NBC)
            if key_mask_col is None:
                nc.vector.memset(v[:, :, :, 64:65], 1.0)
            else:
                for h in range(HPC):
                    nc.vector.tensor_copy(
                        v[:, h, :, 64],
                        mb[:, key_mask_col:key_mask_col + NBC])

        def vaug_chunk_tr(vT_sb, vaug_sb, c, key_mask_col=None):
            # PE-transpose V chunk c ([128 (h,d), 128 tok] -> [128 tok,
            # (h,d)]) and scatter into vaug's per-head 65-column groups;
            # key_mask zeroes dropped keys (per-partition scalar, fused
            # into the scatter copy).
            ptr = ps_aux.tile([128, 128], BF16, tag="psaux", name="ptr")
            nc.tensor.transpose(ptr[:], vT_sb[:, 128 * c:128 * (c + 1)],
                                ident[:])
            dst = vaug_sb.rearrange("p (h c d) -> p h c d", h=HPC, c=NBC)
            src = ptr.rearrange("p (h d) -> p h d", h=HPC)
            if key_mask_col is None:
                nc.vector.tensor_copy(dst[:, :, c, 0:64], src)
            else:
                nc.vector.tensor_scalar_mul(
                    dst[:, :, c, 0:64], src,
                    mb[:, key_mask_col + c:key_mask_col + c + 1])

        def vaug_slice(vaug_sb, h, c):
            base = 65 * (NBC * h + c)
            return vaug_sb[:, base:base + 65]

        def rsqrt_of(veps):
            # [128, 1] f32 -> rstd = (veps)^-0.5 via exp-bits seed (~1.5%
            # err) + one fused Newton step (3 DVE ops, ~3e-3 max err)
            bfi = lns.tile([128, 1], F32, tag="bfi")
            nc.vector.tensor_copy(bfi[:], veps.bitcast(I32))
            y = lns.tile([128, 1], F32, tag="rsy")
            nc.scalar.activation(y[:], bfi[:], EXP, scale=RS_SCALE,
                                 bias=rsb[:])
            t2 = lns.tile([128, 1], F32, tag="rst")
            nc.vector.scalar_tensor_tensor(t2[:], veps, y[:], y[:],
                                           op0=MULT, op1=MULT)
            c2 = lns.tile([128, 1], F32, tag="rsc")
            nc.vector.tensor_scalar(c2[:], t2[:], -0.5, 1.5,
                                    op0=MULT, op1=ADD)
            yn = lns.tile([128, 1], F32, tag="rsn")
            nc.vector.tensor_scalar(yn[:], c2[:], y[:], 0.0,
                                    op0=MULT, op1=ADD)
            return yn

        def ln_inplace(pre, dst):
            bnst = lns.tile([128, 12], F32, tag="bnst")
            nc.vector.bn_stats(bnst[:, 0:6], pre[:, 0:512])
            nc.vector.bn_stats(bnst[:, 6:12], pre[:, 512:1024])
            stats = lns.tile([128, 2], F32, tag="stats")
            nc.vector.bn_aggr(stats[:], bnst[:])
            veps = lns.tile([128, 1], F32, tag="veps")
            nc.vector.tensor_scalar_add(veps[:], stats[:, 1:2], EPS)
            rstd = rsqrt_of(veps[:])
            nc.vector.tensor_scalar(dst[:], pre[:], stats[:, 0:1], rstd[:],
                                    op0=SUB, op1=MULT)

        def attention(pools, QT_sb, KT_sb, vaug_sb, stage_sb, blocks,
                      fillers=None, filler_delay=0):
            # Software-pipelined attention over ONE batch. Per work unit
            # (t, chunk-group): scores for GW ki-chunks of both heads land
            # in per-head psums (h0 on PE rows 0-63, h1 on rows 64-127 —
            # row tiling), one Exp per head covers the group. The attnV
            # matmuls of the PREVIOUS unit are emitted after this unit's
            # scores so the PE queue never stalls on the exp; softmax
            # divisions are delayed one more unit.
            ps_s, ps_o, ptp, smalls = pools
            units = []
            for t in range(NBT):
                clist = [c for c in range(NBC) if (t, c) in blocks]
                groups = [clist[i:i + GW] for i in range(0, len(clist), GW)]
                for gi, grp in enumerate(groups):
                    units.append((t, grp, gi == 0, gi == len(groups) - 1))

            po = {}          # live accumulation psums, per head
            pending = None   # (unit, pt4 dict)
            div_q = []       # (t, po) awaiting division emission

            def emit_attnv(unit, pt4):
                t, grp, isfirst, islast = unit
                if isfirst:
                    for h in range(HPC):
                        po[h] = ps_o.tile([65, 512], F32, tag=f"po{h}",
                                          name=f"po{h}")
                for ci, c in enumerate(grp):
                    kind = blocks[(t, c)]
                    for h in range(HPC):
                        rhs = pt4[h][:, 512 * ci:512 * (ci + 1)]
                        if kind != 'full':
                            idx = kind[1]
                            nc.vector.tensor_tensor(
                                rhs, rhs, cm[:, 512 * idx:512 * (idx + 1)],
                                op=MULT)
                        nc.tensor.matmul(
                            po[h][:], lhsT=vaug_slice(vaug_sb, h, c),
                            rhs=rhs, start=(isfirst and ci == 0),
                            stop=(islast and ci == len(grp) - 1))
                if islast:
                    div_q.append((t, dict(po)))

            def emit_division():
                t, po_t = div_q.pop(0)
                for h in range(HPC):
                    osb = smalls.tile([65, 512], F32, tag="osb")
                    nc.vector.tensor_copy(osb[:], po_t[h][:])
                    recip = smalls.tile([1, 512], F32R, tag="recip")
                    with nc.allow_low_precision(reason="softmax recip row"):
                        nc.vector.reciprocal(recip[:], osb[64:65, :])
                    pb = ps_aux.tile([65, 512], F32, tag="psaux", name="pb")
                    nc.tensor.matmul(pb[:], lhsT=ones65[:].bitcast(F32R),
                                     rhs=recip[:], start=True, stop=True)
                    pbsb = smalls.tile([64, 512], F32, tag="pbsb")
                    nc.vector.tensor_copy(pbsb[:], pb[0:64, :])
                    nc.vector.tensor_tensor(
                        stage_sb[0:64,
                                 h * SB + 512 * t:h * SB + 512 * t + 512],
                        osb[0:64, :], pbsb[:], op=MULT)

            fillers = list(fillers) if fillers else []
            unit_no = [0]

            def emit_warmer():
                # Keep PE array activity high: attention's half-array matmuls
                # don't generate enough PE activity for the HAM to ever
                # unthrottle the clock. Emit either a unit of real full-array
                # work (pipelined filler) or a dummy matmul.
                if fillers and unit_no[0] >= filler_delay:
                    fillers.pop(0)()
                    return
                pd = ps_aux.tile([128, 512], F32, tag="psaux", name="pd")
                nc.tensor.matmul(pd[:], lhsT=KT_sb[:, 0:128],
                                 rhs=KT_sb[:, 1024:1536],
                                 start=True, stop=True)

            for unit in units:
                t, grp, isfirst, islast = unit
                ps4 = {h: ps_s.tile([128, GW * 512], F32,
                                    tag=f"ps4h{h}", name=f"ps4h{h}")
                       for h in range(HPC)}
                for ci, c in enumerate(grp):
                    for h in range(HPC):
                        nc.tensor.matmul(
                            ps4[h][:, 512 * ci:512 * (ci + 1)],
                            lhsT=KT_sb[64 * h:64 * (h + 1),
                                       128 * c:128 * c + 128],
                            rhs=QT_sb[64 * h:64 * (h + 1),
                                      512 * t:512 * t + 512],
                            start=True, stop=True)
                pt4 = {}
                for h in range(HPC):
                    pt4[h] = ptp.tile([128, GW * 512], BF16,
                                      tag=f"pt4h{h}", name=f"pt4h{h}")
                    nc.scalar.activation(pt4[h][:], ps4[h][:], EXP,
                                         scale=0.125)
                emit_warmer()
                if pending is not None:
                    emit_attnv(*pending)
                    if div_q and len(div_q) > 1:
                        emit_division()
                emit_warmer()
                pending = (unit, pt4)
                unit_no[0] += 1
            emit_attnv(*pending)
            while div_q:
                emit_division()
            for f in fillers:
                f()

        def stage_to_a2a(stage_sb, a2a_in_t):
            # on the gpsimd (SWDGE) ring: this DMA waits on the attention
            # divisions, and the next gpsimd op is the a2a trigger that
            # needs it anyway — keeps the sync ring free for prefetches
            for h in range(HPC):
                o = a2a_in_t.rearrange("(j r) s -> r j s", r=128)
                nc.gpsimd.dma_start(
                    out=o[64 * h:64 * (h + 1)],
                    in_=stage_sb.rearrange("r (h j s) -> r h j s",
                                           h=HPC, j=NCORES)[:, h])

        def wo_ln_closures(pool, pget, at_sb, wo_sb, resid_of, outm_of,
                           outT_sb, post=None):
            # Returns filler closures computing, for m in {0,1}:
            # outm_of(m) = LN(resid_of(m) + at^T @ wo), plus the bf16
            # PE-transpose of the LN output into outT_sb, then post().
            # Order: all 4 matmul closures first, then the two LN chains
            # (so their serial DVE/ACT latency overlaps PE work), then the
            # PE transposes.
            closures = []
            pre_box = {}
            obf_box = {}

            def mk_mm(m, eh):
                def f():
                    if pget is not None:
                        pget()
                    if m not in pre_box:
                        pre_box[m] = pool.tile([128, 1024], F32, tag="pre",
                                               name=f"pre{m}")
                    pre = pre_box[m]
                    pw = ps_aux.tile([128, 512], F32, tag="psaux", name="pw")
                    for dc in range(KD):
                        nc.tensor.matmul(
                            pw[:],
                            lhsT=at_sb[:, TB * dc + 128 * m:
                                       TB * dc + 128 * m + 128],
                            rhs=wo_sb[:, 1024 * dc + 512 * eh:
                                      1024 * dc + 512 * eh + 512],
                            start=(dc == 0), stop=(dc == KD - 1))
                    nc.vector.tensor_tensor(
                        pre[:, 512 * eh:512 * (eh + 1)], pw[:],
                        resid_of(m)[:, 512 * eh:512 * (eh + 1)], op=ADD)
                return f

            def mk_ln(m):
                def f():
                    pre = pre_box.pop(m)
                    outm = outm_of(m)
                    ln_inplace(pre, outm)
                    obf = pool.tile([128, 1024], BF16, tag="obf",
                                    name=f"obf{m}")
                    nc.vector.tensor_copy(obf[:], outm)
                    obf_box[m] = obf
                return f

            def mk_tr(m):
                def f():
                    obf = obf_box.pop(m)
                    for j in range(KD):
                        ptr = ps_aux.tile([128, 128], BF16, tag="psaux",
                                          name="ptr2")
                        nc.tensor.transpose(ptr[:],
                                            obf[:, 128 * j:128 * (j + 1)],
                                            ident[:])
                        nc.vector.tensor_copy(
                            outT_sb[:, TB * j + 128 * m:
                                    TB * j + 128 * m + 128], ptr[:])
                    if post is not None and m == 1:
                        post()
                return f

            for m in range(2):
                closures.append(mk_mm(m, 0))
                closures.append(mk_mm(m, 1))
            for m in range(2):
                closures.append(mk_ln(m))
            for m in range(2):
                closures.append(mk_tr(m))
            return closures

        def q2_closures(pool, wq2_sb, bq2_sb, outT_sb, qt2_box):
            closures = []

            def mk(j):
                def f():
                    if "t" not in qt2_box:
                        qt2_box["t"] = pool.tile([128, KD * TB], BF16,
                                                 tag="qt2", name="qt2")
                    qt2 = qt2_box["t"]
                    pq = ps_aux.tile([128, TB], F32, tag="psaux", name="pq")
                    for k in range(KD):
                        nc.tensor.matmul(
                            pq[:],
                            lhsT=wq2_sb[:, 1024 * j + 128 * k:
                                        1024 * j + 128 * k + 128],
                            rhs=outT_sb[:, TB * k:TB * (k + 1)],
                            start=(k == 0), stop=(k == KD - 1))
                    nc.vector.tensor_scalar_add(
                        qt2[:, TB * j:TB * (j + 1)], pq[:],
                        bq2_sb[:, j:j + 1])
                return f

            for j in range(KD):
                closures.append(mk(j))
            return closures

        def ffn_w1_closures(pool, w1s_pool, b1f_sb, outT_sb, hT_box, tag):
            closures = []

            def mk(fc):
                def f():
                    if "t" not in hT_box:
                        hT_box["t"] = pool.tile([128, FC * TB], BF16,
                                                tag=tag, name=tag)
                    hT = hT_box["t"]
                    w1t = w1s_pool.tile([128, KD * 128], BF16, tag="w1s")
                    nc.sync.dma_start(out=w1t[:],
                                      in_=w1_d[:, 1024 * fc:1024 * (fc + 1)])
                    ph = ps_aux.tile([128, TB], F32, tag="psaux", name="ph")
                    for k in range(KD):
                        nc.tensor.matmul(
                            ph[:],
                            lhsT=w1t[:, 128 * k:128 * (k + 1)],
                            rhs=outT_sb[:, TB * k:TB * (k + 1)],
                            start=(k == 0), stop=(k == KD - 1))
                    nc.vector.tensor_scalar(hT[:, TB * fc:TB * (fc + 1)],
                                            ph[:], b1f_sb[:, fc:fc + 1], 0.0,
                                            op0=ADD, op1=MAX)
                return f

            for fc in range(FC):
                closures.append(mk(fc))
            return closures

        out1v = out1.rearrange("p (m e) -> p m e", m=4)
        out2v = out2.rearrange("p (m e) -> p m e", m=4)

        # =====================================================================
        # p3k: cross-attn K/V/Q tensors that survive into attn2 phases
        with tc.tile_pool(name="p3k", bufs=1) as p3k:
            KT2 = [p3k.tile([128, SB], BF16, tag=f"KT2{b}", name=f"KT2{b}")
                   for b in range(B)]
            vaug2 = [p3k.tile([128, HPC * NBC * 65], BF16, tag=f"vaug2{b}",
                              name=f"vaug2{b}") for b in range(B)]
            QT2 = [p3k.tile([128, SB], BF16, tag=f"QT2{b}", name=f"QT2{b}")
                   for b in range(B)]

            # ============ phases 1-2: QKV1 + self attention ==================
            with tc.tile_pool(name="pA", bufs=1) as pA, \
                 tc.tile_pool(name="p12s", bufs=2) as p12s:
                wq1 = pA.tile([128, KD * 128], BF16, tag="wq1")
                wk1 = pA.tile([128, KD * 128], BF16, tag="wk1")
                wv1 = pA.tile([128, KD * 128], BF16, tag="wv1")
                nc.sync.dma_start(out=wq1[:], in_=wq1_d[:])
                nc.sync.dma_start(out=wk1[:], in_=wk1_d[:])
                nc.sync.dma_start(out=wv1[:], in_=wv1_d[:])
                bq1 = pA.tile([128, 1], F32, tag="bq1")
                bk1 = pA.tile([128, 1], F32, tag="bk1")
                bv1 = pA.tile([128, 1], F32, tag="bv1")
                nc.sync.dma_start(out=bq1[:], in_=bq1_d[:])
                nc.sync.dma_start(out=bk1[:], in_=bk1_d[:])
                nc.sync.dma_start(out=bv1[:], in_=bv1_d[:])
                wk2 = pA.tile([128, KD * 128], BF16, tag="wk2")
                wv2 = pA.tile([128, KD * 128], BF16, tag="wv2")
                nc.sync.dma_start(out=wk2[:], in_=wk2_d[:])
                nc.sync.dma_start(out=wv2[:], in_=wv2_d[:])
                bk2 = pA.tile([128, 1], F32, tag="bk2")
                bv2 = pA.tile([128, 1], F32, tag="bv2")
                nc.sync.dma_start(out=bk2[:], in_=bk2_d[:])
                nc.sync.dma_start(out=bv2[:], in_=bv2_d[:])

                xTd = xT_d.rearrange("(k p) t -> p k t", p=128)
                encTd = encT_d.rearrange("(k p) t -> p k t", p=128)

                def load_half(tl, src_view, b):
                    # chunked per-512-token-tile DMA of one batch half
                    v = tl.rearrange("p (k t) -> p k t", k=KD)
                    for jj in range(4):
                        nc.sync.dma_start(
                            out=v[:, :, 512 * jj:512 * (jj + 1)],
                            in_=src_view[:, :, SB * b + 512 * jj:
                                         SB * b + 512 * jj + 512])

                # xT halves share one 32KB slot; the b1 half's DMA waits
                # for the b0 projections to finish reading the slot.
                xTh0 = pA.tile([128, KD * SB], BF16, tag="xTh", name="xTh0")
                load_half(xTh0, xTd, 0)
                encTh0 = pA.tile([128, KD * SB], BF16, tag="encTh",
                                 name="encTh0")
                load_half(encTh0, encTd, 0)

                QT = [pA.tile([128, SB], BF16, tag=f"QT{b}", name=f"QT{b}")
                      for b in range(B)]
                KT = [pA.tile([128, SB], BF16, tag=f"KT{b}", name=f"KT{b}")
                      for b in range(B)]
                vT1 = pA.tile([128, SB], BF16, tag="vTs", name="vT1")
                vaug1 = [pA.tile([128, HPC * NBC * 65], BF16,
                                 tag=f"vaug1{b}", name=f"vaug1{b}")
                         for b in range(B)]
                stage1 = [pA.tile([64, HPC * SB], BF16, tag=f"stage1{b}",
                                  name=f"stage1{b}") for b in range(B)]


                # QKV1(b0) directly
                for jj in range(4):
                    proj_tile(vT1, wv1, bv1, xTh0, jj)
                nc.vector.memset(vaug1[0][:], 1.0)
                for c in range(NBC):
                    vaug_chunk_tr(vT1, vaug1[0], c)
                for jj in range(4):
                    proj_tile(QT[0], wq1, bq1, xTh0, jj)
                for jj in range(4):
                    proj_tile(KT[0], wk1, bk1, xTh0, jj)

                # xT b1 half: slot reuse waits for the QKV1(b0) reads, DMA
                # flies while attn1(b0) computes
                xTh1 = pA.tile([128, KD * SB], BF16, tag="xTh", name="xTh1")
                load_half(xTh1, xTd, 1)

                # attn1(b0) with QKV1(b1) as fillers
                f_a1b0 = []
                for jj in range(4):
                    f_a1b0.append(lambda jj=jj: proj_tile(vT1, wv1, bv1,
                                                          xTh1, jj))
                f_a1b0.append(lambda: nc.vector.memset(vaug1[1][:], 1.0))
                for c0 in range(0, NBC, 4):
                    def fv(c0=c0):
                        for c in range(c0, c0 + 4):
                            vaug_chunk_tr(vT1, vaug1[1], c)
                    f_a1b0.append(fv)
                for jj in range(4):
                    f_a1b0.append(lambda jj=jj: proj_tile(QT[1], wq1, bq1,
                                                          xTh1, jj))
                for jj in range(4):
                    f_a1b0.append(lambda jj=jj: proj_tile(KT[1], wk1, bk1,
                                                          xTh1, jj))

                with tc.tile_pool(name="ps_s1", bufs=1, space="PSUM") as ps_s, \
                     tc.tile_pool(name="ps_o1", bufs=1, space="PSUM") as ps_o, \
                     tc.tile_pool(name="pt1", bufs=2) as ptp:
                    attention((ps_s, ps_o, ptp, p12s),
                              QT[0], KT[0], vaug1[0], stage1[0], self_blocks,
                              fillers=f_a1b0, filler_delay=3)
                stage_to_a2a(stage1[0], a11i[0])
                a2a(a11i[0], a11o[0])

                # vT2 shares vT1's slot: vT1 dies once vaug1[1] is built
                # (a filler of attn1(b0), emitted above)
                vT2 = pA.tile([128, SB], BF16, tag="vTs", name="vT2")

                # attn1(b1) with cross K/V prep as fillers; the encT b1
                # half-load is itself a filler (its slot-reuse waits for
                # all b0 readers, which precede it in the list)
                ench = {0: encTh0}

                def load_ench1():
                    ench[1] = pA.tile([128, KD * SB], BF16, tag="encTh",
                                      name="encTh1")
                    load_half(ench[1], encTd, 1)

                f_a1b1 = []
                for b in range(B):
                    if b == 1:
                        f_a1b1.append(load_ench1)
                    for jj in range(4):
                        f_a1b1.append(lambda b=b, jj=jj: proj_tile(
                            vT2, wv2, bv2, ench[b], jj))
                    f_a1b1.append(lambda b=b: nc.vector.memset(
                        vaug2[b][:], 1.0))
                    for c0 in range(0, NBC, 4):
                        def fv2(b=b, c0=c0):
                            for c in range(c0, c0 + 4):
                                vaug_chunk_tr(vT2, vaug2[b], c,
                                              key_mask_col=NBC * b)
                        f_a1b1.append(fv2)
                    for jj in range(4):
                        f_a1b1.append(lambda b=b, jj=jj: proj_tile(
                            KT2[b], wk2, bk2, ench[b], jj))

                with tc.tile_pool(name="ps_s2", bufs=1, space="PSUM") as ps_s, \
                     tc.tile_pool(name="ps_o2", bufs=1, space="PSUM") as ps_o, \
                     tc.tile_pool(name="pt2", bufs=2) as ptp:
                    attention((ps_s, ps_o, ptp, p12s),
                              QT[1], KT[1], vaug1[1], stage1[1], self_blocks,
                              fillers=f_a1b1, filler_delay=0)
                stage_to_a2a(stage1[1], a11i[1])
                # NOTE: a2a1(b1)'s trigger is deferred until after a2aq(b0)
                # so attn2(b0) can start as early as possible; nothing needs
                # a2a1(b1) before the W1(b1) fillers inside attn2(b0).
            # pA closed

            # ============ phase 3: wo1+LN1+q2 (b0 direct, b1 as fillers) ====
            with tc.tile_pool(name="pW1", bufs=1) as pW1, \
                 tc.tile_pool(name="pW1s", bufs=2) as pW1s:
                wo1 = pW1.tile([128, KD * 1024], BF16, tag="wo1")
                nc.sync.dma_start(out=wo1[:], in_=wo1_d[:])
                wq2 = pW1.tile([128, KD * KD * 128], BF16, tag="wq2")
                nc.sync.dma_start(out=wq2[:], in_=wq2_d[:])
                bq2 = pW1.tile([128, KD], F32, tag="bq2")
                nc.sync.dma_start(out=bq2[:], in_=bq2_d[:])
                bo2 = pW1.tile([128, 1024], F32, tag="bo2")
                nc.sync.dma_start(out=bo2[:], in_=bo2_d[:])
                outT1 = [pW1.tile([128, KD * TB], BF16, tag=f"outT1{b}",
                                  name=f"outT1{b}") for b in range(B)]
                xow = pW1.tile([128, 4 * 1024], F32, tag="xow")
                nc.sync.dma_start(
                    out=xow.rearrange("p (m e) -> p m e", m=4),
                    in_=xown_d.rearrange("(m p) e -> p m e", p=128))
                xowv = xow.rearrange("p (m e) -> p m e", m=4)
                at1 = [pW1.tile([128, KD * TB], BF16, tag=f"at1{b}",
                                name=f"at1{b}") for b in range(B)]
                at1_loaded = [False, False]

                def load_at1(b):
                    # gpsimd ring: rides right behind the a2a it waits on
                    if not at1_loaded[b]:
                        at1_loaded[b] = True
                        nc.gpsimd.dma_start(
                            out=at1[b].rearrange("p (dc s) -> p dc s", dc=KD),
                            in_=a11o[b].rearrange("(dc p) s -> p dc s",
                                                  p=128))

                qt2_box = [{}, {}]

                def post_bias1(b):
                    # fold bo2 into out1 (after outT1 transpose, before wo2)
                    for m in range(2):
                        mm = 2 * b + m
                        nc.vector.tensor_tensor(
                            out1v[:, mm], out1v[:, mm], bo2[:], op=ADD)

                def finish_q2(b):
                    qt2 = qt2_box[b]["t"]
                    nc.gpsimd.dma_start(
                        out=a1qi[b].rearrange("(j p) s -> p j s", p=128),
                        in_=qt2.rearrange("p (j s) -> p j s", j=KD))
                    a2a(a1qi[b], a1qo[b])
                    nc.gpsimd.dma_start(
                        out=QT2[b].rearrange("p (i s) -> p i s", i=NCORES),
                        in_=a1qo[b].rearrange("(i p) s -> p i s", p=128))

                # b0 direct
                load_at1(0)
                w1cl = wo_ln_closures(
                    pW1s, None, at1[0], wo1,
                    resid_of=lambda m: xowv[:, m],
                    outm_of=lambda m: out1v[:, m],
                    outT_sb=outT1[0], post=lambda: post_bias1(0))
                for f in w1cl:
                    f()
                for f in q2_closures(pW1, wq2, bq2, outT1[0], qt2_box[0]):
                    f()
                finish_q2(0)
                # trigger a2a1(b1) only now: the cc stream is serial, and
                # a2aq(b0) gates attn2(b0) while a2a1(b1) is only needed by
                # the W1(b1) fillers ~10 units into attn2(b0)
                a2a(a11i[1], a11o[1])

                # attn2(b0) with W1(b1) + q2(b1) as fillers
                f_a2b0 = wo_ln_closures(
                    pW1s, lambda: load_at1(1), at1[1], wo1,
                    resid_of=lambda m: xowv[:, 2 + m],
                    outm_of=lambda m: out1v[:, 2 + m],
                    outT_sb=outT1[1], post=lambda: post_bias1(1))
                f_a2b0 += q2_closures(pW1, wq2, bq2, outT1[1], qt2_box[1])
                f_a2b0.append(lambda: finish_q2(1))

                with tc.tile_pool(name="pX", bufs=1) as pX, \
                     tc.tile_pool(name="ps_s3", bufs=1, space="PSUM") as ps_s, \
                     tc.tile_pool(name="ps_o3", bufs=1, space="PSUM") as ps_o, \
                     tc.tile_pool(name="pt3", bufs=2) as ptp:
                    stage2_b0 = pX.tile([64, HPC * SB], BF16, tag="stage2")
                    attention((ps_s, ps_o, ptp, pW1s),
                              QT2[0], KT2[0], vaug2[0], stage2_b0,
                              CROSS_BLOCKS, fillers=f_a2b0, filler_delay=10)
                    stage_to_a2a(stage2_b0, a12i[0])
                a2a(a12i[0], a12o[0])
            # pW1 closed

            # ============ phase 4: attn2(b1) + wo2/FFN-w1(b0) fillers =======
            with tc.tile_pool(name="p78", bufs=1) as p78, \
                 tc.tile_pool(name="p78s", bufs=2) as p78s, \
                 tc.tile_pool(name="w1str", bufs=3) as w1s_pool, \
                 tc.tile_pool(name="w2str", bufs=3) as w2s_pool:
                wo2 = p78.tile([128, KD * 1024], BF16, tag="wo2")
                nc.sync.dma_start(out=wo2[:], in_=wo2_d[:])
                b1f = p78.tile([128, FC], F32, tag="b1f")
                nc.sync.dma_start(out=b1f[:], in_=b1f_d[:])
                b2 = p78.tile([128, 1024], F32, tag="b2")
                nc.sync.dma_start(out=b2[:], in_=b2_d[:])

                at2 = [p78.tile([128, KD * TB], BF16, tag=f"at2{b}",
                                name=f"at2{b}") for b in range(B)]
                at2_loaded = [False, False]

                def load_at2(b):
                    # gpsimd ring: rides right behind the a2a it waits on
                    if not at2_loaded[b]:
                        at2_loaded[b] = True
                        nc.gpsimd.dma_start(
                            out=at2[b].rearrange("p (dc s) -> p dc s", dc=KD),
                            in_=a12o[b].rearrange("(dc p) s -> p dc s",
                                                  p=128))

                outT2 = [p78.tile([128, KD * TB], BF16, tag=f"outT2{b}",
                                  name=f"outT2{b}") for b in range(B)]
                hT_box = [{}, {}]

                def post_bias2(b):
                    # fold b2 into out2 (after outT2 transpose, before FFN w2)
                    for m in range(2):
                        mm = 2 * b + m
                        nc.vector.tensor_tensor(
                            out2v[:, mm], out2v[:, mm], b2[:], op=ADD)

                f_a2b1 = wo_ln_closures(
                    p78s, lambda: load_at2(0), at2[0], wo2,
                    resid_of=lambda m: out1v[:, m],
                    outm_of=lambda m: out2v[:, m],
                    outT_sb=outT2[0], post=lambda: post_bias2(0))
                f_a2b1 += ffn_w1_closures(p78, w1s_pool, b1f, outT2[0],
                                          hT_box[0], tag="hT")

                with tc.tile_pool(name="pX2", bufs=1) as pX2, \
                     tc.tile_pool(name="ps_s4", bufs=1, space="PSUM") as ps_s, \
                     tc.tile_pool(name="ps_o4", bufs=1, space="PSUM") as ps_o, \
                     tc.tile_pool(name="pt4", bufs=2) as ptp:
                    stage2_b1 = pX2.tile([64, HPC * SB], BF16, tag="stage2b")
                    attention((ps_s, ps_o, ptp, p78s),
                              QT2[1], KT2[1], vaug2[1], stage2_b1,
                              CROSS_BLOCKS, fillers=f_a2b1, filler_delay=8)
                    stage_to_a2a(stage2_b1, a12i[1])
                a2a(a12i[1], a12o[1])

                # ============ phase 5: FFN-w2+LN3(b0); then all of b1 =======
                def ffn_w2_ln3(b):
                    hT = hT_box[b]["t"]
                    with tc.tile_pool(name=f"ps_f{b}", bufs=1,
                                      space="PSUM") as ps_f:
                        py = {(m, eh): ps_f.tile([128, 512], F32,
                                                 tag=f"py{m}{eh}",
                                                 name=f"py{m}{eh}")
                              for m in range(2) for eh in range(2)}
                        for fc in range(FC):
                            w2t = w2s_pool.tile([128, 1024], BF16, tag="w2s")
                            nc.sync.dma_start(
                                out=w2t[:],
                                in_=w2_d[:, 1024 * fc:1024 * (fc + 1)])
                            for m in range(2):
                                for eh in range(2):
                                    nc.tensor.matmul(
                                        py[(m, eh)][:],
                                        lhsT=hT[:, TB * fc + 128 * m:
                                                TB * fc + 128 * m + 128],
                                        rhs=w2t[:, 512 * eh:512 * (eh + 1)],
                                        start=(fc == 0), stop=(fc == FC - 1))
                        pres = {}
                        for m in range(2):
                            mm = 2 * b + m
                            pre = p78s.tile([128, 1024], F32, tag="pref",
                                            name=f"pref{m}")
                            for eh in range(2):
                                nc.vector.tensor_tensor(
                                    pre[:, 512 * eh:512 * (eh + 1)],
                                    py[(m, eh)][:],
                                    out2v[:, mm, 512 * eh:512 * (eh + 1)],
                                    op=ADD)
                            pres[m] = pre
                        for m in range(2):
                            outf = p78s.tile([128, 1024], F32, tag="outf",
                                             name=f"outf{m}")
                            ln_inplace(pres[m], outf)
                            nc.sync.dma_start(
                                out=out_d[256 * b + 128 * m:
                                          256 * b + 128 * m + 128, :],
                                in_=outf[:])

                ffn_w2_ln3(0)

                # b1 drain: wo2(b1) + LN2(b1) + FFN(b1)
                w2cl = wo_ln_closures(
                    p78s, lambda: load_at2(1), at2[1], wo2,
                    resid_of=lambda m: out1v[:, 2 + m],
                    outm_of=lambda m: out2v[:, 2 + m],
                    outT_sb=outT2[1], post=lambda: post_bias2(1))
                for f in w2cl:
                    f()
                for f in ffn_w1_closures(p78, w1s_pool, b1f, outT2[1],
                                         hT_box[1], tag="hT"):
                    f()
                ffn_w2_ln3(1)

    nc.compile()
    return nc


def _to_bf(a):
    return np.ascontiguousarray(np.asarray(a, np.float32).astype(BF))


def _rechunk_k(w):
    """[K*128, M] -> [128, K*M] with col k*M + m = w[k*128 + p, m]."""
    K = w.shape[0] // 128
    M = w.shape[1]
    return np.ascontiguousarray(
        w.reshape(K, 128, M).transpose(1, 0, 2).reshape(128, K * M))


def _analyze_self_mask(mask):
    """mask [S, S] (1 = disallowed), orientation [q, k].

    Returns blocks dict (t, c) -> 'full' | ('tile', idx), list of unique
    multiplicative tiles [128, 512] (bf16), for a block grid over one batch.
    Blocks where everything is disallowed are omitted.
    """
    add = np.float32(-1e9) * np.asarray(mask, np.float32)
    mult = np.exp(add.T)  # [k, q] multiplicative
    blocks = {}
    tiles = []
    tile_ids = {}
    for t in range(NBT):
        for c in range(NBC):
            sub = mult[128 * c:128 * (c + 1), 512 * t:512 * (t + 1)]
            if not sub.any():
                continue
            if (sub == 1.0).all():
                blocks[(t, c)] = 'full'
                continue
            key = sub.tobytes()
            if key not in tile_ids:
                tile_ids[key] = len(tiles)
                tiles.append(sub.astype(BF))
            blocks[(t, c)] = ('tile', tile_ids[key])
    return blocks, tiles


def kernel(**inputs):
    from concourse.bass_utils import run_bass_kernel_spmd

    x = np.asarray(inputs["x"], np.float32)
    enc = np.asarray(inputs["enc_output"], np.float32)
    lam = np.asarray(inputs["look_ahead_mask"], np.float32)[0, 0]
    pad = np.asarray(inputs["padding_mask"], np.float32)  # [B,1,1,S]

    self_blocks, ctiles = _analyze_self_mask(lam)
    n_ctiles = len(ctiles)
    key = (tuple(sorted(self_blocks.items())), n_ctiles)
    if key not in _PROG_CACHE:
        _PROG_CACHE[key] = _build_program(self_blocks, n_ctiles)
    nc = _PROG_CACHE[key]

    # ---- shared (core-independent) host prep ----
    xf = x.reshape(TOK, D_MODEL)             # flattened batch-major tokens
    encf = enc.reshape(TOK, D_MODEL)
    xT = _to_bf(xf.T)                        # [1024, 4096]
    encT = _to_bf(encf.T)
    if n_ctiles:
        cmask = np.concatenate(ctiles, axis=1)
    else:
        cmask = np.zeros((128, 512), BF)
    cmask = np.ascontiguousarray(cmask)
    # cross-attn key-keep mask per enc token: [128, B*16], col b*16+c
    mb = np.exp(np.float32(-1e9) * pad[:, 0, 0, :]).reshape(B, NBC, 128)
    mb = np.ascontiguousarray(mb.transpose(2, 0, 1).reshape(128, B * NBC)
                              ).astype(np.float32)

    w1f = np.asarray(inputs["ffn_w1"], np.float32)
    # w1 stationary layout: [128, fc*1024 + k*128 + m] = w1[k*128+p, fc*128+m]
    w1r = w1f.reshape(KD, 128, FC, 128).transpose(1, 2, 0, 3)
    w1r = _to_bf(w1r.reshape(128, FC * KD * 128))
    w2r = _to_bf(_rechunk_k(np.asarray(inputs["ffn_w2"], np.float32)))
    # b1 per-partition per-chunk [128, FC]; b2/bo2 pre-broadcast [128, 1024]
    b1 = np.ascontiguousarray(
        np.asarray(inputs["ffn_b1"], np.float32).reshape(FC, 128).T)
    b2 = np.ascontiguousarray(np.broadcast_to(
        np.asarray(inputs["ffn_b2"], np.float32)[None, :], (128, 1024)))

    wo1r = _to_bf(_rechunk_k(np.asarray(inputs["mha1_wo"], np.float32)))
    wo2r = _to_bf(_rechunk_k(np.asarray(inputs["mha2_wo"], np.float32)))
    bo1 = np.asarray(inputs["mha1_bo"], np.float32)
    bo2 = np.ascontiguousarray(np.broadcast_to(
        np.asarray(inputs["mha2_bo"], np.float32)[None, :], (128, 1024)))
    ident = np.eye(128, dtype=np.float32).astype(BF)

    wq2_full = np.asarray(inputs["mha2_wq"], np.float32)
    # wq2 stationary layout: [128, j*1024 + k*128 + m] = wq2[k*128+p, j*128+m]
    wq2r = wq2_full.reshape(KD, 128, KD, 128).transpose(1, 2, 0, 3)
    wq2r = _to_bf(wq2r.reshape(128, KD * KD * 128))
    bq2 = np.asarray(inputs["mha2_bq"], np.float32).reshape(KD, 128)
    bq2 = np.ascontiguousarray(bq2.T).astype(np.float32)  # [128, KD]

    in_maps = []
    for j in range(NCORES):
        hs = slice(128 * j, 128 * (j + 1))       # this core's 2 heads' cols
        xo = np.concatenate([xf[TB * j:TB * (j + 1)],
                             xf[S + TB * j:S + TB * (j + 1)]], axis=0)
        xo = np.ascontiguousarray(xo + bo1[None, :])
        m = {
            "xT": xT, "encT": encT, "x_own": xo,
            "cmask": cmask, "mbias": mb,
            "w1": w1r, "b1": b1, "w2": w2r, "b2": b2,
            "wo1": wo1r, "wo2": wo2r, "bo2": bo2,
            "wq2": wq2r, "bq2": bq2, "ident": ident,
        }
        for pre, name in (("wq1", "mha1_wq"), ("wk1", "mha1_wk"),
                          ("wv1", "mha1_wv"), ("wk2", "mha2_wk"),
                          ("wv2", "mha2_wv")):
            w = np.asarray(inputs[name], np.float32)[:, hs]
            m[pre] = _to_bf(_rechunk_k(w))
        for pre, name in (("bq1", "mha1_bq"), ("bk1", "mha1_bk"),
                          ("bv1", "mha1_bv"), ("bk2", "mha2_bk"),
                          ("bv2", "mha2_bv")):
            bvec = np.asarray(inputs[name], np.float32)[hs]
            m[pre] = np.ascontiguousarray(bvec[:, None])
        in_maps.append(m)

    res = run_bass_kernel_spmd(nc, in_maps, list(range(NCORES)))
    out = np.empty((TOK, D_MODEL), np.float32)
    for j in range(NCORES):
        r = res.results[j]["out"]
        out[TB * j:TB * (j + 1)] = r[0:TB]
        out[S + TB * j:S + TB * (j + 1)] = r[TB:2 * TB]
    return out.reshape(B, S, D_MODEL)


# revision 41
# speedup vs baseline: 1.0836x; 1.0186x over previous
"""Trainium2 Bass kernel for a transformer decoder layer (self-attn + cross-attn + FFN).

Sharding: 8-way tensor parallel over heads for both attentions (2 heads/core);
token-sharded for wo projections, layernorms and FFN with each core owning 256
tokens of EACH batch. Head<->token redistribution uses six half-size (per-batch)
AllToAll collectives. The two batches are independent through the whole layer,
so the schedule is batch-pipelined: every collective flies while the other
batch computes (attn1(b1) covers a2a1(b0); wo1/q2(b1) and wo2+FFN-w1(b0) run as
PE filler work inside the ACT-bound cross-attention phases; etc.).

All matmuls run in bf16 with fp32 PSUM accumulation. Attention keeps the
[feature, token] (transposed) layout throughout: scoresT uses kT-chunk
stationary x qT moving, probs come out as PT[ki, qi] which feeds attnV
directly with V-natural (+ones column) stationary, producing attn^T and the
softmax denominator in one accumulation group. Normalization happens via a
reciprocal row broadcast with a rank-1 fp32r matmul, off the critical path.
The cross-attention padding mask is folded into V by zeroing masked key rows
(incl. the ones column), which removes them from output and denominator.

LayerNorm rstd is computed as exp(scale*bits(var+eps)+bias) (the exponent-bits
log approximation folded into ACT's free affine) polished by two Newton
iterations on DVE — Exp is the only ACT table function in the whole kernel,
avoiding the ~1.3us-per-switch activation-table ping-pong between Exp and Ln.
"""

import sys

TRN_REPO = "/opt/trn_rl_repo"
if TRN_REPO not in sys.path:
    sys.path.insert(0, TRN_REPO)

import numpy as np
import ml_dtypes

D_MODEL = 1024
N_HEADS = 16
DFF = 4096
B, S = 2, 2048
EPS = 1e-6
DEPTH = D_MODEL // N_HEADS  # 64

NCORES = 8
HPC = N_HEADS // NCORES     # heads per core = 2
TOK = B * S                 # 4096 flattened tokens
SB = S                      # tokens per batch = 2048
TB = SB // NCORES           # tokens per core per batch = 256
KD = D_MODEL // 128         # 8 contraction chunks over d_model
FC = DFF // 128             # 32 chunks over dff
NBT = SB // 512             # 4 q-tiles per batch
NBC = SB // 128             # 16 ki-chunks per batch

BF = ml_dtypes.bfloat16

# rsqrt-via-exp-bits constants: rsqrt(v) ~= exp(RS_SCALE*float(bits(v)) + RS_BIAS)
_LN2 = float(np.log(2.0))
RS_SCALE = -0.5 * _LN2 / (1 << 23)
RS_BIAS = 0.5 * _LN2 * (127 + 0.0430357)

_PROG_CACHE = {}


def _build_program(self_blocks, n_ctiles):
    """Emit the SPMD Bass program (same program on all 8 cores)."""
    import concourse.bacc as bacc
    import concourse.mybir as mybir
    from concourse import tile

    F32 = mybir.dt.float32
    F32R = mybir.dt.float32r
    I32 = mybir.dt.int32
    BF16 = mybir.dt.bfloat16
    EXP = mybir.ActivationFunctionType.Exp
    ADD = mybir.AluOpType.add
    MULT = mybir.AluOpType.mult
    SUB = mybir.AluOpType.subtract
    MAX = mybir.AluOpType.max

    nc = bacc.Bacc("TRN2", target_bir_lowering=False, debug=False,
                   num_devices=NCORES)

    def din(name, shape, dt=BF16):
        return nc.dram_tensor(name, shape, dt, kind="ExternalInput")

    xT_d = din("xT", [D_MODEL, TOK])
    encT_d = din("encT", [D_MODEL, TOK])
    xown_d = din("x_own", [2 * TB, D_MODEL], F32)   # bo1 pre-folded on host
    wq1_d = din("wq1", [128, KD * 128])
    wk1_d = din("wk1", [128, KD * 128])
    wv1_d = din("wv1", [128, KD * 128])
    bq1_d = din("bq1", [128, 1], F32)
    bk1_d = din("bk1", [128, 1], F32)
    bv1_d = din("bv1", [128, 1], F32)
    wo1_d = din("wo1", [128, KD * 1024])
    wq2_d = din("wq2", [128, KD * KD * 128])
    bq2_d = din("bq2", [128, KD], F32)
    wk2_d = din("wk2", [128, KD * 128])
    wv2_d = din("wv2", [128, KD * 128])
    bk2_d = din("bk2", [128, 1], F32)
    bv2_d = din("bv2", [128, 1], F32)
    wo2_d = din("wo2", [128, KD * 1024])
    bo2_d = din("bo2", [128, 1024], F32)   # pre-broadcast
    w1_d = din("w1", [128, FC * KD * 128])
    b1f_d = din("b1", [128, FC], F32)      # per-partition per-chunk
    w2_d = din("w2", [128, FC * 1024])
    b2_d = din("b2", [128, 1024], F32)     # pre-broadcast
    ident_d = din("ident", [128, 128])
    cm_d = din("cmask", [128, max(n_ctiles, 1) * 512])
    mb_d = din("mbias", [128, B * NBC], F32)
    out_d = nc.dram_tensor("out", [2 * TB, D_MODEL], F32, kind="ExternalOutput")

    CROSS_BLOCKS = {(t, c): 'full' for t in range(NBT) for c in range(NBC)}
    GROUPS = [list(range(NCORES))]
    GW = 2  # ki-chunks per merged exp group

    with tile.TileContext(nc) as tc:
      with tc.tile_pool(name="const", bufs=1) as constp, \
           tc.tile_pool(name="fbuf", bufs=1) as fbuf, \
           tc.tile_pool(name="lns", bufs=2) as lns, \
           tc.tile_pool(name="dram", bufs=1, space="DRAM") as dram, \
           tc.tile_pool(name="ps_aux", bufs=2, space="PSUM") as ps_aux:

        # ---- constants ----
        ones65 = constp.tile([1, 65], F32)
        nc.vector.memset(ones65[:], 1.0)
        rsb = constp.tile([128, 1], F32)
        nc.vector.memset(rsb[:], RS_BIAS)
        ident = constp.tile([128, 128], BF16)
        nc.sync.dma_start(out=ident[:], in_=ident_d[:])
        cm = constp.tile([128, max(n_ctiles, 1) * 512], BF16)
        nc.sync.dma_start(out=cm[:], in_=cm_d[:])
        mb = constp.tile([128, B * NBC], F32)
        nc.sync.dma_start(out=mb[:], in_=mb_d[:])

        # ---- persistent activations ----
        out1 = fbuf.tile([128, 4 * 1024], F32, tag="out1")

        # ---- a2a dram buffers (per batch) ----
        bar_in = dram.tile([NCORES, 16], BF16)
        bar_out = dram.tile([NCORES, 16], BF16)
        a11i = [dram.tile([NCORES * 128, TB], BF16, name=f"a11i{b}")
                for b in range(B)]
        a11o = [dram.tile([NCORES * 128, TB], BF16, name=f"a11o{b}")
                for b in range(B)]
        a1qi = [dram.tile([NCORES * 128, TB], BF16, name=f"a1qi{b}")
                for b in range(B)]
        a1qo = [dram.tile([NCORES * 128, TB], BF16, name=f"a1qo{b}")
                for b in range(B)]
        a12i = [dram.tile([NCORES * 128, TB], BF16, name=f"a12i{b}")
                for b in range(B)]
        a12o = [dram.tile([NCORES * 128, TB], BF16, name=f"a12o{b}")
                for b in range(B)]

        # startup barrier: absorb cross-core launch skew here (overlapped
        # with the initial input DMAs) instead of inside the first real a2a
        nc.sync.dma_start(out=bar_in[:], in_=ident[0:NCORES, 0:16])
        nc.gpsimd.collective_compute(
            "AllToAll", mybir.AluOpType.bypass, replica_groups=GROUPS,
            ins=[bar_in.opt()], outs=[bar_out.opt()])

        def a2a(in_t, out_t):
            nc.gpsimd.collective_compute(
                "AllToAll", mybir.AluOpType.bypass, replica_groups=GROUPS,
                ins=[in_t.opt()], outs=[out_t.opt()])

        # ---------------- shared helpers ----------------
        def proj_tile(dst, w_sb, bias, src_sb, jj):
            # dst[:, 512*jj:...] = (w_chunk^T @ srcT)[dcol, tok] + bias.
            # src_sb is one batch half [128, KD*SB]; jj in 0..3.
            ps = ps_aux.tile([128, 512], F32, tag="psaux", name="psp")
            for k in range(KD):
                nc.tensor.matmul(
                    ps[:],
                    lhsT=w_sb[:, 128 * k:128 * (k + 1)],
                    rhs=src_sb[:, k * SB + 512 * jj:k * SB + 512 * jj + 512],
                    start=(k == 0), stop=(k == KD - 1))
            nc.vector.tensor_scalar_add(dst[:, 512 * jj:512 * (jj + 1)],
                                        ps[:], bias[:])

        def vaug_ones(vaug_sb, key_mask_col=None):
            # write only the 65th (ones/denominator) column of each group:
            # the 0:64 columns are fully overwritten by the chunk transposes
            v = vaug_sb.rearrange("p (h c d) -> p h c d", h=HPC, c=NBC)
            if key_mask_col is None:
                nc.vector.memset(v[:, :, :, 64:65], 1.0)
            else:
                for h in range(HPC):
                    nc.vector.tensor_copy(
                        v[:, h, :, 64],
                        mb[:, key_mask_col:key_mask_col + NBC])

        def vaug_chunk_tr(vT_sb, vaug_sb, c, key_mask_col=None):
            # PE-transpose V chunk c ([128 (h,d), 128 tok] -> [128 tok,
            # (h,d)]) and scatter into vaug's per-head 65-column groups;
            # key_mask zeroes dropped keys (per-partition scalar, fused
            # into the scatter copy).
            ptr = ps_aux.tile([128, 128], BF16, tag="psaux", name="ptr")
            nc.tensor.transpose(ptr[:], vT_sb[:, 128 * c:128 * (c + 1)],
                                ident[:])
            dst = vaug_sb.rearrange("p (h c d) -> p h c d", h=HPC, c=NBC)
            src = ptr.rearrange("p (h d) -> p h d", h=HPC)
            if key_mask_col is None:
                nc.vector.tensor_copy(dst[:, :, c, 0:64], src)
            else:
                nc.vector.tensor_scalar_mul(
                    dst[:, :, c, 0:64], src,
                    mb[:, key_mask_col + c:key_mask_col + c + 1])

        def vaug_slice(vaug_sb, h, c):
            base = 65 * (NBC * h + c)
            return vaug_sb[:, base:base + 65]

        def rsqrt_of(veps):
            # [128, 1] f32 -> rstd = (veps)^-0.5 via exp-bits seed (~1.5%
            # err) + one fused Newton step (3 DVE ops, ~3e-3 max err)
            bfi = lns.tile([128, 1], F32, tag="bfi")
            nc.vector.tensor_copy(bfi[:], veps.bitcast(I32))
            y = lns.tile([128, 1], F32, tag="rsy")
            nc.scalar.activation(y[:], bfi[:], EXP, scale=RS_SCALE,
                                 bias=rsb[:])
            t2 = lns.tile([128, 1], F32, tag="rst")
            nc.vector.scalar_tensor_tensor(t2[:], veps, y[:], y[:],
                                           op0=MULT, op1=MULT)
            c2 = lns.tile([128, 1], F32, tag="rsc")
            nc.vector.tensor_scalar(c2[:], t2[:], -0.5, 1.5,
                                    op0=MULT, op1=ADD)
            yn = lns.tile([128, 1], F32, tag="rsn")
            nc.vector.tensor_scalar(yn[:], c2[:], y[:], 0.0,
                                    op0=MULT, op1=ADD)
            return yn

        def ln_inplace(pre, dst):
            bnst = lns.tile([128, 12], F32, tag="bnst")
            nc.vector.bn_stats(bnst[:, 0:6], pre[:, 0:512])
            nc.vector.bn_stats(bnst[:, 6:12], pre[:, 512:1024])
            stats = lns.tile([128, 2], F32, tag="stats")
            nc.vector.bn_aggr(stats[:], bnst[:])
            veps = lns.tile([128, 1], F32, tag="veps")
            nc.vector.tensor_scalar_add(veps[:], stats[:, 1:2], EPS)
            rstd = rsqrt_of(veps[:])
            nc.vector.tensor_scalar(dst[:], pre[:], stats[:, 0:1], rstd[:],
                                    op0=SUB, op1=MULT)

        def attention(pools, QT_sb, KT_sb, vaug_sb, stage_sb, blocks,
                      fillers=None, filler_delay=0):
            # Software-pipelined attention over ONE batch. Per work unit
            # (t, chunk-group): scores for GW ki-chunks of both heads land
            # in per-head psums (h0 on PE rows 0-63, h1 on rows 64-127 —
            # row tiling), one Exp per head covers the group. The attnV
            # matmuls of the PREVIOUS unit are emitted after this unit's
            # scores so the PE queue never stalls on the exp; softmax
            # divisions are delayed one more unit.
            ps_s, ps_o, ptp, smalls = pools
            units = []
            for t in range(NBT):
                clist = [c for c in range(NBC) if (t, c) in blocks]
                groups = [clist[i:i + GW] for i in range(0, len(clist), GW)]
                for gi, grp in enumerate(groups):
                    units.append((t, grp, gi == 0, gi == len(groups) - 1))

            po = {}          # live accumulation psums, per head
            pending = None   # (unit, pt4 dict)
            div_q = []       # (t, po) awaiting division emission

            def emit_attnv(unit, pt4):
                t, grp, isfirst, islast = unit
                if isfirst:
                    for h in range(HPC):
                        po[h] = ps_o.tile([65, 512], F32, tag=f"po{h}",
                                          name=f"po{h}")
                for ci, c in enumerate(grp):
                    kind = blocks[(t, c)]
                    for h in range(HPC):
                        rhs = pt4[h][:, 512 * ci:512 * (ci + 1)]
                        if kind != 'full':
                            idx = kind[1]
                            nc.vector.tensor_tensor(
                                rhs, rhs, cm[:, 512 * idx:512 * (idx + 1)],
                                op=MULT)
                        nc.tensor.matmul(
                            po[h][:], lhsT=vaug_slice(vaug_sb, h, c),
                            rhs=rhs, start=(isfirst and ci == 0),
                            stop=(islast and ci == len(grp) - 1))
                if islast:
                    div_q.append((t, dict(po)))

            def emit_division():
                t, po_t = div_q.pop(0)
                for h in range(HPC):
                    osb = smalls.tile([65, 512], F32, tag="osb")
                    nc.vector.tensor_copy(osb[:], po_t[h][:])
                    recip = smalls.tile([1, 512], F32R, tag="recip")
                    with nc.allow_low_precision(reason="softmax recip row"):
                        nc.vector.reciprocal(recip[:], osb[64:65, :])
                    pb = ps_aux.tile([65, 512], F32, tag="psaux", name="pb")
                    nc.tensor.matmul(pb[:], lhsT=ones65[:].bitcast(F32R),
                                     rhs=recip[:], start=True, stop=True)
                    pbsb = smalls.tile([64, 512], F32, tag="pbsb")
                    nc.vector.tensor_copy(pbsb[:], pb[0:64, :])
                    nc.vector.tensor_tensor(
                        stage_sb[0:64,
                                 h * SB + 512 * t:h * SB + 512 * t + 512],
                        osb[0:64, :], pbsb[:], op=MULT)

            fillers = list(fillers) if fillers else []
            unit_no = [0]

            def emit_warmer():
                # Keep PE array activity high: attention's half-array matmuls
                # don't generate enough PE activity for the HAM to ever
                # unthrottle the clock. Emit either a unit of real full-array
                # work (pipelined filler) or a dummy matmul.
                if fillers and unit_no[0] >= filler_delay:
                    fillers.pop(0)()
                    return
                pd = ps_aux.tile([128, 512], F32, tag="psaux", name="pd")
                nc.tensor.matmul(pd[:], lhsT=KT_sb[:, 0:128],
                                 rhs=KT_sb[:, 1024:1536],
                                 start=True, stop=True)

            for unit in units:
                t, grp, isfirst, islast = unit
                ps4 = {h: ps_s.tile([128, GW * 512], F32,
                                    tag=f"ps4h{h}", name=f"ps4h{h}")
                       for h in range(HPC)}
                for ci, c in enumerate(grp):
                    for h in range(HPC):
                        nc.tensor.matmul(
                            ps4[h][:, 512 * ci:512 * (ci + 1)],
                            lhsT=KT_sb[64 * h:64 * (h + 1),
                                       128 * c:128 * c + 128],
                            rhs=QT_sb[64 * h:64 * (h + 1),
                                      512 * t:512 * t + 512],
                            start=True, stop=True)
                pt4 = {}
                for h in range(HPC):
                    pt4[h] = ptp.tile([128, GW * 512], BF16,
                                      tag=f"pt4h{h}", name=f"pt4h{h}")
                    nc.scalar.activation(pt4[h][:], ps4[h][:], EXP,
                                         scale=0.125)
                emit_warmer()
                if pending is not None:
                    emit_attnv(*pending)
                    if div_q and len(div_q) > 1:
                        emit_division()
                emit_warmer()
                pending = (unit, pt4)
                unit_no[0] += 1
            emit_attnv(*pending)
            # interleave leftover fillers with the division tail so the PE
            # has work while the serial DVE division chains drain
            while div_q:
                emit_division()
                if fillers:
                    fillers.pop(0)()
            for f in fillers:
                f()

        def stage_to_a2a(stage_sb, a2a_in_t):
            # on the gpsimd (SWDGE) ring: this DMA waits on the attention
            # divisions, and the next gpsimd op is the a2a trigger that
            # needs it anyway — keeps the sync ring free for prefetches
            for h in range(HPC):
                o = a2a_in_t.rearrange("(j r) s -> r j s", r=128)
                nc.gpsimd.dma_start(
                    out=o[64 * h:64 * (h + 1)],
                    in_=stage_sb.rearrange("r (h j s) -> r h j s",
                                           h=HPC, j=NCORES)[:, h])

        def wo_ln_closures(pool, pget, at_sb, wo_sb, resid_of, outm_of,
                           outT_sb, post=None):
            # Returns filler closures computing, for m in {0,1}:
            # outm_of(m) = LN(resid_of(m) + at^T @ wo), plus the bf16
            # PE-transpose of the LN output into outT_sb, then post().
            # Order: all 4 matmul closures first, then the two LN chains
            # (so their serial DVE/ACT latency overlaps PE work), then the
            # PE transposes.
            closures = []
            pre_box = {}
            obf_box = {}

            def mk_mm(m, eh):
                def f():
                    if pget is not None:
                        pget()
                    if m not in pre_box:
                        pre_box[m] = pool.tile([128, 1024], F32, tag="pre",
                                               name=f"pre{m}")
                    pre = pre_box[m]
                    pw = ps_aux.tile([128, 512], F32, tag="psaux", name="pw")
                    for dc in range(KD):
                        nc.tensor.matmul(
                            pw[:],
                            lhsT=at_sb[:, TB * dc + 128 * m:
                                       TB * dc + 128 * m + 128],
                            rhs=wo_sb[:, 1024 * dc + 512 * eh:
                                      1024 * dc + 512 * eh + 512],
                            start=(dc == 0), stop=(dc == KD - 1))
                    nc.vector.tensor_tensor(
                        pre[:, 512 * eh:512 * (eh + 1)], pw[:],
                        resid_of(m)[:, 512 * eh:512 * (eh + 1)], op=ADD)
                return f

            def mk_ln(m):
                def f():
                    pre = pre_box.pop(m)
                    outm = outm_of(m)
                    ln_inplace(pre, outm)
                    obf = pool.tile([128, 1024], BF16, tag="obf",
                                    name=f"obf{m}")
                    nc.vector.tensor_copy(obf[:], outm)
                    obf_box[m] = obf
                return f

            def mk_tr(m):
                def f():
                    obf = obf_box.pop(m)
                    for j in range(KD):
                        ptr = ps_aux.tile([128, 128], BF16, tag="psaux",
                                          name="ptr2")
                        nc.tensor.transpose(ptr[:],
                                            obf[:, 128 * j:128 * (j + 1)],
                                            ident[:])
                        nc.vector.tensor_copy(
                            outT_sb[:, TB * j + 128 * m:
                                    TB * j + 128 * m + 128], ptr[:])
                    if post is not None and m == 1:
                        post()
                return f

            for m in range(2):
                closures.append(mk_mm(m, 0))
                closures.append(mk_mm(m, 1))
            for m in range(2):
                closures.append(mk_ln(m))
            for m in range(2):
                closures.append(mk_tr(m))
            return closures

        def q2_closures(pool, wq2_sb, bq2_sb, outT_sb, qt2_box):
            closures = []

            def mk(j):
                def f():
                    if "t" not in qt2_box:
                        qt2_box["t"] = pool.tile([128, KD * TB], BF16,
                                                 tag="qt2", name="qt2")
                    qt2 = qt2_box["t"]
                    pq = ps_aux.tile([128, TB], F32, tag="psaux", name="pq")
                    for k in range(KD):
                        nc.tensor.matmul(
                            pq[:],
                            lhsT=wq2_sb[:, 1024 * j + 128 * k:
                                        1024 * j + 128 * k + 128],
                            rhs=outT_sb[:, TB * k:TB * (k + 1)],
                            start=(k == 0), stop=(k == KD - 1))
                    nc.vector.tensor_scalar_add(
                        qt2[:, TB * j:TB * (j + 1)], pq[:],
                        bq2_sb[:, j:j + 1])
                return f

            for j in range(KD):
                closures.append(mk(j))
            return closures

        def ffn_w1_closures(pool, w1s_pool, b1f_sb, outT_sb, hT_box, tag):
            closures = []

            def mk(fc):
                def f():
                    if "t" not in hT_box:
                        hT_box["t"] = pool.tile([128, FC * TB], BF16,
                                                tag=tag, name=tag)
                    hT = hT_box["t"]
                    w1t = w1s_pool.tile([128, KD * 128], BF16, tag="w1s")
                    nc.sync.dma_start(out=w1t[:],
                                      in_=w1_d[:, 1024 * fc:1024 * (fc + 1)])
                    ph = ps_aux.tile([128, TB], F32, tag="psaux", name="ph")
                    for k in range(KD):
                        nc.tensor.matmul(
                            ph[:],
                            lhsT=w1t[:, 128 * k:128 * (k + 1)],
                            rhs=outT_sb[:, TB * k:TB * (k + 1)],
                            start=(k == 0), stop=(k == KD - 1))
                    nc.vector.tensor_scalar(hT[:, TB * fc:TB * (fc + 1)],
                                            ph[:], b1f_sb[:, fc:fc + 1], 0.0,
                                            op0=ADD, op1=MAX)
                return f

            for fc in range(FC):
                closures.append(mk(fc))
            return closures

        out1v = out1.rearrange("p (m e) -> p m e", m=4)

        # =====================================================================
        # p3k: cross-attn K/V/Q tensors that survive into attn2 phases
        with tc.tile_pool(name="p3k", bufs=1) as p3k:
            KT2 = [p3k.tile([128, SB], BF16, tag=f"KT2{b}", name=f"KT2{b}")
                   for b in range(B)]
            vaug2 = [p3k.tile([128, HPC * NBC * 65], BF16, tag=f"vaug2{b}",
                              name=f"vaug2{b}") for b in range(B)]
            QT2 = [p3k.tile([128, SB], BF16, tag=f"QT2{b}", name=f"QT2{b}")
                   for b in range(B)]

            # ============ phases 1-2: QKV1 + self attention ==================
            with tc.tile_pool(name="pA", bufs=1) as pA, \
                 tc.tile_pool(name="p12s", bufs=2) as p12s:
                wq1 = pA.tile([128, KD * 128], BF16, tag="wq1")
                wk1 = pA.tile([128, KD * 128], BF16, tag="wk1")
                wv1 = pA.tile([128, KD * 128], BF16, tag="wv1")
                nc.sync.dma_start(out=wq1[:], in_=wq1_d[:])
                nc.sync.dma_start(out=wk1[:], in_=wk1_d[:])
                nc.sync.dma_start(out=wv1[:], in_=wv1_d[:])
                bq1 = pA.tile([128, 1], F32, tag="bq1")
                bk1 = pA.tile([128, 1], F32, tag="bk1")
                bv1 = pA.tile([128, 1], F32, tag="bv1")
                nc.sync.dma_start(out=bq1[:], in_=bq1_d[:])
                nc.sync.dma_start(out=bk1[:], in_=bk1_d[:])
                nc.sync.dma_start(out=bv1[:], in_=bv1_d[:])
                wk2 = pA.tile([128, KD * 128], BF16, tag="wk2")
                wv2 = pA.tile([128, KD * 128], BF16, tag="wv2")
                nc.sync.dma_start(out=wk2[:], in_=wk2_d[:])
                nc.sync.dma_start(out=wv2[:], in_=wv2_d[:])
                bk2 = pA.tile([128, 1], F32, tag="bk2")
                bv2 = pA.tile([128, 1], F32, tag="bv2")
                nc.sync.dma_start(out=bk2[:], in_=bk2_d[:])
                nc.sync.dma_start(out=bv2[:], in_=bv2_d[:])

                xTd = xT_d.rearrange("(k p) t -> p k t", p=128)
                encTd = encT_d.rearrange("(k p) t -> p k t", p=128)

                def load_half(tl, src_view, b):
                    # chunked per-512-token-tile DMA of one batch half
                    v = tl.rearrange("p (k t) -> p k t", k=KD)
                    for jj in range(4):
                        nc.sync.dma_start(
                            out=v[:, :, 512 * jj:512 * (jj + 1)],
                            in_=src_view[:, :, SB * b + 512 * jj:
                                         SB * b + 512 * jj + 512])

                # b0-only self-attn tensors live in pXT, which closes right
                # after attn1(b0) so its space can be reused by pW1a (the
                # wo1/q2 tensors needed by the W1(b0) fillers of attn1(b1))
                from contextlib import ExitStack
                pXT_es = ExitStack()
                pW1a_es = ExitStack()
                pXT = pXT_es.enter_context(tc.tile_pool(name="pXT", bufs=1))

                # xT halves share one 32KB slot; the b1 half's DMA waits
                # for the b0 projections to finish reading the slot.
                xTh0 = pXT.tile([128, KD * SB], BF16, tag="xTh", name="xTh0")
                load_half(xTh0, xTd, 0)
                encTh0 = pA.tile([128, KD * SB], BF16, tag="encTh",
                                 name="encTh0")
                load_half(encTh0, encTd, 0)

                QT = [(pXT if b == 0 else pA).tile(
                          [128, SB], BF16, tag=f"QT{b}", name=f"QT{b}")
                      for b in range(B)]
                KT = [(pXT if b == 0 else pA).tile(
                          [128, SB], BF16, tag=f"KT{b}", name=f"KT{b}")
                      for b in range(B)]
                vT1 = pA.tile([128, SB], BF16, tag="vTs", name="vT1")
                vaug1 = [(pXT if b == 0 else pA).tile(
                             [128, HPC * NBC * 65], BF16,
                             tag=f"vaug1{b}", name=f"vaug1{b}")
                         for b in range(B)]
                stage1 = [(pXT if b == 0 else pA).tile(
                              [64, HPC * SB], BF16, tag=f"stage1{b}",
                              name=f"stage1{b}") for b in range(B)]


                # QKV1(b0) directly
                for jj in range(4):
                    proj_tile(vT1, wv1, bv1, xTh0, jj)
                vaug_ones(vaug1[0])
                for c in range(NBC):
                    vaug_chunk_tr(vT1, vaug1[0], c)
                for jj in range(4):
                    proj_tile(QT[0], wq1, bq1, xTh0, jj)
                for jj in range(4):
                    proj_tile(KT[0], wk1, bk1, xTh0, jj)

                # xT b1 half: slot reuse waits for the QKV1(b0) reads, DMA
                # flies while attn1(b0) computes
                xTh1 = pXT.tile([128, KD * SB], BF16, tag="xTh", name="xTh1")
                load_half(xTh1, xTd, 1)

                # attn1(b0) with QKV1(b1) as fillers
                f_a1b0 = []
                for jj in range(4):
                    f_a1b0.append(lambda jj=jj: proj_tile(vT1, wv1, bv1,
                                                          xTh1, jj))
                f_a1b0.append(lambda: vaug_ones(vaug1[1]))
                for c0 in range(0, NBC, 4):
                    def fv(c0=c0):
                        for c in range(c0, c0 + 4):
                            vaug_chunk_tr(vT1, vaug1[1], c)
                    f_a1b0.append(fv)
                for jj in range(4):
                    f_a1b0.append(lambda jj=jj: proj_tile(QT[1], wq1, bq1,
                                                          xTh1, jj))
                for jj in range(4):
                    f_a1b0.append(lambda jj=jj: proj_tile(KT[1], wk1, bk1,
                                                          xTh1, jj))

                with tc.tile_pool(name="ps_s1", bufs=1, space="PSUM") as ps_s, \
                     tc.tile_pool(name="ps_o1", bufs=1, space="PSUM") as ps_o, \
                     tc.tile_pool(name="pt1", bufs=2) as ptp:
                    attention((ps_s, ps_o, ptp, p12s),
                              QT[0], KT[0], vaug1[0], stage1[0], self_blocks,
                              fillers=f_a1b0, filler_delay=3)
                stage_to_a2a(stage1[0], a11i[0])
                a2a(a11i[0], a11o[0])
                # pXT closed: its 52KB is reused by pW1a below
                pXT_es.close()
                # right-side pool: its lifetime (mid-attn1(b1) era through
                # attn2(b0)) straddles the left-side pool stack boundaries
                pW1a = pW1a_es.enter_context(
                    tc.tile_pool(name="pW1a", bufs=1, side="right"))

                wo1 = pW1a.tile([128, KD * 1024], BF16, tag="wo1")
                wq2 = pW1a.tile([128, KD * KD * 128], BF16, tag="wq2")
                bq2 = pW1a.tile([128, KD], F32, tag="bq2")
                outT1_0 = pW1a.tile([128, KD * TB], BF16, tag="outT1_0")
                at1_0 = pW1a.tile([128, KD * TB], BF16, tag="at1_0")
                xow0 = pW1a.tile([128, 2 * 1024], F32, tag="xow0")
                # at1(b0) load on the gpsimd ring, right behind a2a1(b0)
                nc.gpsimd.dma_start(
                    out=at1_0.rearrange("p (dc s) -> p dc s", dc=KD),
                    in_=a11o[0].rearrange("(dc p) s -> p dc s", p=128))

                def wload():
                    # W-phase weight prefetch; emitted on the sync ring
                    # AFTER the encTh1 chunks so it can't delay them
                    nc.sync.dma_start(out=wo1[:], in_=wo1_d[:])
                    nc.sync.dma_start(out=wq2[:], in_=wq2_d[:])
                    nc.sync.dma_start(out=bq2[:], in_=bq2_d[:])
                    nc.sync.dma_start(
                        out=xow0.rearrange("p (m e) -> p m e", m=2),
                        in_=xown_d[0:TB].rearrange("(m p) e -> p m e",
                                                   p=128))

                qt2_box = [{}, {}]

                def finish_q2(b):
                    qt2 = qt2_box[b]["t"]
                    nc.gpsimd.dma_start(
                        out=a1qi[b].rearrange("(j p) s -> p j s", p=128),
                        in_=qt2.rearrange("p (j s) -> p j s", j=KD))
                    a2a(a1qi[b], a1qo[b])
                    nc.gpsimd.dma_start(
                        out=QT2[b].rearrange("p (i s) -> p i s", i=NCORES),
                        in_=a1qo[b].rearrange("(i p) s -> p i s", p=128))

                # vT2 shares vT1's slot: vT1 dies once vaug1[1] is built
                # (a filler of attn1(b0), emitted above)
                vT2 = pA.tile([128, SB], BF16, tag="vTs", name="vT2")

                # attn1(b1) fillers: cross K/V prep, then W1(b0) + q2(b0).
                # The encT b1 half-load is itself a filler (its slot-reuse
                # waits for all b0 readers, which precede it in the list).
                ench = {0: encTh0}

                def load_ench1():
                    ench[1] = pA.tile([128, KD * SB], BF16, tag="encTh",
                                      name="encTh1")
                    load_half(ench[1], encTd, 1)

                f_a1b1 = []
                for jj in range(4):
                    f_a1b1.append(lambda jj=jj: proj_tile(
                        vT2, wv2, bv2, ench[0], jj))
                for jj in range(4):
                    f_a1b1.append(lambda jj=jj: proj_tile(
                        KT2[0], wk2, bk2, ench[0], jj))
                f_a1b1.append(load_ench1)
                f_a1b1.append(lambda: vaug_ones(vaug2[0], key_mask_col=0))
                for c0 in range(0, NBC, 4):
                    def fv2(c0=c0):
                        for c in range(c0, c0 + 4):
                            vaug_chunk_tr(vT2, vaug2[0], c, key_mask_col=0)
                    f_a1b1.append(fv2)
                for jj in range(4):
                    f_a1b1.append(lambda jj=jj: proj_tile(
                        vT2, wv2, bv2, ench[1], jj))
                f_a1b1.append(lambda: vaug_ones(vaug2[1], key_mask_col=NBC))
                for c0 in range(0, NBC, 4):
                    def fv3(c0=c0):
                        for c in range(c0, c0 + 4):
                            vaug_chunk_tr(vT2, vaug2[1], c, key_mask_col=NBC)
                    f_a1b1.append(fv3)
                for jj in range(4):
                    f_a1b1.append(lambda jj=jj: proj_tile(
                        KT2[1], wk2, bk2, ench[1], jj))
                f_a1b1.append(wload)
                f_a1b1 += wo_ln_closures(
                    p12s, None, at1_0, wo1,
                    resid_of=lambda m: xow0.rearrange(
                        "p (m e) -> p m e", m=2)[:, m],
                    outm_of=lambda m: out1v[:, m],
                    outT_sb=outT1_0, post=None)
                f_a1b1 += q2_closures(pW1a, wq2, bq2, outT1_0, qt2_box[0])

                with tc.tile_pool(name="ps_s2", bufs=1, space="PSUM") as ps_s, \
                     tc.tile_pool(name="ps_o2", bufs=1, space="PSUM") as ps_o, \
                     tc.tile_pool(name="pt2", bufs=2) as ptp:
                    attention((ps_s, ps_o, ptp, p12s),
                              QT[1], KT[1], vaug1[1], stage1[1], self_blocks,
                              fillers=f_a1b1, filler_delay=0)
                stage_to_a2a(stage1[1], a11i[1])
            # pA closed

            # ============ phase 3: a2aq(b0) + attn2(b0) =====================
            finish_q2(0)
            # trigger a2a1(b1) only now: the cc stream is serial, and
            # a2aq(b0) gates attn2(b0) while a2a1(b1) is only needed by
            # the W1(b1) fillers ~16 units into attn2(b0)
            a2a(a11i[1], a11o[1])

            with tc.tile_pool(name="pW1b", bufs=1) as pW1b, \
                 tc.tile_pool(name="pW1s", bufs=2) as pW1s:
                bo2 = pW1b.tile([128, 1024], F32, tag="bo2")
                nc.sync.dma_start(out=bo2[:], in_=bo2_d[:])
                outT1_1 = pW1b.tile([128, KD * TB], BF16, tag="outT1_1")
                at1_1 = pW1b.tile([128, KD * TB], BF16, tag="at1_1")
                xow1 = pW1b.tile([128, 2 * 1024], F32, tag="xow1")
                nc.sync.dma_start(
                    out=xow1.rearrange("p (m e) -> p m e", m=2),
                    in_=xown_d[TB:2 * TB].rearrange("(m p) e -> p m e",
                                                    p=128))
                at1_loaded = [False]

                def load_at1_1():
                    # gpsimd ring: rides right behind a2a1(b1)
                    if not at1_loaded[0]:
                        at1_loaded[0] = True
                        nc.gpsimd.dma_start(
                            out=at1_1.rearrange("p (dc s) -> p dc s", dc=KD),
                            in_=a11o[1].rearrange("(dc p) s -> p dc s",
                                                  p=128))

                def post_bias1(b):
                    # fold bo2 into out1 (after outT1 transpose, before wo2)
                    for m in range(2):
                        mm = 2 * b + m
                        nc.vector.tensor_tensor(
                            out1v[:, mm], out1v[:, mm], bo2[:], op=ADD)

                post_bias1(0)

                # attn2(b0) with W1(b1) + q2(b1) as fillers
                f_a2b0 = wo_ln_closures(
                    pW1s, load_at1_1, at1_1, wo1,
                    resid_of=lambda m: xow1.rearrange(
                        "p (m e) -> p m e", m=2)[:, m],
                    outm_of=lambda m: out1v[:, 2 + m],
                    outT_sb=outT1_1, post=lambda: post_bias1(1))
                f_a2b0 += q2_closures(pW1a, wq2, bq2, outT1_1, qt2_box[1])
                f_a2b0.append(lambda: finish_q2(1))

                with tc.tile_pool(name="pX", bufs=1) as pX, \
                     tc.tile_pool(name="ps_s3", bufs=1, space="PSUM") as ps_s, \
                     tc.tile_pool(name="ps_o3", bufs=1, space="PSUM") as ps_o, \
                     tc.tile_pool(name="pt3", bufs=2) as ptp:
                    stage2_b0 = pX.tile([64, HPC * SB], BF16, tag="stage2")
                    attention((ps_s, ps_o, ptp, pW1s),
                              QT2[0], KT2[0], vaug2[0], stage2_b0,
                              CROSS_BLOCKS, fillers=f_a2b0, filler_delay=16)
                    stage_to_a2a(stage2_b0, a12i[0])
                a2a(a12i[0], a12o[0])
            # pW1b closed
            pW1a_es.close()

            # ============ phase 4: attn2(b1) + wo2/FFN-w1(b0) fillers =======
            with tc.tile_pool(name="p78", bufs=1) as p78, \
                 tc.tile_pool(name="p78s", bufs=2) as p78s, \
                 tc.tile_pool(name="w1str", bufs=3) as w1s_pool, \
                 tc.tile_pool(name="w2str", bufs=3) as w2s_pool:
                out2 = p78.tile([128, 4 * 1024], F32, tag="out2")
                out2v = out2.rearrange("p (m e) -> p m e", m=4)
                wo2 = p78.tile([128, KD * 1024], BF16, tag="wo2")
                nc.sync.dma_start(out=wo2[:], in_=wo2_d[:])
                b1f = p78.tile([128, FC], F32, tag="b1f")
                nc.sync.dma_start(out=b1f[:], in_=b1f_d[:])
                b2 = p78.tile([128, 1024], F32, tag="b2")
                nc.sync.dma_start(out=b2[:], in_=b2_d[:])

                at2 = [p78.tile([128, KD * TB], BF16, tag=f"at2{b}",
                                name=f"at2{b}") for b in range(B)]
                at2_loaded = [False, False]

                def load_at2(b):
                    # gpsimd ring: rides right behind the a2a it waits on
                    if not at2_loaded[b]:
                        at2_loaded[b] = True
                        nc.gpsimd.dma_start(
                            out=at2[b].rearrange("p (dc s) -> p dc s", dc=KD),
                            in_=a12o[b].rearrange("(dc p) s -> p dc s",
                                                  p=128))

                outT2 = [p78.tile([128, KD * TB], BF16, tag=f"outT2{b}",
                                  name=f"outT2{b}") for b in range(B)]
                hT_box = [{}, {}]

                def post_bias2(b):
                    # fold b2 into out2 (after outT2 transpose, before FFN w2)
                    for m in range(2):
                        mm = 2 * b + m
                        nc.vector.tensor_tensor(
                            out2v[:, mm], out2v[:, mm], b2[:], op=ADD)

                f_a2b1 = wo_ln_closures(
                    p78s, lambda: load_at2(0), at2[0], wo2,
                    resid_of=lambda m: out1v[:, m],
                    outm_of=lambda m: out2v[:, m],
                    outT_sb=outT2[0], post=lambda: post_bias2(0))
                f_a2b1 += ffn_w1_closures(p78, w1s_pool, b1f, outT2[0],
                                          hT_box[0], tag="hT")

                with tc.tile_pool(name="pX2", bufs=1) as pX2, \
                     tc.tile_pool(name="ps_s4", bufs=1, space="PSUM") as ps_s, \
                     tc.tile_pool(name="ps_o4", bufs=1, space="PSUM") as ps_o, \
                     tc.tile_pool(name="pt4", bufs=2) as ptp:
                    stage2_b1 = pX2.tile([64, HPC * SB], BF16, tag="stage2b")
                    attention((ps_s, ps_o, ptp, p78s),
                              QT2[1], KT2[1], vaug2[1], stage2_b1,
                              CROSS_BLOCKS, fillers=f_a2b1, filler_delay=8)
                    stage_to_a2a(stage2_b1, a12i[1])
                a2a(a12i[1], a12o[1])

                # ============ phase 5: FFN-w2+LN3(b0); then all of b1 =======
                def ffn_w2_ln3(b):
                    hT = hT_box[b]["t"]
                    with tc.tile_pool(name=f"ps_f{b}", bufs=1,
                                      space="PSUM") as ps_f:
                        py = {(m, eh): ps_f.tile([128, 512], F32,
                                                 tag=f"py{m}{eh}",
                                                 name=f"py{m}{eh}")
                              for m in range(2) for eh in range(2)}
                        for fc in range(FC):
                            w2t = w2s_pool.tile([128, 1024], BF16, tag="w2s")
                            nc.sync.dma_start(
                                out=w2t[:],
                                in_=w2_d[:, 1024 * fc:1024 * (fc + 1)])
                            for m in range(2):
                                for eh in range(2):
                                    nc.tensor.matmul(
                                        py[(m, eh)][:],
                                        lhsT=hT[:, TB * fc + 128 * m:
                                                TB * fc + 128 * m + 128],
                                        rhs=w2t[:, 512 * eh:512 * (eh + 1)],
                                        start=(fc == 0), stop=(fc == FC - 1))
                        pres = {}
                        for m in range(2):
                            mm = 2 * b + m
                            pre = p78s.tile([128, 1024], F32, tag="pref",
                                            name=f"pref{m}")
                            for eh in range(2):
                                nc.vector.tensor_tensor(
                                    pre[:, 512 * eh:512 * (eh + 1)],
                                    py[(m, eh)][:],
                                    out2v[:, mm, 512 * eh:512 * (eh + 1)],
                                    op=ADD)
                            pres[m] = pre
                        for m in range(2):
                            outf = p78s.tile([128, 1024], F32, tag="outf",
                                             name=f"outf{m}")
                            ln_inplace(pres[m], outf)
                            nc.sync.dma_start(
                                out=out_d[256 * b + 128 * m:
                                          256 * b + 128 * m + 128, :],
                                in_=outf[:])

                ffn_w2_ln3(0)

                # b1 drain: wo2(b1) + LN2(b1) + FFN(b1)
                w2cl = wo_ln_closures(
                    p78s, lambda: load_at2(1), at2[1], wo2,
                    resid_of=lambda m: out1v[:, 2 + m],
                    outm_of=lambda m: out2v[:, 2 + m],
                    outT_sb=outT2[1], post=lambda: post_bias2(1))
                for f in w2cl:
                    f()
                for f in ffn_w1_closures(p78, w1s_pool, b1f, outT2[1],
                                         hT_box[1], tag="hT"):
                    f()
                ffn_w2_ln3(1)

    nc.compile()
    return nc


def _to_bf(a):
    return np.ascontiguousarray(np.asarray(a, np.float32).astype(BF))


def _rechunk_k(w):
    """[K*128, M] -> [128, K*M] with col k*M + m = w[k*128 + p, m]."""
    K = w.shape[0] // 128
    M = w.shape[1]
    return np.ascontiguousarray(
        w.reshape(K, 128, M).transpose(1, 0, 2).reshape(128, K * M))


def _analyze_self_mask(mask):
    """mask [S, S] (1 = disallowed), orientation [q, k].

    Returns blocks dict (t, c) -> 'full' | ('tile', idx), list of unique
    multiplicative tiles [128, 512] (bf16), for a block grid over one batch.
    Blocks where everything is disallowed are omitted.
    """
    add = np.float32(-1e9) * np.asarray(mask, np.float32)
    mult = np.exp(add.T)  # [k, q] multiplicative
    blocks = {}
    tiles = []
    tile_ids = {}
    for t in range(NBT):
        for c in range(NBC):
            sub = mult[128 * c:128 * (c + 1), 512 * t:512 * (t + 1)]
            if not sub.any():
                continue
            if (sub == 1.0).all():
                blocks[(t, c)] = 'full'
                continue
            key = sub.tobytes()
            if key not in tile_ids:
                tile_ids[key] = len(tiles)
                tiles.append(sub.astype(BF))
            blocks[(t, c)] = ('tile', tile_ids[key])
    return blocks, tiles


def kernel(**inputs):
    from concourse.bass_utils import run_bass_kernel_spmd

    x = np.asarray(inputs["x"], np.float32)
    enc = np.asarray(inputs["enc_output"], np.float32)
    lam = np.asarray(inputs["look_ahead_mask"], np.float32)[0, 0]
    pad = np.asarray(inputs["padding_mask"], np.float32)  # [B,1,1,S]

    self_blocks, ctiles = _analyze_self_mask(lam)
    n_ctiles = len(ctiles)
    key = (tuple(sorted(self_blocks.items())), n_ctiles)
    if key not in _PROG_CACHE:
        _PROG_CACHE[key] = _build_program(self_blocks, n_ctiles)
    nc = _PROG_CACHE[key]

    # ---- shared (core-independent) host prep ----
    xf = x.reshape(TOK, D_MODEL)             # flattened batch-major tokens
    encf = enc.reshape(TOK, D_MODEL)
    xT = _to_bf(xf.T)                        # [1024, 4096]
    encT = _to_bf(encf.T)
    if n_ctiles:
        cmask = np.concatenate(ctiles, axis=1)
    else:
        cmask = np.zeros((128, 512), BF)
    cmask = np.ascontiguousarray(cmask)
    # cross-attn key-keep mask per enc token: [128, B*16], col b*16+c
    mb = np.exp(np.float32(-1e9) * pad[:, 0, 0, :]).reshape(B, NBC, 128)
    mb = np.ascontiguousarray(mb.transpose(2, 0, 1).reshape(128, B * NBC)
                              ).astype(np.float32)

    w1f = np.asarray(inputs["ffn_w1"], np.float32)
    # w1 stationary layout: [128, fc*1024 + k*128 + m] = w1[k*128+p, fc*128+m]
    w1r = w1f.reshape(KD, 128, FC, 128).transpose(1, 2, 0, 3)
    w1r = _to_bf(w1r.reshape(128, FC * KD * 128))
    w2r = _to_bf(_rechunk_k(np.asarray(inputs["ffn_w2"], np.float32)))
    # b1 per-partition per-chunk [128, FC]; b2/bo2 pre-broadcast [128, 1024]
    b1 = np.ascontiguousarray(
        np.asarray(inputs["ffn_b1"], np.float32).reshape(FC, 128).T)
    b2 = np.ascontiguousarray(np.broadcast_to(
        np.asarray(inputs["ffn_b2"], np.float32)[None, :], (128, 1024)))

    wo1r = _to_bf(_rechunk_k(np.asarray(inputs["mha1_wo"], np.float32)))
    wo2r = _to_bf(_rechunk_k(np.asarray(inputs["mha2_wo"], np.float32)))
    bo1 = np.asarray(inputs["mha1_bo"], np.float32)
    bo2 = np.ascontiguousarray(np.broadcast_to(
        np.asarray(inputs["mha2_bo"], np.float32)[None, :], (128, 1024)))
    ident = np.eye(128, dtype=np.float32).astype(BF)

    wq2_full = np.asarray(inputs["mha2_wq"], np.float32)
    # wq2 stationary layout: [128, j*1024 + k*128 + m] = wq2[k*128+p, j*128+m]
    wq2r = wq2_full.reshape(KD, 128, KD, 128).transpose(1, 2, 0, 3)
    wq2r = _to_bf(wq2r.reshape(128, KD * KD * 128))
    bq2 = np.asarray(inputs["mha2_bq"], np.float32).reshape(KD, 128)
    bq2 = np.ascontiguousarray(bq2.T).astype(np.float32)  # [128, KD]

    in_maps = []
    for j in range(NCORES):
        hs = slice(128 * j, 128 * (j + 1))       # this core's 2 heads' cols
        xo = np.concatenate([xf[TB * j:TB * (j + 1)],
                             xf[S + TB * j:S + TB * (j + 1)]], axis=0)
        xo = np.ascontiguousarray(xo + bo1[None, :])
        m = {
            "xT": xT, "encT": encT, "x_own": xo,
            "cmask": cmask, "mbias": mb,
            "w1": w1r, "b1": b1, "w2": w2r, "b2": b2,
            "wo1": wo1r, "wo2": wo2r, "bo2": bo2,
            "wq2": wq2r, "bq2": bq2, "ident": ident,
        }
        for pre, name in (("wq1", "mha1_wq"), ("wk1", "mha1_wk"),
                          ("wv1", "mha1_wv"), ("wk2", "mha2_wk"),
                          ("wv2", "mha2_wv")):
            w = np.asarray(inputs[name], np.float32)[:, hs]
            m[pre] = _to_bf(_rechunk_k(w))
        for pre, name in (("bq1", "mha1_bq"), ("bk1", "mha1_bk"),
                          ("bv1", "mha1_bv"), ("bk2", "mha2_bk"),
                          ("bv2", "mha2_bv")):
            bvec = np.asarray(inputs[name], np.float32)[hs]
            m[pre] = np.ascontiguousarray(bvec[:, None])
        in_maps.append(m)

    res = run_bass_kernel_spmd(nc, in_maps, list(range(NCORES)))
    out = np.empty((TOK, D_MODEL), np.float32)
    for j in range(NCORES):
        r = res.results[j]["out"]
        out[TB * j:TB * (j + 1)] = r[0:TB]
        out[S + TB * j:S + TB * (j + 1)] = r[TB:2 * TB]
    return out.reshape(B, S, D_MODEL)


# revision 44
# speedup vs baseline: 1.1626x; 1.0729x over previous
"""Trainium2 Bass kernel for a transformer decoder layer (self-attn + cross-attn + FFN).

Sharding: 8-way tensor parallel over heads for both attentions (2 heads/core);
token-sharded for wo projections, layernorms and FFN with each core owning 256
tokens of EACH batch. Head<->token redistribution uses six half-size (per-batch)
AllToAll collectives. The two batches are independent through the whole layer,
so the schedule is batch-pipelined: every collective flies while the other
batch computes (attn1(b1) covers a2a1(b0); wo1/q2(b1) and wo2+FFN-w1(b0) run as
PE filler work inside the ACT-bound cross-attention phases; etc.).

All matmuls run in bf16 with fp32 PSUM accumulation. Attention keeps the
[feature, token] (transposed) layout throughout: scoresT uses kT-chunk
stationary x qT moving, probs come out as PT[ki, qi] which feeds attnV
directly with V-natural (+ones column) stationary, producing attn^T and the
softmax denominator in one accumulation group. Normalization happens via a
reciprocal row broadcast with a rank-1 fp32r matmul, off the critical path.
The cross-attention padding mask is folded into V by zeroing masked key rows
(incl. the ones column), which removes them from output and denominator.

LayerNorm rstd is computed as exp(scale*bits(var+eps)+bias) (the exponent-bits
log approximation folded into ACT's free affine) polished by two Newton
iterations on DVE — Exp is the only ACT table function in the whole kernel,
avoiding the ~1.3us-per-switch activation-table ping-pong between Exp and Ln.
"""

import sys

TRN_REPO = "/opt/trn_rl_repo"
if TRN_REPO not in sys.path:
    sys.path.insert(0, TRN_REPO)

import numpy as np
import ml_dtypes

D_MODEL = 1024
N_HEADS = 16
DFF = 4096
B, S = 2, 2048
EPS = 1e-6
DEPTH = D_MODEL // N_HEADS  # 64

NCORES = 8
HPC = N_HEADS // NCORES     # heads per core = 2
TOK = B * S                 # 4096 flattened tokens
SB = S                      # tokens per batch = 2048
TB = SB // NCORES           # tokens per core per batch = 256
KD = D_MODEL // 128         # 8 contraction chunks over d_model
FC = DFF // 128             # 32 chunks over dff
NBT = SB // 512             # 4 q-tiles per batch
NBC = SB // 128             # 16 ki-chunks per batch

BF = ml_dtypes.bfloat16

# rsqrt-via-exp-bits constants: rsqrt(v) ~= exp(RS_SCALE*float(bits(v)) + RS_BIAS)
_LN2 = float(np.log(2.0))
RS_SCALE = -0.5 * _LN2 / (1 << 23)
RS_BIAS = 0.5 * _LN2 * (127 + 0.0430357)

_PROG_CACHE = {}


def _build_program(self_blocks, n_ctiles):
    """Emit the SPMD Bass program (same program on all 8 cores)."""
    import concourse.bacc as bacc
    import concourse.mybir as mybir
    from concourse import tile

    F32 = mybir.dt.float32
    F32R = mybir.dt.float32r
    I32 = mybir.dt.int32
    BF16 = mybir.dt.bfloat16
    EXP = mybir.ActivationFunctionType.Exp
    ADD = mybir.AluOpType.add
    MULT = mybir.AluOpType.mult
    SUB = mybir.AluOpType.subtract
    MAX = mybir.AluOpType.max

    nc = bacc.Bacc("TRN2", target_bir_lowering=False, debug=False,
                   num_devices=NCORES)

    def din(name, shape, dt=BF16):
        return nc.dram_tensor(name, shape, dt, kind="ExternalInput")

    xT_d = din("xT", [D_MODEL, TOK])
    encT_d = din("encT", [D_MODEL, TOK])
    xown_d = din("x_own", [2 * TB, D_MODEL], F32)   # bo1 pre-folded on host
    wq1_d = din("wq1", [128, KD * 128])
    wk1_d = din("wk1", [128, KD * 128])
    wv1_d = din("wv1", [128, KD * 128])
    bq1_d = din("bq1", [128, 1], F32)
    bk1_d = din("bk1", [128, 1], F32)
    bv1_d = din("bv1", [128, 1], F32)
    wo1_d = din("wo1", [128, KD * 1024])
    wq2_d = din("wq2", [128, KD * KD * 128])
    bq2_d = din("bq2", [128, KD], F32)
    wk2_d = din("wk2", [128, KD * 128])
    wv2_d = din("wv2", [128, KD * 128])
    bk2_d = din("bk2", [128, 1], F32)
    bv2_d = din("bv2", [128, 1], F32)
    wo2_d = din("wo2", [128, KD * 1024])
    bo2_d = din("bo2", [128, 1024], F32)   # pre-broadcast
    w1_d = din("w1", [128, FC * KD * 128])
    b1f_d = din("b1", [128, FC], F32)      # per-partition per-chunk
    w2_d = din("w2", [128, FC * 1024])
    b2_d = din("b2", [128, 1024], F32)     # pre-broadcast
    ident_d = din("ident", [128, 128])
    cm_d = din("cmask", [128, max(n_ctiles, 1) * 512])
    mb_d = din("mbias", [128, B * NBC], F32)
    out_d = nc.dram_tensor("out", [2 * TB, D_MODEL], F32, kind="ExternalOutput")

    CROSS_BLOCKS = {(t, c): 'full' for t in range(NBT) for c in range(NBC)}
    GROUPS = [list(range(NCORES))]
    GW = 2  # ki-chunks per merged exp group

    with tile.TileContext(nc) as tc:
      with tc.tile_pool(name="const", bufs=1) as constp, \
           tc.tile_pool(name="fbuf", bufs=1) as fbuf, \
           tc.tile_pool(name="lns", bufs=2) as lns, \
           tc.tile_pool(name="dram", bufs=1, space="DRAM") as dram, \
           tc.tile_pool(name="ps_aux", bufs=2, space="PSUM") as ps_aux:

        # ---- constants ----
        ones65 = constp.tile([1, 65], F32)
        nc.vector.memset(ones65[:], 1.0)
        rsb = constp.tile([128, 1], F32)
        nc.vector.memset(rsb[:], RS_BIAS)
        ident = constp.tile([128, 128], BF16)
        nc.sync.dma_start(out=ident[:], in_=ident_d[:])
        cm = constp.tile([128, max(n_ctiles, 1) * 512], BF16)
        nc.sync.dma_start(out=cm[:], in_=cm_d[:])
        mb = constp.tile([128, B * NBC], F32)
        nc.sync.dma_start(out=mb[:], in_=mb_d[:])

        # ---- persistent activations ----
        out1 = fbuf.tile([128, 4 * 1024], F32, tag="out1")

        # ---- a2a dram buffers (per batch) ----
        bar_in = dram.tile([NCORES, 16], BF16)
        bar_out = dram.tile([NCORES, 16], BF16)
        a11i = [dram.tile([NCORES * 128, TB], BF16, name=f"a11i{b}")
                for b in range(B)]
        a11o = [dram.tile([NCORES * 128, TB], BF16, name=f"a11o{b}")
                for b in range(B)]
        a1qi = [dram.tile([NCORES * 128, TB], BF16, name=f"a1qi{b}")
                for b in range(B)]
        a1qo = [dram.tile([NCORES * 128, TB], BF16, name=f"a1qo{b}")
                for b in range(B)]
        a12i = [dram.tile([NCORES * 128, TB], BF16, name=f"a12i{b}")
                for b in range(B)]
        a12o = [dram.tile([NCORES * 128, TB], BF16, name=f"a12o{b}")
                for b in range(B)]

        # startup barrier: absorb cross-core launch skew here (overlapped
        # with the initial input DMAs) instead of inside the first real a2a
        nc.sync.dma_start(out=bar_in[:], in_=ident[0:NCORES, 0:16])
        nc.gpsimd.collective_compute(
            "AllToAll", mybir.AluOpType.bypass, replica_groups=GROUPS,
            ins=[bar_in.opt()], outs=[bar_out.opt()])

        def a2a(in_t, out_t):
            nc.gpsimd.collective_compute(
                "AllToAll", mybir.AluOpType.bypass, replica_groups=GROUPS,
                ins=[in_t.opt()], outs=[out_t.opt()])

        # ---------------- shared helpers ----------------
        def proj_tile(dst, w_sb, bias, src_sb, jj):
            # dst[:, 512*jj:...] = (w_chunk^T @ srcT)[dcol, tok] + bias.
            # src_sb is one batch half [128, KD*SB]; jj in 0..3.
            ps = ps_aux.tile([128, 512], F32, tag="psaux", name="psp")
            for k in range(KD):
                nc.tensor.matmul(
                    ps[:],
                    lhsT=w_sb[:, 128 * k:128 * (k + 1)],
                    rhs=src_sb[:, k * SB + 512 * jj:k * SB + 512 * jj + 512],
                    start=(k == 0), stop=(k == KD - 1))
            nc.vector.tensor_scalar_add(dst[:, 512 * jj:512 * (jj + 1)],
                                        ps[:], bias[:])

        def vaug_ones(vaug_sb, key_mask_col=None):
            # write only the 65th (ones/denominator) column of each group:
            # the 0:64 columns are fully overwritten by the chunk transposes
            v = vaug_sb.rearrange("p (h c d) -> p h c d", h=HPC, c=NBC)
            if key_mask_col is None:
                nc.vector.memset(v[:, :, :, 64:65], 1.0)
            else:
                for h in range(HPC):
                    nc.vector.tensor_copy(
                        v[:, h, :, 64],
                        mb[:, key_mask_col:key_mask_col + NBC])

        def vaug_chunk_tr(vT_sb, vaug_sb, c, key_mask_col=None):
            # PE-transpose V chunk c ([128 (h,d), 128 tok] -> [128 tok,
            # (h,d)]) and scatter into vaug's per-head 65-column groups;
            # key_mask zeroes dropped keys (per-partition scalar, fused
            # into the scatter copy).
            ptr = ps_aux.tile([128, 128], BF16, tag="psaux", name="ptr")
            nc.tensor.transpose(ptr[:], vT_sb[:, 128 * c:128 * (c + 1)],
                                ident[:])
            dst = vaug_sb.rearrange("p (h c d) -> p h c d", h=HPC, c=NBC)
            src = ptr.rearrange("p (h d) -> p h d", h=HPC)
            if key_mask_col is None:
                nc.vector.tensor_copy(dst[:, :, c, 0:64], src)
            else:
                nc.vector.tensor_scalar_mul(
                    dst[:, :, c, 0:64], src,
                    mb[:, key_mask_col + c:key_mask_col + c + 1])

        def vaug_slice(vaug_sb, h, c):
            base = 65 * (NBC * h + c)
            return vaug_sb[:, base:base + 65]

        def rsqrt_of(veps):
            # [128, 1] f32 -> rstd = (veps)^-0.5 via exp-bits seed (~1.5%
            # err) + one fused Newton step (3 DVE ops, ~3e-3 max err)
            bfi = lns.tile([128, 1], F32, tag="bfi")
            nc.vector.tensor_copy(bfi[:], veps.bitcast(I32))
            y = lns.tile([128, 1], F32, tag="rsy")
            nc.scalar.activation(y[:], bfi[:], EXP, scale=RS_SCALE,
                                 bias=rsb[:])
            t2 = lns.tile([128, 1], F32, tag="rst")
            nc.vector.scalar_tensor_tensor(t2[:], veps, y[:], y[:],
                                           op0=MULT, op1=MULT)
            c2 = lns.tile([128, 1], F32, tag="rsc")
            nc.vector.tensor_scalar(c2[:], t2[:], -0.5, 1.5,
                                    op0=MULT, op1=ADD)
            yn = lns.tile([128, 1], F32, tag="rsn")
            nc.vector.tensor_scalar(yn[:], c2[:], y[:], 0.0,
                                    op0=MULT, op1=ADD)
            return yn

        def ln_inplace(pre, dst):
            bnst = lns.tile([128, 12], F32, tag="bnst")
            nc.vector.bn_stats(bnst[:, 0:6], pre[:, 0:512])
            nc.vector.bn_stats(bnst[:, 6:12], pre[:, 512:1024])
            stats = lns.tile([128, 2], F32, tag="stats")
            nc.vector.bn_aggr(stats[:], bnst[:])
            veps = lns.tile([128, 1], F32, tag="veps")
            nc.vector.tensor_scalar_add(veps[:], stats[:, 1:2], EPS)
            rstd = rsqrt_of(veps[:])
            nc.vector.tensor_scalar(dst[:], pre[:], stats[:, 0:1], rstd[:],
                                    op0=SUB, op1=MULT)

        def attention(pools, QT_sb, KT_sb, vaug_sb, stage_sb, blocks,
                      fillers=None, filler_delay=0):
            # Software-pipelined attention over ONE batch. Per work unit
            # (t, chunk-group): scores for GW ki-chunks of both heads land
            # in per-head psums (h0 on PE rows 0-63, h1 on rows 64-127 —
            # row tiling), one Exp per head covers the group. The attnV
            # matmuls of the PREVIOUS unit are emitted after this unit's
            # scores so the PE queue never stalls on the exp; softmax
            # divisions are delayed one more unit.
            ps_s, ps_o, ptp, smalls = pools
            units = []
            for t in range(NBT):
                clist = [c for c in range(NBC) if (t, c) in blocks]
                groups = [clist[i:i + GW] for i in range(0, len(clist), GW)]
                for gi, grp in enumerate(groups):
                    units.append((t, grp, gi == 0, gi == len(groups) - 1))

            po = {}          # live accumulation psums, per head
            pending = None   # (unit, pt4 dict)
            div_q = []       # (t, po) awaiting division emission

            def emit_attnv(unit, pt4):
                t, grp, isfirst, islast = unit
                if isfirst:
                    for h in range(HPC):
                        po[h] = ps_o.tile([65, 512], F32, tag=f"po{h}",
                                          name=f"po{h}")
                for ci, c in enumerate(grp):
                    kind = blocks[(t, c)]
                    for h in range(HPC):
                        rhs = pt4[h][:, 512 * ci:512 * (ci + 1)]
                        if kind != 'full':
                            idx = kind[1]
                            nc.vector.tensor_tensor(
                                rhs, rhs, cm[:, 512 * idx:512 * (idx + 1)],
                                op=MULT)
                        nc.tensor.matmul(
                            po[h][:], lhsT=vaug_slice(vaug_sb, h, c),
                            rhs=rhs, start=(isfirst and ci == 0),
                            stop=(islast and ci == len(grp) - 1))
                if islast:
                    div_q.append((t, dict(po)))

            def emit_division():
                t, po_t = div_q.pop(0)
                for h in range(HPC):
                    osb = smalls.tile([65, 512], F32, tag="osb")
                    nc.vector.tensor_copy(osb[:], po_t[h][:])
                    recip = smalls.tile([1, 512], F32R, tag="recip")
                    with nc.allow_low_precision(reason="softmax recip row"):
                        nc.vector.reciprocal(recip[:], osb[64:65, :])
                    pb = ps_aux.tile([65, 512], F32, tag="psaux", name="pb")
                    nc.tensor.matmul(pb[:], lhsT=ones65[:].bitcast(F32R),
                                     rhs=recip[:], start=True, stop=True)
                    pbsb = smalls.tile([64, 512], F32, tag="pbsb")
                    nc.vector.tensor_copy(pbsb[:], pb[0:64, :])
                    nc.vector.tensor_tensor(
                        stage_sb[0:64,
                                 h * SB + 512 * t:h * SB + 512 * t + 512],
                        osb[0:64, :], pbsb[:], op=MULT)

            fillers = list(fillers) if fillers else []
            unit_no = [0]

            def emit_warmer():
                # Keep PE array activity high: attention's half-array matmuls
                # don't generate enough PE activity for the HAM to ever
                # unthrottle the clock. Emit either a unit of real full-array
                # work (pipelined filler) or a dummy matmul.
                if fillers and unit_no[0] >= filler_delay:
                    fillers.pop(0)()
                    return
                pd = ps_aux.tile([128, 512], F32, tag="psaux", name="pd")
                nc.tensor.matmul(pd[:], lhsT=KT_sb[:, 0:128],
                                 rhs=KT_sb[:, 1024:1536],
                                 start=True, stop=True)

            for unit in units:
                t, grp, isfirst, islast = unit
                ps4 = {h: ps_s.tile([128, GW * 512], F32,
                                    tag=f"ps4h{h}", name=f"ps4h{h}")
                       for h in range(HPC)}
                for ci, c in enumerate(grp):
                    for h in range(HPC):
                        nc.tensor.matmul(
                            ps4[h][:, 512 * ci:512 * (ci + 1)],
                            lhsT=KT_sb[64 * h:64 * (h + 1),
                                       128 * c:128 * c + 128],
                            rhs=QT_sb[64 * h:64 * (h + 1),
                                      512 * t:512 * t + 512],
                            start=True, stop=True)
                pt4 = {}
                for h in range(HPC):
                    pt4[h] = ptp.tile([128, GW * 512], BF16,
                                      tag=f"pt4h{h}", name=f"pt4h{h}")
                    nc.scalar.activation(pt4[h][:], ps4[h][:], EXP,
                                         scale=0.125)
                emit_warmer()
                if pending is not None:
                    emit_attnv(*pending)
                    if div_q and len(div_q) > 1:
                        emit_division()
                emit_warmer()
                pending = (unit, pt4)
                unit_no[0] += 1
            emit_attnv(*pending)
            # interleave leftover fillers with the division tail so the PE
            # has work while the serial DVE division chains drain
            while div_q:
                emit_division()
                if fillers:
                    fillers.pop(0)()
            for f in fillers:
                f()

        def stage_to_a2a(stage_sb, a2a_in_t):
            # on the gpsimd (SWDGE) ring: this DMA waits on the attention
            # divisions, and the next gpsimd op is the a2a trigger that
            # needs it anyway — keeps the sync ring free for prefetches
            for h in range(HPC):
                o = a2a_in_t.rearrange("(j r) s -> r j s", r=128)
                nc.gpsimd.dma_start(
                    out=o[64 * h:64 * (h + 1)],
                    in_=stage_sb.rearrange("r (h j s) -> r h j s",
                                           h=HPC, j=NCORES)[:, h])

        def wo_ln_closures(pool, pget, at_sb, wo_sb, resid_of, outm_of,
                           outT_sb, post=None):
            # Returns filler closures computing, for m in {0,1}:
            # outm_of(m) = LN(resid_of(m) + at^T @ wo), plus the bf16
            # PE-transpose of the LN output into outT_sb, then post().
            # Order: all 4 matmul closures first, then the two LN chains
            # (so their serial DVE/ACT latency overlaps PE work), then the
            # PE transposes.
            closures = []
            pre_box = {}
            obf_box = {}

            def mk_mm(m, eh):
                def f():
                    if pget is not None:
                        pget()
                    if m not in pre_box:
                        pre_box[m] = pool.tile([128, 1024], F32, tag="pre",
                                               name=f"pre{m}")
                    pre = pre_box[m]
                    pw = ps_aux.tile([128, 512], F32, tag="psaux", name="pw")
                    for dc in range(KD):
                        nc.tensor.matmul(
                            pw[:],
                            lhsT=at_sb[:, TB * dc + 128 * m:
                                       TB * dc + 128 * m + 128],
                            rhs=wo_sb[:, 1024 * dc + 512 * eh:
                                      1024 * dc + 512 * eh + 512],
                            start=(dc == 0), stop=(dc == KD - 1))
                    nc.vector.tensor_tensor(
                        pre[:, 512 * eh:512 * (eh + 1)], pw[:],
                        resid_of(m)[:, 512 * eh:512 * (eh + 1)], op=ADD)
                return f

            def mk_ln(m):
                def f():
                    pre = pre_box.pop(m)
                    outm = outm_of(m)
                    ln_inplace(pre, outm)
                    obf = pool.tile([128, 1024], BF16, tag="obf",
                                    name=f"obf{m}")
                    nc.vector.tensor_copy(obf[:], outm)
                    obf_box[m] = obf
                return f

            def mk_tr(m):
                def f():
                    obf = obf_box.pop(m)
                    for j in range(KD):
                        ptr = ps_aux.tile([128, 128], BF16, tag="psaux",
                                          name="ptr2")
                        nc.tensor.transpose(ptr[:],
                                            obf[:, 128 * j:128 * (j + 1)],
                                            ident[:])
                        nc.vector.tensor_copy(
                            outT_sb[:, TB * j + 128 * m:
                                    TB * j + 128 * m + 128], ptr[:])
                    if post is not None and m == 1:
                        post()
                return f

            for m in range(2):
                closures.append(mk_mm(m, 0))
                closures.append(mk_mm(m, 1))
            for m in range(2):
                closures.append(mk_ln(m))
            for m in range(2):
                closures.append(mk_tr(m))
            return closures

        def q2_closures(pool, wq2_sb, bq2_sb, outT_sb, qt2_box):
            closures = []

            def mk(j):
                def f():
                    if "t" not in qt2_box:
                        qt2_box["t"] = pool.tile([128, KD * TB], BF16,
                                                 tag="qt2", name="qt2")
                    qt2 = qt2_box["t"]
                    pq = ps_aux.tile([128, TB], F32, tag="psaux", name="pq")
                    for k in range(KD):
                        nc.tensor.matmul(
                            pq[:],
                            lhsT=wq2_sb[:, 1024 * j + 128 * k:
                                        1024 * j + 128 * k + 128],
                            rhs=outT_sb[:, TB * k:TB * (k + 1)],
                            start=(k == 0), stop=(k == KD - 1))
                    nc.vector.tensor_scalar_add(
                        qt2[:, TB * j:TB * (j + 1)], pq[:],
                        bq2_sb[:, j:j + 1])
                return f

            for j in range(KD):
                closures.append(mk(j))
            return closures

        def ffn_w1_closures(pool, w1s_pool, b1f_sb, outT_sb, hT_box, tag):
            closures = []

            def mk(fc):
                def f():
                    if "t" not in hT_box:
                        hT_box["t"] = pool.tile([128, FC * TB], BF16,
                                                tag=tag, name=tag)
                    hT = hT_box["t"]
                    w1t = w1s_pool.tile([128, KD * 128], BF16, tag="w1s")
                    nc.sync.dma_start(out=w1t[:],
                                      in_=w1_d[:, 1024 * fc:1024 * (fc + 1)])
                    ph = ps_aux.tile([128, TB], F32, tag="psaux", name="ph")
                    for k in range(KD):
                        nc.tensor.matmul(
                            ph[:],
                            lhsT=w1t[:, 128 * k:128 * (k + 1)],
                            rhs=outT_sb[:, TB * k:TB * (k + 1)],
                            start=(k == 0), stop=(k == KD - 1))
                    nc.vector.tensor_scalar(hT[:, TB * fc:TB * (fc + 1)],
                                            ph[:], b1f_sb[:, fc:fc + 1], 0.0,
                                            op0=ADD, op1=MAX)
                return f

            for fc in range(FC):
                closures.append(mk(fc))
            return closures

        out1v = out1.rearrange("p (m e) -> p m e", m=4)

        # =====================================================================
        # p3k: cross-attn K/V/Q tensors that survive into attn2 phases
        with tc.tile_pool(name="p3k", bufs=1) as p3k:
            KT2 = [p3k.tile([128, SB], BF16, tag=f"KT2{b}", name=f"KT2{b}")
                   for b in range(B)]
            vaug2 = [p3k.tile([128, HPC * NBC * 65], BF16, tag=f"vaug2{b}",
                              name=f"vaug2{b}") for b in range(B)]
            QT2 = [p3k.tile([128, SB], BF16, tag=f"QT2{b}", name=f"QT2{b}")
                   for b in range(B)]

            # ============ phases 1-2: QKV1 + self attention ==================
            with tc.tile_pool(name="pA", bufs=1) as pA, \
                 tc.tile_pool(name="p12s", bufs=2) as p12s:
                wq1 = pA.tile([128, KD * 128], BF16, tag="wq1")
                wk1 = pA.tile([128, KD * 128], BF16, tag="wk1")
                wv1 = pA.tile([128, KD * 128], BF16, tag="wv1")
                nc.sync.dma_start(out=wq1[:], in_=wq1_d[:])
                nc.sync.dma_start(out=wk1[:], in_=wk1_d[:])
                nc.sync.dma_start(out=wv1[:], in_=wv1_d[:])
                bq1 = pA.tile([128, 1], F32, tag="bq1")
                bk1 = pA.tile([128, 1], F32, tag="bk1")
                bv1 = pA.tile([128, 1], F32, tag="bv1")
                nc.sync.dma_start(out=bq1[:], in_=bq1_d[:])
                nc.sync.dma_start(out=bk1[:], in_=bk1_d[:])
                nc.sync.dma_start(out=bv1[:], in_=bv1_d[:])
                wk2 = pA.tile([128, KD * 128], BF16, tag="wk2")
                wv2 = pA.tile([128, KD * 128], BF16, tag="wv2")
                nc.sync.dma_start(out=wk2[:], in_=wk2_d[:])
                nc.sync.dma_start(out=wv2[:], in_=wv2_d[:])
                bk2 = pA.tile([128, 1], F32, tag="bk2")
                bv2 = pA.tile([128, 1], F32, tag="bv2")
                nc.sync.dma_start(out=bk2[:], in_=bk2_d[:])
                nc.sync.dma_start(out=bv2[:], in_=bv2_d[:])

                xTd = xT_d.rearrange("(k p) t -> p k t", p=128)
                encTd = encT_d.rearrange("(k p) t -> p k t", p=128)

                def load_half(tl, src_view, b):
                    # chunked per-512-token-tile DMA of one batch half
                    v = tl.rearrange("p (k t) -> p k t", k=KD)
                    for jj in range(4):
                        nc.sync.dma_start(
                            out=v[:, :, 512 * jj:512 * (jj + 1)],
                            in_=src_view[:, :, SB * b + 512 * jj:
                                         SB * b + 512 * jj + 512])

                # b0-only self-attn tensors live in pXT, which closes right
                # after attn1(b0) so its space can be reused by pW1a (the
                # wo1/q2 tensors needed by the W1(b0) fillers of attn1(b1))
                from contextlib import ExitStack
                pXT_es = ExitStack()
                pW1a_es = ExitStack()
                pXT = pXT_es.enter_context(tc.tile_pool(name="pXT", bufs=1))

                # xT halves share one 32KB slot; the b1 half's DMA waits
                # for the b0 projections to finish reading the slot.
                xTh0 = pXT.tile([128, KD * SB], BF16, tag="xTh", name="xTh0")
                load_half(xTh0, xTd, 0)
                encTh0 = pA.tile([128, KD * SB], BF16, tag="encTh",
                                 name="encTh0")
                load_half(encTh0, encTd, 0)

                QT = [(pXT if b == 0 else pA).tile(
                          [128, SB], BF16, tag=f"QT{b}", name=f"QT{b}")
                      for b in range(B)]
                KT = [(pXT if b == 0 else pA).tile(
                          [128, SB], BF16, tag=f"KT{b}", name=f"KT{b}")
                      for b in range(B)]
                vT1 = pA.tile([128, SB], BF16, tag="vTs", name="vT1")
                vaug1 = [(pXT if b == 0 else pA).tile(
                             [128, HPC * NBC * 65], BF16,
                             tag=f"vaug1{b}", name=f"vaug1{b}")
                         for b in range(B)]
                stage1 = [(pXT if b == 0 else pA).tile(
                              [64, HPC * SB], BF16, tag=f"stage1{b}",
                              name=f"stage1{b}") for b in range(B)]


                # QKV1(b0) directly
                for jj in range(4):
                    proj_tile(vT1, wv1, bv1, xTh0, jj)
                vaug_ones(vaug1[0])
                for c in range(NBC):
                    vaug_chunk_tr(vT1, vaug1[0], c)
                for jj in range(4):
                    proj_tile(QT[0], wq1, bq1, xTh0, jj)
                for jj in range(4):
                    proj_tile(KT[0], wk1, bk1, xTh0, jj)

                # xT b1 half: slot reuse waits for the QKV1(b0) reads, DMA
                # flies while attn1(b0) computes
                xTh1 = pXT.tile([128, KD * SB], BF16, tag="xTh", name="xTh1")
                load_half(xTh1, xTd, 1)

                # attn1(b0) with QKV1(b1) as fillers
                f_a1b0 = []
                for jj in range(4):
                    f_a1b0.append(lambda jj=jj: proj_tile(vT1, wv1, bv1,
                                                          xTh1, jj))
                f_a1b0.append(lambda: vaug_ones(vaug1[1]))
                for c0 in range(0, NBC, 4):
                    def fv(c0=c0):
                        for c in range(c0, c0 + 4):
                            vaug_chunk_tr(vT1, vaug1[1], c)
                    f_a1b0.append(fv)
                for jj in range(4):
                    f_a1b0.append(lambda jj=jj: proj_tile(QT[1], wq1, bq1,
                                                          xTh1, jj))
                for jj in range(4):
                    f_a1b0.append(lambda jj=jj: proj_tile(KT[1], wk1, bk1,
                                                          xTh1, jj))

                with tc.tile_pool(name="ps_s1", bufs=1, space="PSUM") as ps_s, \
                     tc.tile_pool(name="ps_o1", bufs=1, space="PSUM") as ps_o, \
                     tc.tile_pool(name="pt1", bufs=2) as ptp:
                    attention((ps_s, ps_o, ptp, p12s),
                              QT[0], KT[0], vaug1[0], stage1[0], self_blocks,
                              fillers=f_a1b0, filler_delay=3)
                stage_to_a2a(stage1[0], a11i[0])
                a2a(a11i[0], a11o[0])
                # pXT closed: its 52KB is reused by pW1a below
                pXT_es.close()
                # right-side pool: its lifetime (mid-attn1(b1) era through
                # attn2(b0)) straddles the left-side pool stack boundaries
                pW1a = pW1a_es.enter_context(
                    tc.tile_pool(name="pW1a", bufs=1, side="right"))

                wo1 = pW1a.tile([128, KD * 1024], BF16, tag="wo1")
                wq2 = pW1a.tile([128, KD * KD * 128], BF16, tag="wq2")
                bq2 = pW1a.tile([128, KD], F32, tag="bq2")
                outT1_0 = pW1a.tile([128, KD * TB], BF16, tag="outT1_0")
                at1_0 = pW1a.tile([128, KD * TB], BF16, tag="at1_0")
                xow0 = pW1a.tile([128, 2 * 1024], F32, tag="xow0")
                # at1(b0) load on the gpsimd ring, right behind a2a1(b0)
                nc.gpsimd.dma_start(
                    out=at1_0.rearrange("p (dc s) -> p dc s", dc=KD),
                    in_=a11o[0].rearrange("(dc p) s -> p dc s", p=128))

                def wload():
                    # W-phase weight prefetch; emitted on the sync ring
                    # AFTER the encTh1 chunks so it can't delay them
                    nc.sync.dma_start(out=wo1[:], in_=wo1_d[:])
                    nc.sync.dma_start(out=wq2[:], in_=wq2_d[:])
                    nc.sync.dma_start(out=bq2[:], in_=bq2_d[:])
                    nc.sync.dma_start(
                        out=xow0.rearrange("p (m e) -> p m e", m=2),
                        in_=xown_d[0:TB].rearrange("(m p) e -> p m e",
                                                   p=128))

                qt2_box = [{}, {}]

                def finish_q2(b):
                    qt2 = qt2_box[b]["t"]
                    nc.gpsimd.dma_start(
                        out=a1qi[b].rearrange("(j p) s -> p j s", p=128),
                        in_=qt2.rearrange("p (j s) -> p j s", j=KD))
                    a2a(a1qi[b], a1qo[b])
                    nc.gpsimd.dma_start(
                        out=QT2[b].rearrange("p (i s) -> p i s", i=NCORES),
                        in_=a1qo[b].rearrange("(i p) s -> p i s", p=128))

                # vT2 shares vT1's slot: vT1 dies once vaug1[1] is built
                # (a filler of attn1(b0), emitted above)
                vT2 = pA.tile([128, SB], BF16, tag="vTs", name="vT2")

                # attn1(b1) fillers: cross K/V prep, then W1(b0) + q2(b0).
                # The encT b1 half-load is itself a filler (its slot-reuse
                # waits for all b0 readers, which precede it in the list).
                ench = {0: encTh0}

                def load_ench1():
                    ench[1] = pA.tile([128, KD * SB], BF16, tag="encTh",
                                      name="encTh1")
                    load_half(ench[1], encTd, 1)

                # filler order: cross-prep for b0 first, then W1(b0)+q2(b0)
                # EARLY so a2aq(b0) is triggered mid-attention (its flight
                # overlaps the attn1(b1) tail), then the b1 cross-prep
                f_a1b1 = []
                for jj in range(4):
                    f_a1b1.append(lambda jj=jj: proj_tile(
                        vT2, wv2, bv2, ench[0], jj))
                for jj in range(4):
                    f_a1b1.append(lambda jj=jj: proj_tile(
                        KT2[0], wk2, bk2, ench[0], jj))

                def ench1_and_wload():
                    load_ench1()
                    wload()
                f_a1b1.append(ench1_and_wload)
                f_a1b1.append(lambda: vaug_ones(vaug2[0], key_mask_col=0))
                for c0 in range(0, NBC, 4):
                    def fv2(c0=c0):
                        for c in range(c0, c0 + 4):
                            vaug_chunk_tr(vT2, vaug2[0], c, key_mask_col=0)
                    f_a1b1.append(fv2)
                f_a1b1 += wo_ln_closures(
                    p12s, None, at1_0, wo1,
                    resid_of=lambda m: xow0.rearrange(
                        "p (m e) -> p m e", m=2)[:, m],
                    outm_of=lambda m: out1v[:, m],
                    outT_sb=outT1_0, post=None)
                f_a1b1 += q2_closures(pW1a, wq2, bq2, outT1_0, qt2_box[0])
                f_a1b1.append(lambda: finish_q2(0))
                for jj in range(4):
                    f_a1b1.append(lambda jj=jj: proj_tile(
                        vT2, wv2, bv2, ench[1], jj))
                f_a1b1.append(lambda: vaug_ones(vaug2[1], key_mask_col=NBC))
                for c0 in range(0, NBC, 4):
                    def fv3(c0=c0):
                        for c in range(c0, c0 + 4):
                            vaug_chunk_tr(vT2, vaug2[1], c, key_mask_col=NBC)
                    f_a1b1.append(fv3)
                for jj in range(4):
                    f_a1b1.append(lambda jj=jj: proj_tile(
                        KT2[1], wk2, bk2, ench[1], jj))

                with tc.tile_pool(name="ps_s2", bufs=1, space="PSUM") as ps_s, \
                     tc.tile_pool(name="ps_o2", bufs=1, space="PSUM") as ps_o, \
                     tc.tile_pool(name="pt2", bufs=2) as ptp:
                    attention((ps_s, ps_o, ptp, p12s),
                              QT[1], KT[1], vaug1[1], stage1[1], self_blocks,
                              fillers=f_a1b1, filler_delay=0)
                stage_to_a2a(stage1[1], a11i[1])
            # pA closed

            # ============ phase 3: a2a1(b1) + attn2(b0) =====================
            # (a2aq(b0) was already triggered mid-attn1(b1); a2a1(b1) is
            # only needed by the W1(b1) fillers ~12 units into attn2(b0))
            a2a(a11i[1], a11o[1])

            with tc.tile_pool(name="pW1b", bufs=1) as pW1b, \
                 tc.tile_pool(name="pW1s", bufs=2) as pW1s:
                bo2 = pW1b.tile([128, 1024], F32, tag="bo2")
                nc.sync.dma_start(out=bo2[:], in_=bo2_d[:])
                outT1_1 = pW1b.tile([128, KD * TB], BF16, tag="outT1_1")
                at1_1 = pW1b.tile([128, KD * TB], BF16, tag="at1_1")
                xow1 = pW1b.tile([128, 2 * 1024], F32, tag="xow1")
                nc.sync.dma_start(
                    out=xow1.rearrange("p (m e) -> p m e", m=2),
                    in_=xown_d[TB:2 * TB].rearrange("(m p) e -> p m e",
                                                    p=128))
                at1_loaded = [False]

                def load_at1_1():
                    # gpsimd ring: rides right behind a2a1(b1)
                    if not at1_loaded[0]:
                        at1_loaded[0] = True
                        nc.gpsimd.dma_start(
                            out=at1_1.rearrange("p (dc s) -> p dc s", dc=KD),
                            in_=a11o[1].rearrange("(dc p) s -> p dc s",
                                                  p=128))

                def post_bias1(b):
                    # fold bo2 into out1 (after outT1 transpose, before wo2)
                    for m in range(2):
                        mm = 2 * b + m
                        nc.vector.tensor_tensor(
                            out1v[:, mm], out1v[:, mm], bo2[:], op=ADD)

                post_bias1(0)

                # attn2(b0) with W1(b1) + q2(b1) as fillers
                f_a2b0 = wo_ln_closures(
                    pW1s, load_at1_1, at1_1, wo1,
                    resid_of=lambda m: xow1.rearrange(
                        "p (m e) -> p m e", m=2)[:, m],
                    outm_of=lambda m: out1v[:, 2 + m],
                    outT_sb=outT1_1, post=lambda: post_bias1(1))
                f_a2b0 += q2_closures(pW1a, wq2, bq2, outT1_1, qt2_box[1])
                f_a2b0.append(lambda: finish_q2(1))

                with tc.tile_pool(name="pX", bufs=1) as pX, \
                     tc.tile_pool(name="ps_s3", bufs=1, space="PSUM") as ps_s, \
                     tc.tile_pool(name="ps_o3", bufs=1, space="PSUM") as ps_o, \
                     tc.tile_pool(name="pt3", bufs=2) as ptp:
                    stage2_b0 = pX.tile([64, HPC * SB], BF16, tag="stage2")
                    attention((ps_s, ps_o, ptp, pW1s),
                              QT2[0], KT2[0], vaug2[0], stage2_b0,
                              CROSS_BLOCKS, fillers=f_a2b0, filler_delay=12)
                    stage_to_a2a(stage2_b0, a12i[0])
                a2a(a12i[0], a12o[0])
            # pW1b closed
            pW1a_es.close()

            # ============ phase 4: attn2(b1) + wo2/FFN-w1(b0) fillers =======
            with tc.tile_pool(name="p78", bufs=1) as p78, \
                 tc.tile_pool(name="p78s", bufs=2) as p78s, \
                 tc.tile_pool(name="w1str", bufs=3) as w1s_pool, \
                 tc.tile_pool(name="w2str", bufs=3) as w2s_pool:
                out2 = p78.tile([128, 4 * 1024], F32, tag="out2")
                out2v = out2.rearrange("p (m e) -> p m e", m=4)
                wo2 = p78.tile([128, KD * 1024], BF16, tag="wo2")
                nc.sync.dma_start(out=wo2[:], in_=wo2_d[:])
                b1f = p78.tile([128, FC], F32, tag="b1f")
                nc.sync.dma_start(out=b1f[:], in_=b1f_d[:])
                b2 = p78.tile([128, 1024], F32, tag="b2")
                nc.sync.dma_start(out=b2[:], in_=b2_d[:])

                at2 = [p78.tile([128, KD * TB], BF16, tag=f"at2{b}",
                                name=f"at2{b}") for b in range(B)]
                at2_loaded = [False, False]

                def load_at2(b):
                    # gpsimd ring: rides right behind the a2a it waits on
                    if not at2_loaded[b]:
                        at2_loaded[b] = True
                        nc.gpsimd.dma_start(
                            out=at2[b].rearrange("p (dc s) -> p dc s", dc=KD),
                            in_=a12o[b].rearrange("(dc p) s -> p dc s",
                                                  p=128))

                outT2 = [p78.tile([128, KD * TB], BF16, tag=f"outT2{b}",
                                  name=f"outT2{b}") for b in range(B)]
                hT_box = [{}, {}]

                def post_bias2(b):
                    # fold b2 into out2 (after outT2 transpose, before FFN w2)
                    for m in range(2):
                        mm = 2 * b + m
                        nc.vector.tensor_tensor(
                            out2v[:, mm], out2v[:, mm], b2[:], op=ADD)

                f_a2b1 = wo_ln_closures(
                    p78s, lambda: load_at2(0), at2[0], wo2,
                    resid_of=lambda m: out1v[:, m],
                    outm_of=lambda m: out2v[:, m],
                    outT_sb=outT2[0], post=lambda: post_bias2(0))
                f_a2b1 += ffn_w1_closures(p78, w1s_pool, b1f, outT2[0],
                                          hT_box[0], tag="hT")

                with tc.tile_pool(name="pX2", bufs=1) as pX2, \
                     tc.tile_pool(name="ps_s4", bufs=1, space="PSUM") as ps_s, \
                     tc.tile_pool(name="ps_o4", bufs=1, space="PSUM") as ps_o, \
                     tc.tile_pool(name="pt4", bufs=2) as ptp:
                    stage2_b1 = pX2.tile([64, HPC * SB], BF16, tag="stage2b")
                    attention((ps_s, ps_o, ptp, p78s),
                              QT2[1], KT2[1], vaug2[1], stage2_b1,
                              CROSS_BLOCKS, fillers=f_a2b1, filler_delay=8)
                    stage_to_a2a(stage2_b1, a12i[1])
                a2a(a12i[1], a12o[1])

                # ============ phase 5: FFN-w2+LN3(b0); then all of b1 =======
                def ffn_w2_ln3(b):
                    hT = hT_box[b]["t"]
                    with tc.tile_pool(name=f"ps_f{b}", bufs=1,
                                      space="PSUM") as ps_f:
                        py = {(m, eh): ps_f.tile([128, 512], F32,
                                                 tag=f"py{m}{eh}",
                                                 name=f"py{m}{eh}")
                              for m in range(2) for eh in range(2)}
                        for fc in range(FC):
                            w2t = w2s_pool.tile([128, 1024], BF16, tag="w2s")
                            nc.sync.dma_start(
                                out=w2t[:],
                                in_=w2_d[:, 1024 * fc:1024 * (fc + 1)])
                            for m in range(2):
                                for eh in range(2):
                                    nc.tensor.matmul(
                                        py[(m, eh)][:],
                                        lhsT=hT[:, TB * fc + 128 * m:
                                                TB * fc + 128 * m + 128],
                                        rhs=w2t[:, 512 * eh:512 * (eh + 1)],
                                        start=(fc == 0), stop=(fc == FC - 1))
                        pres = {}
                        for m in range(2):
                            mm = 2 * b + m
                            pre = p78s.tile([128, 1024], F32, tag="pref",
                                            name=f"pref{m}")
                            for eh in range(2):
                                nc.vector.tensor_tensor(
                                    pre[:, 512 * eh:512 * (eh + 1)],
                                    py[(m, eh)][:],
                                    out2v[:, mm, 512 * eh:512 * (eh + 1)],
                                    op=ADD)
                            pres[m] = pre
                        for m in range(2):
                            outf = p78s.tile([128, 1024], F32, tag="outf",
                                             name=f"outf{m}")
                            ln_inplace(pres[m], outf)
                            nc.sync.dma_start(
                                out=out_d[256 * b + 128 * m:
                                          256 * b + 128 * m + 128, :],
                                in_=outf[:])

                ffn_w2_ln3(0)

                # b1 drain: wo2(b1) + LN2(b1) + FFN(b1)
                w2cl = wo_ln_closures(
                    p78s, lambda: load_at2(1), at2[1], wo2,
                    resid_of=lambda m: out1v[:, 2 + m],
                    outm_of=lambda m: out2v[:, 2 + m],
                    outT_sb=outT2[1], post=lambda: post_bias2(1))
                for f in w2cl:
                    f()
                for f in ffn_w1_closures(p78, w1s_pool, b1f, outT2[1],
                                         hT_box[1], tag="hT"):
                    f()
                ffn_w2_ln3(1)

    nc.compile()
    return nc


def _to_bf(a):
    return np.ascontiguousarray(np.asarray(a, np.float32).astype(BF))


def _rechunk_k(w):
    """[K*128, M] -> [128, K*M] with col k*M + m = w[k*128 + p, m]."""
    K = w.shape[0] // 128
    M = w.shape[1]
    return np.ascontiguousarray(
        w.reshape(K, 128, M).transpose(1, 0, 2).reshape(128, K * M))


def _analyze_self_mask(mask):
    """mask [S, S] (1 = disallowed), orientation [q, k].

    Returns blocks dict (t, c) -> 'full' | ('tile', idx), list of unique
    multiplicative tiles [128, 512] (bf16), for a block grid over one batch.
    Blocks where everything is disallowed are omitted.
    """
    add = np.float32(-1e9) * np.asarray(mask, np.float32)
    mult = np.exp(add.T)  # [k, q] multiplicative
    blocks = {}
    tiles = []
    tile_ids = {}
    for t in range(NBT):
        for c in range(NBC):
            sub = mult[128 * c:128 * (c + 1), 512 * t:512 * (t + 1)]
            if not sub.any():
                continue
            if (sub == 1.0).all():
                blocks[(t, c)] = 'full'
                continue
            key = sub.tobytes()
            if key not in tile_ids:
                tile_ids[key] = len(tiles)
                tiles.append(sub.astype(BF))
            blocks[(t, c)] = ('tile', tile_ids[key])
    return blocks, tiles


def kernel(**inputs):
    from concourse.bass_utils import run_bass_kernel_spmd

    x = np.asarray(inputs["x"], np.float32)
    enc = np.asarray(inputs["enc_output"], np.float32)
    lam = np.asarray(inputs["look_ahead_mask"], np.float32)[0, 0]
    pad = np.asarray(inputs["padding_mask"], np.float32)  # [B,1,1,S]

    self_blocks, ctiles = _analyze_self_mask(lam)
    n_ctiles = len(ctiles)
    key = (tuple(sorted(self_blocks.items())), n_ctiles)
    if key not in _PROG_CACHE:
        _PROG_CACHE[key] = _build_program(self_blocks, n_ctiles)
    nc = _PROG_CACHE[key]

    # ---- shared (core-independent) host prep ----
    xf = x.reshape(TOK, D_MODEL)             # flattened batch-major tokens
    encf = enc.reshape(TOK, D_MODEL)
    xT = _to_bf(xf.T)                        # [1024, 4096]
    encT = _to_bf(encf.T)
    if n_ctiles:
        cmask = np.concatenate(ctiles, axis=1)
    else:
        cmask = np.zeros((128, 512), BF)
    cmask = np.ascontiguousarray(cmask)
    # cross-attn key-keep mask per enc token: [128, B*16], col b*16+c
    mb = np.exp(np.float32(-1e9) * pad[:, 0, 0, :]).reshape(B, NBC, 128)
    mb = np.ascontiguousarray(mb.transpose(2, 0, 1).reshape(128, B * NBC)
                              ).astype(np.float32)

    w1f = np.asarray(inputs["ffn_w1"], np.float32)
    # w1 stationary layout: [128, fc*1024 + k*128 + m] = w1[k*128+p, fc*128+m]
    w1r = w1f.reshape(KD, 128, FC, 128).transpose(1, 2, 0, 3)
    w1r = _to_bf(w1r.reshape(128, FC * KD * 128))
    w2r = _to_bf(_rechunk_k(np.asarray(inputs["ffn_w2"], np.float32)))
    # b1 per-partition per-chunk [128, FC]; b2/bo2 pre-broadcast [128, 1024]
    b1 = np.ascontiguousarray(
        np.asarray(inputs["ffn_b1"], np.float32).reshape(FC, 128).T)
    b2 = np.ascontiguousarray(np.broadcast_to(
        np.asarray(inputs["ffn_b2"], np.float32)[None, :], (128, 1024)))

    wo1r = _to_bf(_rechunk_k(np.asarray(inputs["mha1_wo"], np.float32)))
    wo2r = _to_bf(_rechunk_k(np.asarray(inputs["mha2_wo"], np.float32)))
    bo1 = np.asarray(inputs["mha1_bo"], np.float32)
    bo2 = np.ascontiguousarray(np.broadcast_to(
        np.asarray(inputs["mha2_bo"], np.float32)[None, :], (128, 1024)))
    ident = np.eye(128, dtype=np.float32).astype(BF)

    wq2_full = np.asarray(inputs["mha2_wq"], np.float32)
    # wq2 stationary layout: [128, j*1024 + k*128 + m] = wq2[k*128+p, j*128+m]
    wq2r = wq2_full.reshape(KD, 128, KD, 128).transpose(1, 2, 0, 3)
    wq2r = _to_bf(wq2r.reshape(128, KD * KD * 128))
    bq2 = np.asarray(inputs["mha2_bq"], np.float32).reshape(KD, 128)
    bq2 = np.ascontiguousarray(bq2.T).astype(np.float32)  # [128, KD]

    in_maps = []
    for j in range(NCORES):
        hs = slice(128 * j, 128 * (j + 1))       # this core's 2 heads' cols
        xo = np.concatenate([xf[TB * j:TB * (j + 1)],
                             xf[S + TB * j:S + TB * (j + 1)]], axis=0)
        xo = np.ascontiguousarray(xo + bo1[None, :])
        m = {
            "xT": xT, "encT": encT, "x_own": xo,
            "cmask": cmask, "mbias": mb,
            "w1": w1r, "b1": b1, "w2": w2r, "b2": b2,
            "wo1": wo1r, "wo2": wo2r, "bo2": bo2,
            "wq2": wq2r, "bq2": bq2, "ident": ident,
        }
        for pre, name in (("wq1", "mha1_wq"), ("wk1", "mha1_wk"),
                          ("wv1", "mha1_wv"), ("wk2", "mha2_wk"),
                          ("wv2", "mha2_wv")):
            w = np.asarray(inputs[name], np.float32)[:, hs]
            m[pre] = _to_bf(_rechunk_k(w))
        for pre, name in (("bq1", "mha1_bq"), ("bk1", "mha1_bk"),
                          ("bv1", "mha1_bv"), ("bk2", "mha2_bk"),
                          ("bv2", "mha2_bv")):
            bvec = np.asarray(inputs[name], np.float32)[hs]
            m[pre] = np.ascontiguousarray(bvec[:, None])
        in_maps.append(m)

    res = run_bass_kernel_spmd(nc, in_maps, list(range(NCORES)))
    out = np.empty((TOK, D_MODEL), np.float32)
    for j in range(NCORES):
        r = res.results[j]["out"]
        out[TB * j:TB * (j + 1)] = r[0:TB]
        out[S + TB * j:S + TB * (j + 1)] = r[TB:2 * TB]
    return out.reshape(B, S, D_MODEL)
